# revision 13
# baseline (speedup 1.0000x reference)
"""Trainium2 Bass kernel for nn_BAZ_Network (dense CNN + cov/eig head).

Data-parallel over 8 NeuronCores: 128 samples each.

Launch 1 (per core), software-pipelined over 64 sample-pairs:
  conv trunk as G-packed banded-weight matmuls (bf16, fp32 PSUM), with
  conv biases folded into the matmuls via a ones-row in the rhs (conv0,
  conv1).  Postprocess per (E,O) parity pair is two fused ops over a
  2-sample two-PSUM-bank 3D access pattern:
    op1 (Act):  tE = relu(psE + b)           PSUM -> SBUF bf16
    op2 (DVE):  s  = max(psO + b, tE)        = relu(max(E,O)+b), the
                 maxpool, relu, bias and bf16 cast in one instruction.
  Covariance fp32 raw moments (means on Act copy+accum, squares on Act
  Square+accum, cross-products on DVE stt+accum) are interleaved into
  the pair pipeline; the FC contraction of the conv features against
  wl0[:, :7500] runs per-block (125 accumulating matmuls of 8 cols).
  Stage stagger: conv0(p) | conv1(p-1) | conv2(p-2) | conv3 at odd p |
  FC at p%4==2, which hides the halo-DMA and PSUM-evacuation latency.
Host: branch-exact fp32 netlib-LAPACK ssyevd clone for the 3x3 eigh
  (required to reproduce jnp.linalg.eigh eigenvector signs).
Launch 2 (per core): eig-feature head: 1x1 conv (wc) + relu, remaining
  FC columns wl0[:, 7500:], bias+relu, final linear wl1.
"""

import os
import sys
import time
import numpy as np
import ml_dtypes

sys.path.insert(0, "/opt/trn_rl_repo")
os.environ["BASS_NEVER_TRACE"] = "1"

import concourse.bass as bass  # noqa: E402
import concourse.tile as tile  # noqa: E402
import concourse.mybir as mybir  # noqa: E402
from concourse import bacc  # noqa: E402
from concourse.bass_utils import run_bass_kernel_spmd  # noqa: E402

F32 = mybir.dt.float32
BF16 = mybir.dt.bfloat16
AOP = mybir.AluOpType
ACTF = mybir.ActivationFunctionType
BF = ml_dtypes.bfloat16

NCORES = 8
NS = 128          # samples per core
BN = 8            # samples per block
NBLK = NS // BN
NPAIR = NS // 2   # 64 sample-pairs, the pipeline unit
L0 = 6000

LAST_EXEC_NS = [None, None]
LAST_WALL_S = [None, None]
_CACHE = {}


# ---------------------------------------------------------------- eigh ----
# fp32 netlib-LAPACK ssyevd clone for n=3 (jobz='V', uplo='L').
# Matches jaxlib's CPU eigh (LAPACK >= 3.10 slartg) bit-closely: 0/3072
# eigenvector sign mismatches on the problem distribution.

_F = np.float32
_EPS = _F(np.finfo(np.float32).eps) * _F(0.5)
_EPS2 = _EPS * _EPS
_SAFMIN = _F(np.finfo(np.float32).tiny)


def _slapy2(x, y):
    xa, ya = abs(x), abs(y)
    w, z = max(xa, ya), min(xa, ya)
    if z == 0:
        return w
    return _F(w * _F(np.sqrt(_F(_F(1.0) + _F(_F(z / w) * _F(z / w))))))


def _sign(a, b):
    return abs(a) if b >= 0 else -abs(a)


def _slartg(f, g):
    if g == _F(0.0):
        return _F(1.0), _F(0.0), f
    if f == _F(0.0):
        return _F(0.0), _sign(_F(1.0), g), abs(g)
    d = _F(np.sqrt(_F(f * f + g * g)))
    c = _F(abs(f) / d)
    r = _sign(d, f)
    s = _F(g / r)
    return c, s, r


def _slaev2(a, b, c):
    sm = _F(a + c)
    df = _F(a - c)
    adf = abs(df)
    tb = _F(b + b)
    ab = abs(tb)
    acmx, acmn = (a, c) if abs(a) > abs(c) else (c, a)
    if adf > ab:
        t = _F(ab / adf)
        rt = _F(adf * _F(np.sqrt(_F(_F(1.0) + _F(t * t)))))
    elif adf < ab:
        t = _F(adf / ab)
        rt = _F(ab * _F(np.sqrt(_F(_F(1.0) + _F(t * t)))))
    else:
        rt = _F(ab * _F(np.sqrt(_F(2.0))))
    if sm < 0:
        rt1 = _F(_F(0.5) * _F(sm - rt))
        sgn1 = -1
        rt2 = _F(_F(_F(acmx / rt1) * acmn) - _F(_F(b / rt1) * b))
    elif sm > 0:
        rt1 = _F(_F(0.5) * _F(sm + rt))
        sgn1 = 1
        rt2 = _F(_F(_F(acmx / rt1) * acmn) - _F(_F(b / rt1) * b))
    else:
        rt1 = _F(_F(0.5) * rt)
        rt2 = _F(_F(-0.5) * rt)
        sgn1 = 1
    if df >= 0:
        cs = _F(df + rt)
        sgn2 = 1
    else:
        cs = _F(df - rt)
        sgn2 = -1
    acs = abs(cs)
    if acs > ab:
        ct = _F(-tb / cs)
        sn1 = _F(_F(1.0) / _F(np.sqrt(_F(_F(1.0) + _F(ct * ct)))))
        cs1 = _F(ct * sn1)
    else:
        if ab == 0:
            cs1, sn1 = _F(1.0), _F(0.0)
        else:
            tn = _F(-cs / tb)
            cs1 = _F(_F(1.0) / _F(np.sqrt(_F(_F(1.0) + _F(tn * tn)))))
            sn1 = _F(tn * cs1)
    if sgn1 == sgn2:
        cs1, sn1 = -sn1, cs1
    return rt1, rt2, cs1, sn1


def _ssytrd3(A):
    a00, a10, a20 = A[0, 0], A[1, 0], A[2, 0]
    a11, a21, a22 = A[1, 1], A[2, 1], A[2, 2]
    xnorm = abs(a20)
    if xnorm == _F(0.0):
        beta, v2, tau = a10, a20, _F(0.0)
    else:
        beta = -_sign(_slapy2(a10, xnorm), a10)
        tau = _F(_F(beta - a10) / beta)
        v2 = _F(a20 * _F(_F(1.0) / _F(a10 - beta)))
    e0 = beta
    if tau != _F(0.0):
        x0 = _F(_F(tau * a11) + _F(tau * _F(a21 * v2)))
        x1 = _F(_F(tau * a21) + _F(_F(tau * v2) * a22))
        sdot = _F(_F(x0 * _F(1.0)) + _F(x1 * v2))
        alpha = _F(_F(_F(-0.5) * tau) * sdot)
        w0 = _F(x0 + _F(alpha * _F(1.0)))
        w1 = _F(x1 + _F(alpha * v2))
        t1, t2 = -w0, _F(-1.0)
        a11 = _F(_F(a11 + _F(_F(1.0) * t1)) + _F(w0 * t2))
        a21 = _F(_F(a21 + _F(v2 * t1)) + _F(w1 * t2))
        t1b, t2b = -w1, -v2
        a22 = _F(_F(a22 + _F(v2 * t1b)) + _F(w1 * t2b))
    d = np.array([a00, a11, a22], np.float32)
    e = np.array([e0, a21, 0.0], np.float32)
    return d, e, v2, tau


def _ssteqr3(d, e):
    n = 3
    Z = np.eye(3, dtype=np.float32)
    wc = np.zeros(2, np.float32)
    ws = np.zeros(2, np.float32)
    nmaxit, jtot = 90, 0

    def lasr_b(l, m):
        for j in range(m - 1, l - 1, -1):
            c, s = wc[j - 1], ws[j - 1]
            if c != _F(1.0) or s != _F(0.0):
                for i in range(3):
                    t = Z[i, j]
                    Z[i, j] = _F(_F(c * t) - _F(s * Z[i, j - 1]))
                    Z[i, j - 1] = _F(_F(s * t) + _F(c * Z[i, j - 1]))

    def lasr_f(m, l):
        for j in range(m, l):
            c, s = wc[j - 1], ws[j - 1]
            if c != _F(1.0) or s != _F(0.0):
                for i in range(3):
                    t = Z[i, j]
                    Z[i, j] = _F(_F(c * t) - _F(s * Z[i, j - 1]))
                    Z[i, j - 1] = _F(_F(s * t) + _F(c * Z[i, j - 1]))

    l1 = 1
    while True:
        if l1 > n:
            break
        if l1 > 1:
            e[l1 - 2] = _F(0.0)
        m = n
        for mm in range(l1, n):
            tst = abs(e[mm - 1])
            if tst == _F(0.0):
                m = mm
                break
            if tst <= _F(_F(_F(np.sqrt(abs(d[mm - 1]))) *
                            _F(np.sqrt(abs(d[mm])))) * _EPS):
                e[mm - 1] = _F(0.0)
                m = mm
                break
        l = l1
        lend = m
        l1 = m + 1
        if lend == l:
            continue
        if abs(d[lend - 1]) < abs(d[l - 1]):
            lend, l = l, lend
        if lend > l:
            while True:  # QL
                m = lend
                if l != lend:
                    for mm in range(l, lend):
                        tst = _F(abs(e[mm - 1]) * abs(e[mm - 1]))
                        if tst <= _F(_F(_F(_EPS2 * abs(d[mm - 1])) *
                                        abs(d[mm])) + _SAFMIN):
                            m = mm
                            break
                if m < lend:
                    e[m - 1] = _F(0.0)
                p = d[l - 1]
                if m == l:
                    d[l - 1] = p
                    l += 1
                    if l <= lend:
                        continue
                    break
                if m == l + 1:
                    rt1, rt2, c, s = _slaev2(d[l - 1], e[l - 1], d[l])
                    wc[l - 1] = c
                    ws[l - 1] = s
                    lasr_b(l, l + 1)
                    d[l - 1] = rt1
                    d[l] = rt2
                    e[l - 1] = _F(0.0)
                    l += 2
                    if l <= lend:
                        continue
                    break
                if jtot == nmaxit:
                    break
                jtot += 1
                g = _F(_F(d[l] - p) / _F(_F(2.0) * e[l - 1]))
                r = _slapy2(g, _F(1.0))
                g = _F(_F(d[m - 1] - p) + _F(e[l - 1] / _F(g + _sign(r, g))))
                s = _F(1.0)
                c = _F(1.0)
                p = _F(0.0)
                for i in range(m - 1, l - 1, -1):
                    f = _F(s * e[i - 1])
                    b = _F(c * e[i - 1])
                    c, s, r = _slartg(g, f)
                    if i != m - 1:
                        e[i] = r
                    g = _F(d[i] - p)
                    r = _F(_F(_F(d[i - 1] - g) * s) + _F(_F(_F(2.0) * c) * b))
                    p = _F(s * r)
                    d[i] = _F(g + p)
                    g = _F(_F(c * r) - b)
                    wc[i - 1] = c
                    ws[i - 1] = -s
                lasr_b(l, m)
                d[l - 1] = _F(d[l - 1] - p)
                e[l - 1] = g
        else:
            while True:  # QR
                m = lend
                if l != lend:
                    for mm in range(l, lend, -1):
                        tst = _F(abs(e[mm - 2]) * abs(e[mm - 2]))
                        if tst <= _F(_F(_F(_EPS2 * abs(d[mm - 1])) *
                                        abs(d[mm - 2])) + _SAFMIN):
                            m = mm
                            break
                if m > lend:
                    e[m - 2] = _F(0.0)
                p = d[l - 1]
                if m == l:
                    d[l - 1] = p
                    l -= 1
                    if l >= lend:
                        continue
                    break
                if m == l - 1:
                    rt1, rt2, c, s = _slaev2(d[l - 2], e[l - 2], d[l - 1])
                    wc[m - 1] = c
                    ws[m - 1] = s
                    lasr_f(m, l)
                    d[l - 2] = rt1
                    d[l - 1] = rt2
                    e[l - 2] = _F(0.0)
                    l -= 2
                    if l >= lend:
                        continue
                    break
                if jtot == nmaxit:
                    break
                jtot += 1
                g = _F(_F(d[l - 2] - p) / _F(_F(2.0) * e[l - 2]))
                r = _slapy2(g, _F(1.0))
                g = _F(_F(d[m - 1] - p) + _F(e[l - 2] / _F(g + _sign(r, g))))
                s = _F(1.0)
                c = _F(1.0)
                p = _F(0.0)
                for i in range(m, l):
                    f = _F(s * e[i - 1])
                    b = _F(c * e[i - 1])
                    c, s, r = _slartg(g, f)
                    if i != m:
                        e[i - 2] = r
                    g = _F(d[i - 1] - p)
                    r = _F(_F(_F(d[i] - g) * s) + _F(_F(_F(2.0) * c) * b))
                    p = _F(s * r)
                    d[i - 1] = _F(g + p)
                    g = _F(_F(c * r) - b)
                    wc[i - 1] = c
                    ws[i - 1] = s
                lasr_f(m, l)
                d[l - 1] = _F(d[l - 1] - p)
                e[l - 2] = g
        if jtot >= nmaxit:
            break
    for ii in range(2, n + 1):
        i = ii - 1
        k = i
        p = d[i - 1]
        for j in range(ii, n + 1):
            if d[j - 1] < p:
                k = j
                p = d[j - 1]
        if k != i:
            d[k - 1] = d[i - 1]
            d[i - 1] = p
            tmp = Z[:, k - 1].copy()
            Z[:, k - 1] = Z[:, i - 1]
            Z[:, i - 1] = tmp
    return d, Z


def _eigh3_batch(covs):
    n = covs.shape[0]
    W = np.empty((n, 3), np.float32)
    V = np.empty((n, 3, 3), np.float32)
    for i in range(n):
        d, e, v2, tau = _ssytrd3(covs[i])
        w, Z = _ssteqr3(d, e)
        if tau != _F(0.0):
            for j in range(3):
                vtz = _F(Z[1, j] + _F(v2 * Z[2, j]))
                tvz = _F(tau * vtz)
                Z[1, j] = _F(Z[1, j] - tvz)
                Z[2, j] = _F(Z[2, j] - _F(v2 * tvz))
        W[i] = w
        V[i] = Z
    return W, V


# ------------------------------------------------------------- weights ----

def _prep_weights(ins):
    """Host-side packing of the model weights into device layouts.

    Strided column-pair scheme (see the layer maps below); conv0/conv1
    biases ride a ones-row in the rhs so the matmul itself adds them.
    """
    w0, w1, w2, w3 = ins["w0"], ins["w1"], ins["w2"], ins["w3"]
    b0, b1 = np.asarray(ins["b0"], np.float32), np.asarray(ins["b1"],
                                                           np.float32)

    d = {}
    # conv0: window rows (c:3, j:9): even cols l = 8q-1+j, odd l = 8q+j;
    # k = j - 2g for output slot g; row 27 = ones -> bias.
    W0 = np.zeros((28, 80), np.float32)
    for c in range(3):
        for j in range(9):
            for g in range(4):
                k = j - 2 * g
                if 0 <= k < 3:
                    for o in range(20):
                        W0[c * 9 + j, g * 20 + o] = w0[o, c, k]
    W0[27, :] = np.tile(b0, 4)
    d["W0"] = W0.astype(BF)

    def s1_rows(with_hl, with_hr):
        rows = [(g * 20, 20, g) for g in range(4)]
        if with_hl:
            rows.append((80, 20, -1))
        if with_hr:
            rows.append((100, 20, 4))
        return rows

    def mk(w, blocks, Ghalf, parity, Cout, shift, colbase=None):
        Cin = w.shape[1]
        K = max(rb + Cin for rb, _, _ in blocks)
        if colbase is None:
            colbase = [g * Cout for g in range(Ghalf)]
        W = np.zeros((K, max(colbase) + Cout), np.float32)
        for rb, _, lrel in blocks:
            for g in range(Ghalf):
                pos = 2 * g + parity
                k = (lrel + shift) - pos + 1
                if 0 <= k < 3:
                    for ci in range(Cin):
                        W[rb + ci, colbase[g] + np.arange(Cout)] = w[:, ci, k]
        return W

    # conv1 output M-order: g0->0, g1->64, g2->96, g3->32 (C1B) so conv2's
    # boundary reads sit at legal rhs bases.
    C1B = [0, 64, 96, 32]

    # baseline-layout W1 blocks, then re-rowed for the s1 layout with the
    # ones row at 80: main [0:80], ones 80, hl [81:101], hr [101:121].
    w1e1_base = mk(w1, s1_rows(True, False), 4, 0, 32, 0, C1B)   # [100,128]
    w1e2 = mk(w1, [(rb, 20, lr + 4) for rb, _, lr in
                   s1_rows(False, False)], 4, 0, 32, 0, C1B)     # [80,128]
    w1o1 = mk(w1, s1_rows(False, False), 4, 1, 32, 0, C1B)       # [80,128]
    w1o2_blocks = ([(g * 20, 20, g + 4) for g in range(4)] +
                   [(80, 20, 1000), (100, 20, 8)])
    w1o2_base = mk(w1, w1o2_blocks, 4, 1, 32, 0, C1B)            # [120,128]
    b1t = np.tile(b1, 4)
    W1e1 = np.zeros((101, 128), np.float32)
    W1e1[0:80] = w1e1_base[0:80]
    W1e1[80] = b1t
    W1e1[81:101] = w1e1_base[80:100]
    W1o2 = np.zeros((121, 128), np.float32)
    W1o2[0:80] = w1o2_base[0:80]
    W1o2[80] = b1t
    W1o2[101:121] = w1o2_base[100:120]
    d["W1e1"] = W1e1.astype(BF)
    d["W1e2"] = w1e2.astype(BF)
    d["W1o1"] = w1o1.astype(BF)
    d["W1o2"] = W1o2.astype(BF)

    # conv2 (G=4, Ghalf=2, Cout=64): stored2 rows (g:4, o:32)->128
    s2_main = [(0, 32, 0), (64, 32, 1), (96, 32, 2), (32, 32, 3)]
    d["W2e1"] = mk(w2, [(0, 32, -1)], 2, 0, 64, 0).astype(BF)
    d["W2e2"] = mk(w2, s2_main, 2, 0, 64, 0).astype(BF)
    d["W2o1"] = mk(w2, s2_main, 2, 1, 64, 0).astype(BF)
    d["W2o2"] = mk(w2, [(0, 32, 4)], 2, 1, 64, 0).astype(BF)

    # conv3 (G=12, Ghalf=6, Cout=20): stored3 rows (g:2, o:64)->128.
    # Window w covers pre-pool pos [12w, 12w+12); MM t reads s3 col 6w+t
    # (l3 = 12w + 2t - 2 + g); M = (h:6, o:20) = 120.
    for t in range(7):
        d[f"W3E{t}"] = mk(w3, [(0, 64, 2 * t - 2), (64, 64, 2 * t - 1)],
                          6, 0, 20, 0).astype(BF)
    for t in range(1, 8):
        d[f"W3O{t}"] = mk(w3, [(0, 64, 2 * t - 2), (64, 64, 2 * t - 1)],
                          6, 1, 20, 0).astype(BF)

    # fc: stored4 rows (h:6, o:20)->120, col w: feature (o, l4 = 6w + h)
    wl0 = ins["wl0"]
    WFC = np.zeros((120, 63 * 100), np.float32)
    for w in range(63):
        for h in range(6):
            l4 = 6 * w + h
            if l4 < 375:
                WFC[h * 20:(h + 1) * 20, w * 100:(w + 1) * 100] = \
                    wl0[:, np.arange(20)[:, None] * 375 + l4].T.reshape(
                        20, 100)
    d["WFC"] = WFC.astype(BF)

    d["B2"] = np.tile(ins["b2"], 2).astype(np.float32)[:, None]   # [128]
    d["B3"] = np.tile(ins["b3"], 6).astype(np.float32)[:, None]   # [120]
    # launch 2
    d["wcT"] = ins["wc"][:, :, 0].T.astype(np.float32).copy()      # [3, 20]
    d["bc"] = ins["bc"].astype(np.float32)[:, None]                # [20, 1]
    w0b = np.zeros((7, 20, 100), np.float32)
    for t in range(7):
        for o in range(20):
            w0b[t, o] = ins["wl0"][:, 7500 + o * 7 + t]
    d["w0bT"] = w0b
    d["bl0"] = ins["bl0"].astype(np.float32)[:, None]              # [100, 1]
    d["wl1T"] = ins["wl1"].T.astype(np.float32).copy()             # [100, 2]
    d["bl1"] = ins["bl1"].astype(np.float32)[:, None]              # [2, 1]
    return d


# ------------------------------------------------------------- launch 1 ----

def _build_launch1():
    nc = bacc.Bacc("TRN2", target_bir_lowering=False, debug=False,
                   num_devices=NCORES)
    dram = {}
    for nm, shape, dt in [
        ("x_winE", [28, NS, 750], BF16), ("x_winO", [28, NS, 750], BF16),
        ("x_f32", [NS, 3, L0], F32),
        ("W0", [28, 80], BF16),
        ("W1e1", [101, 128], BF16), ("W1e2", [80, 128], BF16),
        ("W1o1", [80, 128], BF16), ("W1o2", [121, 128], BF16),
        ("W2e1", [32, 128], BF16), ("W2e2", [128, 128], BF16),
        ("W2o1", [128, 128], BF16), ("W2o2", [32, 128], BF16),
    ] + [(f"W3E{t}", [128, 120], BF16) for t in range(7)] + \
        [(f"W3O{t}", [128, 120], BF16) for t in range(1, 8)] + [
        ("WFC", [120, 6300], BF16),
        ("B2", [128, 1], F32), ("B3", [120, 1], F32),
        ("INIT1", [1, BN, 750], BF16), ("INITZ", [128, BN, 1], BF16),
    ]:
        dram[nm] = nc.dram_tensor(nm, shape, dt, kind="ExternalInput").ap()
    out_p0 = nc.dram_tensor("partial0", [100, NS], F32,
                            kind="ExternalOutput").ap()
    out_mom = nc.dram_tensor("mom", [NS, 9], F32,
                             kind="ExternalOutput").ap()

    with tile.TileContext(nc) as tc:
        with tc.tile_pool(name="wpool", bufs=1) as wp, \
             tc.tile_pool(name="covp", bufs=1) as cvp, \
             tc.tile_pool(name="scrp", bufs=1) as scp, \
             tc.tile_pool(name="xw", bufs=2) as xwp, \
             tc.tile_pool(name="s1p", bufs=1) as s1p, \
             tc.tile_pool(name="s2p", bufs=1) as s2p, \
             tc.tile_pool(name="s3p", bufs=1) as s3p, \
             tc.tile_pool(name="s4p", bufs=1) as s4p, \
             tc.tile_pool(name="tep", bufs=4) as tep, \
             tc.tile_pool(name="ps", bufs=4, space="PSUM") as psp:

            xw_tiles = {}

            def issue_xw(b):
                if b >= NBLK or b in xw_tiles:
                    return
                n0 = b * BN
                te = xwp.tile([28, BN, 750], BF16, tag="xwE")
                nc.sync.dma_start(te[:], dram["x_winE"][:, n0:n0 + BN, :])
                to = xwp.tile([28, BN, 750], BF16, tag="xwO")
                nc.sync.dma_start(to[:], dram["x_winO"][:, n0:n0 + BN, :])
                xw_tiles[b] = (te, to)

            # SP DMA queue is FIFO: conv0's first inputs go FIRST, the big
            # covariance / FC-weight transfers ride the Activation queue.
            Ws = {}
            t = wp.tile([28, 80], BF16, name="W0", tag="W0")
            nc.sync.dma_start(t[:], dram["W0"][:])
            Ws["W0"] = t
            issue_xw(0)
            issue_xw(1)
            for nm in (["W1e1", "W1e2", "W1o1", "W1o2",
                        "W2e2", "W2o1", "W2o2"] +
                       [f"W3E{t}" for t in range(7)] +
                       [f"W3O{t}" for t in range(1, 8)]):
                t = wp.tile(list(dram[nm].shape), BF16, name=nm, tag=nm)
                nc.sync.dma_start(t[:], dram[nm][:])
                Ws[nm] = t
            # lhsT base partition must match rhs base: pad this one
            t = wp.tile([64, 128], BF16, name="W2e1", tag="W2e1")
            nc.sync.dma_start(t[32:64], dram["W2e1"][:])
            Ws["W2e1"] = t[32:64]
            B2t = wp.tile([128, 1], F32, tag="B2")
            nc.sync.dma_start(B2t[:], dram["B2"][:])
            B3t = wp.tile([120, 1], F32, tag="B3")
            nc.sync.dma_start(B3t[:], dram["B3"][:])

            momt = cvp.tile([NS, 9], F32, tag="mom")
            p0sb = cvp.tile([100, NS], F32, tag="p0sb")

            # persistent stage buffers; halo-edge and ones rows set once
            s1 = s1p.tile([121, BN, 750], BF16, tag="s1")
            s2 = s2p.tile([128, BN, 377], BF16, tag="s2")
            s3 = s3p.tile([128, BN, 380], BF16, tag="s3")
            s4 = s4p.tile([120, NS, 63], BF16, tag="s4")
            # one-time edge/ones init via DMA (engines cannot address
            # partition bases off the 0/32/64/96 grid; DMA can)
            nc.sync.dma_start(s1[80:81, :, :], dram["INIT1"][:])
            nc.sync.dma_start(s1[81:101, :, 0:1], dram["INITZ"][0:20])
            nc.sync.dma_start(s1[101:121, :, 749:750], dram["INITZ"][0:20])
            nc.sync.dma_start(s2[:, :, 0:1], dram["INITZ"][:])
            nc.sync.dma_start(s2[:, :, 376:377], dram["INITZ"][:])
            nc.sync.dma_start(s3[:, :, 0:1], dram["INITZ"][:])
            for cz in range(376, 380):
                nc.sync.dma_start(s3[:, :, cz:cz + 1], dram["INITZ"][:])

            # big transfers on the Activation HWDGE queue
            xcv = cvp.tile([NS, 3, L0], F32, tag="xcv")
            nc.scalar.dma_start(xcv[:], dram["x_f32"][:])
            t = wp.tile([120, 6300], BF16, name="WFC", tag="WFC")
            nc.scalar.dma_start(t[:], dram["WFC"][:])
            Ws["WFC"] = t

            # interleaved covariance moment ops, one per scheduled step
            cov_ops = []
            for c in range(3):
                cov_ops.append(("mean", c))
            for c in range(3):
                cov_ops.append(("sq", c))
            for k, (c, dch) in enumerate([(0, 1), (0, 2), (1, 2)]):
                cov_ops.append(("xy", k, c, dch))
            cov_sched = {7 + 6 * i: op for i, op in enumerate(cov_ops)}

            def emit_cov(op):
                scr = scp.tile([NS, L0], BF16, tag="cscr")
                if op[0] == "mean":
                    c = op[1]
                    nc.scalar.activation(scr[:], xcv[:, c, :], ACTF.Copy,
                                         accum_out=momt[:, c:c + 1])
                elif op[0] == "sq":
                    c = op[1]
                    nc.scalar.activation(scr[:], xcv[:, c, :], ACTF.Square,
                                         accum_out=momt[:, 3 + c:4 + c])
                else:
                    _, k, c, dch = op
                    nc.vector.scalar_tensor_tensor(
                        scr[:], xcv[:, c, :], 1.0, xcv[:, dch, :],
                        AOP.mult, AOP.mult,
                        accum_out=momt[:, 6 + k:7 + k])

            def conv0(p):
                blk, nb = p // 4, (p % 4) * 2
                xwE, xwO = xw_tiles[blk]
                for ch in range(2):
                    c0 = ch * 375
                    psE = psp.tile([128, 2, 512], F32, tag="ps")
                    psO = psp.tile([128, 2, 512], F32, tag="ps")
                    for i in range(2):
                        nc.tensor.matmul(psE[0:80, i, 0:375], Ws["W0"][:],
                                         xwE[:, nb + i, c0:c0 + 375],
                                         start=True, stop=True)
                    for i in range(2):
                        nc.tensor.matmul(psO[0:80, i, 0:375], Ws["W0"][:],
                                         xwO[:, nb + i, c0:c0 + 375],
                                         start=True, stop=True)
                    tE = tep.tile([128, 2, 384], BF16, tag="tE")
                    nc.scalar.activation(tE[0:80, :, 0:375],
                                         psE[0:80, :, 0:375], ACTF.Relu)
                    nc.vector.scalar_tensor_tensor(
                        s1[0:80, nb:nb + 2, c0:c0 + 375],
                        psO[0:80, :, 0:375], 0.0, tE[0:80, :, 0:375],
                        AOP.max, AOP.max)
                # per-pair halo rows for conv1
                nc.sync.dma_start(s1[81:101, nb:nb + 2, 1:750],
                                  s1[60:80, nb:nb + 2, 0:749])
                nc.sync.dma_start(s1[101:121, nb:nb + 2, 0:749],
                                  s1[0:20, nb:nb + 2, 1:750])

            def conv1(p):
                nb = (p % 4) * 2
                psE = psp.tile([128, 2, 512], F32, tag="ps")
                psO = psp.tile([128, 2, 512], F32, tag="ps")
                for i in range(2):
                    n = nb + i
                    nc.tensor.matmul(psE[0:128, i, 0:375], Ws["W1e1"][:],
                                     s1[0:101, n, 0:750:2],
                                     start=True, stop=False)
                    nc.tensor.matmul(psE[0:128, i, 0:375], Ws["W1e2"][:],
                                     s1[0:80, n, 1:750:2],
                                     start=False, stop=True)
                for i in range(2):
                    n = nb + i
                    nc.tensor.matmul(psO[0:128, i, 0:375], Ws["W1o1"][:],
                                     s1[0:80, n, 0:750:2],
                                     start=True, stop=False)
                    nc.tensor.matmul(psO[0:128, i, 0:375], Ws["W1o2"][:],
                                     s1[0:121, n, 1:750:2],
                                     start=False, stop=True)
                tE = tep.tile([128, 2, 384], BF16, tag="tE")
                nc.scalar.activation(tE[0:128, :, 0:375],
                                     psE[0:128, :, 0:375], ACTF.Relu)
                nc.vector.scalar_tensor_tensor(
                    s2[0:128, nb:nb + 2, 1:376],
                    psO[0:128, :, 0:375], 0.0, tE[0:128, :, 0:375],
                    AOP.max, AOP.max)

            def conv2(p):
                nb = (p % 4) * 2
                psE = psp.tile([128, 2, 512], F32, tag="ps")
                psO = psp.tile([128, 2, 512], F32, tag="ps")
                for i in range(2):
                    n = nb + i
                    nc.tensor.matmul(psE[0:128, i, 0:375], Ws["W2e1"],
                                     s2[32:64, n, 0:375],
                                     start=True, stop=False)
                    nc.tensor.matmul(psE[0:128, i, 0:375], Ws["W2e2"][:],
                                     s2[0:128, n, 1:376],
                                     start=False, stop=True)
                for i in range(2):
                    n = nb + i
                    nc.tensor.matmul(psO[0:128, i, 0:375], Ws["W2o1"][:],
                                     s2[0:128, n, 1:376],
                                     start=True, stop=False)
                    nc.tensor.matmul(psO[0:128, i, 0:375], Ws["W2o2"][:],
                                     s2[0:32, n, 2:377],
                                     start=False, stop=True)
                tE = tep.tile([128, 2, 384], BF16, tag="tE")
                nc.scalar.activation(tE[0:128, :, 0:375],
                                     psE[0:128, :, 0:375], ACTF.Relu,
                                     bias=B2t[:])
                nc.vector.scalar_tensor_tensor(
                    s3[0:128, nb:nb + 2, 1:376],
                    psO[0:128, :, 0:375], B2t[:], tE[0:128, :, 0:375],
                    AOP.add, AOP.max)

            def conv3(qd):
                blk, nq = qd // 2, (qd % 2) * 4
                n0 = blk * BN + nq
                psE = psp.tile([128, 2, 512], F32, tag="ps")
                psO = psp.tile([128, 2, 512], F32, tag="ps")
                for t in range(7):
                    nc.tensor.matmul(
                        psE[0:120, 0, 0:252], Ws[f"W3E{t}"][:],
                        s3[0:128, nq:nq + 4, t:t + 373:6],
                        start=(t == 0), stop=(t == 6))
                for t in range(1, 8):
                    nc.tensor.matmul(
                        psO[0:120, 0, 0:252], Ws[f"W3O{t}"][:],
                        s3[0:128, nq:nq + 4, t:t + 373:6],
                        start=(t == 1), stop=(t == 7))
                tE = tep.tile([128, 512], BF16, tag="tE3")
                nc.scalar.activation(tE[0:120, 0:252], psE[0:120, 0, 0:252],
                                     ACTF.Relu, bias=B3t[:])
                nc.vector.scalar_tensor_tensor(
                    s4[0:120, n0:n0 + 4, 0:63],
                    psO[0:120, 0, 0:252].rearrange("p (n l) -> p n l", n=4),
                    B3t[:],
                    tE[0:120, 0:252].rearrange("p (n l) -> p n l", n=4),
                    AOP.add, AOP.max)

            def fc(half):
                c0 = half * 64
                psfc = psp.tile([128, 2, 512], F32, tag="ps")
                for w in range(63):
                    nc.tensor.matmul(
                        psfc[0:100, 0, 0:64],
                        Ws["WFC"][:, w * 100:(w + 1) * 100],
                        s4[:, c0:c0 + 64, w], start=(w == 0), stop=(w == 62))
                nc.scalar.copy(p0sb[:, c0:c0 + 64], psfc[0:100, 0, 0:64])

            # pipeline: conv0(p) | conv1(p-1) | conv2(p-2) |
            #           conv3((p-3)/2 @ odd p) | fc halves at p=35/66
            for p in range(67):
                if p < NPAIR:
                    if p % 4 == 0:
                        issue_xw(p // 4 + 2)
                    conv0(p)
                if 0 <= p - 1 < NPAIR:
                    conv1(p - 1)
                if 0 <= p - 2 < NPAIR:
                    conv2(p - 2)
                if p % 2 == 1 and 0 <= (p - 3) // 2 < 32:
                    conv3((p - 3) // 2)
                if p == 35 or p == 66:
                    fc(0 if p == 35 else 1)
                if p in cov_sched:
                    emit_cov(cov_sched[p])

            nc.sync.dma_start(out_p0[:], p0sb[:])
            nc.sync.dma_start(out_mom[:], momt[:])

    nc.compile()
    return nc


# ------------------------------------------------------------- launch 2 ----

def _build_launch2():
    nc = bacc.Bacc("TRN2", target_bir_lowering=False, debug=False,
                   num_devices=NCORES)
    dr = {}
    for nm, shape in [("featsT", [3, 7 * NS]), ("p0T", [100, NS]),
                      ("wcT", [3, 20]), ("bc", [20, 1]),
                      ("w0bT", [7, 20, 100]), ("bl0", [100, 1]),
                      ("wl1T", [100, 2]), ("bl1", [2, 1])]:
        dr[nm] = nc.dram_tensor(nm, shape, F32, kind="ExternalInput").ap()
    out2 = nc.dram_tensor("out2", [2, NS], F32, kind="ExternalOutput").ap()

    with tile.TileContext(nc) as tc:
        with tc.tile_pool(name="w2p", bufs=1) as wp, \
             tc.tile_pool(name="ps2", bufs=2, space="PSUM") as psp:
            fT = wp.tile([3, 7 * NS], F32, tag="fT")
            nc.sync.dma_start(fT[:], dr["featsT"][:])
            p0T = wp.tile([100, NS], F32, tag="p0T")
            nc.sync.dma_start(p0T[:], dr["p0T"][:])
            wcT = wp.tile([3, 20], F32, tag="wcT")
            nc.sync.dma_start(wcT[:], dr["wcT"][:])
            bc = wp.tile([20, 1], F32, tag="bc")
            nc.sync.dma_start(bc[:], dr["bc"][:])
            w0bT = [wp.tile([20, 100], F32, name=f"w0bT{t}", tag=f"w0bT{t}")
                    for t in range(7)]
            for t in range(7):
                nc.sync.dma_start(w0bT[t][:], dr["w0bT"][t])
            bl0 = wp.tile([100, 1], F32, tag="bl0")
            nc.sync.dma_start(bl0[:], dr["bl0"][:])
            wl1T = wp.tile([100, 2], F32, tag="wl1T")
            nc.sync.dma_start(wl1T[:], dr["wl1T"][:])
            bl1 = wp.tile([2, 1], F32, tag="bl1")
            nc.sync.dma_start(bl1[:], dr["bl1"][:])

            # h1 = relu(wc @ feats + bc): [20, (t, n)]
            h1 = wp.tile([20, 7 * NS], F32, tag="h1")
            for half in range(2):
                c0 = half * 448
                ps = psp.tile([32, 448], F32, tag="ph")
                nc.tensor.matmul(ps[0:20, :], wcT[:], fT[:, c0:c0 + 448],
                                 start=True, stop=True)
                nc.scalar.activation(h1[:, c0:c0 + 448], ps[0:20, :],
                                     ACTF.Relu, bias=bc[:])
            # z = relu(p0 + sum_t w0b_t.T @ h1_t + bl0)
            psz = psp.tile([100, NS], F32, tag="pz")
            for t in range(7):
                nc.tensor.matmul(psz[:], w0bT[t][:],
                                 h1[:, t * NS:(t + 1) * NS],
                                 start=(t == 0), stop=(t == 6))
            z = wp.tile([100, NS], F32, tag="z")
            nc.vector.scalar_tensor_tensor(z[:], psz[:], bl0[:], p0T[:],
                                           AOP.add, AOP.add)
            nc.vector.tensor_scalar_max(z[:], z[:], 0.0)
            pso = psp.tile([32, NS], F32, tag="po")
            nc.tensor.matmul(pso[0:2, :], wl1T[:], z[:],
                             start=True, stop=True)
            osb = wp.tile([2, NS], F32, tag="osb")
            nc.vector.tensor_scalar(osb[:], pso[0:2, :], bl1[:], None,
                                    AOP.add)
            nc.sync.dma_start(out2[:], osb[:])

    nc.compile()
    return nc


# --------------------------------------------------------------- kernel ----

def kernel(**inputs):
    ins = {k: np.asarray(v) for k, v in inputs.items()}
    x = ins["x"].astype(np.float32)

    if "l1" not in _CACHE:
        _CACHE["l1"] = _build_launch1()
    if "l2" not in _CACHE:
        _CACHE["l2"] = _build_launch2()
    w = _prep_weights(ins)

    xbf = x.astype(BF)
    xwE = np.zeros((28, x.shape[0], 750), BF)
    xwO = np.zeros((28, x.shape[0], 750), BF)
    for c in range(3):
        for j in range(9):
            # even cols: l = 8q - 1 + j ; odd cols: l = 8q + j
            if j == 0:
                xwE[c * 9 + 0, :, 1:750] = xbf[:, c, 7:5992:8]
            else:
                xwE[c * 9 + j] = xbf[:, c, j - 1::8]
            if j == 8:
                xwO[c * 9 + 8, :, 0:749] = xbf[:, c, 8:6000:8]
            else:
                xwO[c * 9 + j] = xbf[:, c, j::8]
    xwE[27] = 1.0
    xwO[27] = 1.0
    shards = [x[i * NS:(i + 1) * NS] for i in range(NCORES)]
    in1 = []
    for i, sh in enumerate(shards):
        sl = slice(i * NS, (i + 1) * NS)
        m = {"x_winE": np.ascontiguousarray(xwE[:, sl]),
             "x_winO": np.ascontiguousarray(xwO[:, sl]),
             "x_f32": sh}
        for nm in (["W0", "W1e1", "W1e2", "W1o1", "W1o2",
                    "W2e1", "W2e2", "W2o1", "W2o2", "WFC", "B2", "B3"] +
                   [f"W3E{t}" for t in range(7)] +
                   [f"W3O{t}" for t in range(1, 8)]):
            m[nm] = w[nm]
        m["INIT1"] = np.ones((1, BN, 750), BF)
        m["INITZ"] = np.zeros((128, BN, 1), BF)
        in1.append(m)
    t0 = time.time()
    res1 = run_bass_kernel_spmd(_CACHE["l1"], in1, list(range(NCORES)))
    LAST_EXEC_NS[0] = res1.exec_time_ns
    LAST_WALL_S[0] = time.time() - t0

    mom = np.concatenate([res1.results[i]["mom"] for i in range(NCORES)], 0)
    partial0 = np.concatenate(
        [res1.results[i]["partial0"].T for i in range(NCORES)], 0)

    # host: cov assembly (fp32) + LAPACK-clone eigh + global normalizers
    Sx = mom[:, 0:3].astype(np.float32)
    Sxx = mom[:, 3:6].astype(np.float32)
    Sxy = mom[:, 6:9].astype(np.float32)
    L = np.float32(L0)
    cov = np.empty((x.shape[0], 3, 3), np.float32)
    for idx, (c, dch) in enumerate([(0, 1), (0, 2), (1, 2)]):
        v = (Sxy[:, idx] - Sx[:, c] * Sx[:, dch] / L) / np.float32(L0 - 1)
        cov[:, c, dch] = v
        cov[:, dch, c] = v
    for c in range(3):
        cov[:, c, c] = (Sxx[:, c] - Sx[:, c] * Sx[:, c] / L) / np.float32(
            L0 - 1)
    vals, vecs = _eigh3_batch(cov)
    covn = cov / np.abs(cov).max()
    valsn = (vals / vals.max())[..., None]
    feats = np.concatenate([covn, valsn, vecs], axis=-1).astype(np.float32)

    in2 = []
    for i in range(NCORES):
        sl = slice(i * NS, (i + 1) * NS)
        m = {"featsT": np.ascontiguousarray(
                 feats[sl].transpose(1, 2, 0).reshape(3, 7 * NS)),
             "p0T": partial0[sl].T.copy(),
             "wcT": w["wcT"], "bc": w["bc"], "w0bT": w["w0bT"],
             "bl0": w["bl0"], "wl1T": w["wl1T"], "bl1": w["bl1"]}
        in2.append(m)
    t0 = time.time()
    res2 = run_bass_kernel_spmd(_CACHE["l2"], in2, list(range(NCORES)))
    LAST_EXEC_NS[1] = res2.exec_time_ns
    LAST_WALL_S[1] = time.time() - t0

    out = np.concatenate([res2.results[i]["out2"].T for i in range(NCORES)],
                         0).astype(np.float32)
    return (out[:, 0:1], out[:, 1:2])


# revision 15
# speedup vs baseline: 1.0436x; 1.0436x over previous
"""Trainium2 Bass kernel for nn_BAZ_Network (dense CNN + cov/eig head).

Data-parallel over 8 NeuronCores: 128 samples each.

Launch 1 (per core), software-pipelined over 64 sample-pairs:
  conv trunk as G-packed banded-weight matmuls (bf16, fp32 PSUM), with
  conv biases folded into the matmuls via a ones-row in the rhs (conv0,
  conv1).  Postprocess per (E,O) parity pair is two fused ops over a
  2-sample two-PSUM-bank 3D access pattern:
    op1 (Act):  tE = relu(psE + b)           PSUM -> SBUF bf16
    op2 (DVE):  s  = max(psO + b, tE)        = relu(max(E,O)+b), the
                 maxpool, relu, bias and bf16 cast in one instruction.
  Covariance fp32 raw moments (means on Act copy+accum, squares on Act
  Square+accum, cross-products on DVE stt+accum) are interleaved into
  the pair pipeline; the FC contraction of the conv features against
  wl0[:, :7500] runs per-block (125 accumulating matmuls of 8 cols).
  Stage stagger: conv0(p) | conv1(p-1) | conv2(p-2) | conv3 at odd p |
  FC at p%4==2, which hides the halo-DMA and PSUM-evacuation latency.
Host: branch-exact fp32 netlib-LAPACK ssyevd clone for the 3x3 eigh
  (required to reproduce jnp.linalg.eigh eigenvector signs).
Launch 2 (per core): eig-feature head: 1x1 conv (wc) + relu, remaining
  FC columns wl0[:, 7500:], bias+relu, final linear wl1.
"""

import os
import sys
import time
import numpy as np
import ml_dtypes

sys.path.insert(0, "/opt/trn_rl_repo")
os.environ["BASS_NEVER_TRACE"] = "1"

import concourse.bass as bass  # noqa: E402
import concourse.tile as tile  # noqa: E402
import concourse.mybir as mybir  # noqa: E402
from concourse import bacc  # noqa: E402
from concourse.bass_utils import run_bass_kernel_spmd  # noqa: E402

F32 = mybir.dt.float32
BF16 = mybir.dt.bfloat16
AOP = mybir.AluOpType
ACTF = mybir.ActivationFunctionType
BF = ml_dtypes.bfloat16

NCORES = 8
NS = 128          # samples per core
BN = 8            # samples per block
NBLK = NS // BN
NPAIR = NS // 2   # 64 sample-pairs, the pipeline unit
L0 = 6000

LAST_EXEC_NS = [None, None]
LAST_WALL_S = [None, None]
_CACHE = {}


# ---------------------------------------------------------------- eigh ----
# fp32 netlib-LAPACK ssyevd clone for n=3 (jobz='V', uplo='L').
# Matches jaxlib's CPU eigh (LAPACK >= 3.10 slartg) bit-closely: 0/3072
# eigenvector sign mismatches on the problem distribution.

_F = np.float32
_EPS = _F(np.finfo(np.float32).eps) * _F(0.5)
_EPS2 = _EPS * _EPS
_SAFMIN = _F(np.finfo(np.float32).tiny)


def _slapy2(x, y):
    xa, ya = abs(x), abs(y)
    w, z = max(xa, ya), min(xa, ya)
    if z == 0:
        return w
    return _F(w * _F(np.sqrt(_F(_F(1.0) + _F(_F(z / w) * _F(z / w))))))


def _sign(a, b):
    return abs(a) if b >= 0 else -abs(a)


def _slartg(f, g):
    if g == _F(0.0):
        return _F(1.0), _F(0.0), f
    if f == _F(0.0):
        return _F(0.0), _sign(_F(1.0), g), abs(g)
    d = _F(np.sqrt(_F(f * f + g * g)))
    c = _F(abs(f) / d)
    r = _sign(d, f)
    s = _F(g / r)
    return c, s, r


def _slaev2(a, b, c):
    sm = _F(a + c)
    df = _F(a - c)
    adf = abs(df)
    tb = _F(b + b)
    ab = abs(tb)
    acmx, acmn = (a, c) if abs(a) > abs(c) else (c, a)
    if adf > ab:
        t = _F(ab / adf)
        rt = _F(adf * _F(np.sqrt(_F(_F(1.0) + _F(t * t)))))
    elif adf < ab:
        t = _F(adf / ab)
        rt = _F(ab * _F(np.sqrt(_F(_F(1.0) + _F(t * t)))))
    else:
        rt = _F(ab * _F(np.sqrt(_F(2.0))))
    if sm < 0:
        rt1 = _F(_F(0.5) * _F(sm - rt))
        sgn1 = -1
        rt2 = _F(_F(_F(acmx / rt1) * acmn) - _F(_F(b / rt1) * b))
    elif sm > 0:
        rt1 = _F(_F(0.5) * _F(sm + rt))
        sgn1 = 1
        rt2 = _F(_F(_F(acmx / rt1) * acmn) - _F(_F(b / rt1) * b))
    else:
        rt1 = _F(_F(0.5) * rt)
        rt2 = _F(_F(-0.5) * rt)
        sgn1 = 1
    if df >= 0:
        cs = _F(df + rt)
        sgn2 = 1
    else:
        cs = _F(df - rt)
        sgn2 = -1
    acs = abs(cs)
    if acs > ab:
        ct = _F(-tb / cs)
        sn1 = _F(_F(1.0) / _F(np.sqrt(_F(_F(1.0) + _F(ct * ct)))))
        cs1 = _F(ct * sn1)
    else:
        if ab == 0:
            cs1, sn1 = _F(1.0), _F(0.0)
        else:
            tn = _F(-cs / tb)
            cs1 = _F(_F(1.0) / _F(np.sqrt(_F(_F(1.0) + _F(tn * tn)))))
            sn1 = _F(tn * cs1)
    if sgn1 == sgn2:
        cs1, sn1 = -sn1, cs1
    return rt1, rt2, cs1, sn1


def _ssytrd3(A):
    a00, a10, a20 = A[0, 0], A[1, 0], A[2, 0]
    a11, a21, a22 = A[1, 1], A[2, 1], A[2, 2]
    xnorm = abs(a20)
    if xnorm == _F(0.0):
        beta, v2, tau = a10, a20, _F(0.0)
    else:
        beta = -_sign(_slapy2(a10, xnorm), a10)
        tau = _F(_F(beta - a10) / beta)
        v2 = _F(a20 * _F(_F(1.0) / _F(a10 - beta)))
    e0 = beta
    if tau != _F(0.0):
        x0 = _F(_F(tau * a11) + _F(tau * _F(a21 * v2)))
        x1 = _F(_F(tau * a21) + _F(_F(tau * v2) * a22))
        sdot = _F(_F(x0 * _F(1.0)) + _F(x1 * v2))
        alpha = _F(_F(_F(-0.5) * tau) * sdot)
        w0 = _F(x0 + _F(alpha * _F(1.0)))
        w1 = _F(x1 + _F(alpha * v2))
        t1, t2 = -w0, _F(-1.0)
        a11 = _F(_F(a11 + _F(_F(1.0) * t1)) + _F(w0 * t2))
        a21 = _F(_F(a21 + _F(v2 * t1)) + _F(w1 * t2))
        t1b, t2b = -w1, -v2
        a22 = _F(_F(a22 + _F(v2 * t1b)) + _F(w1 * t2b))
    d = np.array([a00, a11, a22], np.float32)
    e = np.array([e0, a21, 0.0], np.float32)
    return d, e, v2, tau


def _ssteqr3(d, e):
    n = 3
    Z = np.eye(3, dtype=np.float32)
    wc = np.zeros(2, np.float32)
    ws = np.zeros(2, np.float32)
    nmaxit, jtot = 90, 0

    def lasr_b(l, m):
        for j in range(m - 1, l - 1, -1):
            c, s = wc[j - 1], ws[j - 1]
            if c != _F(1.0) or s != _F(0.0):
                for i in range(3):
                    t = Z[i, j]
                    Z[i, j] = _F(_F(c * t) - _F(s * Z[i, j - 1]))
                    Z[i, j - 1] = _F(_F(s * t) + _F(c * Z[i, j - 1]))

    def lasr_f(m, l):
        for j in range(m, l):
            c, s = wc[j - 1], ws[j - 1]
            if c != _F(1.0) or s != _F(0.0):
                for i in range(3):
                    t = Z[i, j]
                    Z[i, j] = _F(_F(c * t) - _F(s * Z[i, j - 1]))
                    Z[i, j - 1] = _F(_F(s * t) + _F(c * Z[i, j - 1]))

    l1 = 1
    while True:
        if l1 > n:
            break
        if l1 > 1:
            e[l1 - 2] = _F(0.0)
        m = n
        for mm in range(l1, n):
            tst = abs(e[mm - 1])
            if tst == _F(0.0):
                m = mm
                break
            if tst <= _F(_F(_F(np.sqrt(abs(d[mm - 1]))) *
                            _F(np.sqrt(abs(d[mm])))) * _EPS):
                e[mm - 1] = _F(0.0)
                m = mm
                break
        l = l1
        lend = m
        l1 = m + 1
        if lend == l:
            continue
        if abs(d[lend - 1]) < abs(d[l - 1]):
            lend, l = l, lend
        if lend > l:
            while True:  # QL
                m = lend
                if l != lend:
                    for mm in range(l, lend):
                        tst = _F(abs(e[mm - 1]) * abs(e[mm - 1]))
                        if tst <= _F(_F(_F(_EPS2 * abs(d[mm - 1])) *
                                        abs(d[mm])) + _SAFMIN):
                            m = mm
                            break
                if m < lend:
                    e[m - 1] = _F(0.0)
                p = d[l - 1]
                if m == l:
                    d[l - 1] = p
                    l += 1
                    if l <= lend:
                        continue
                    break
                if m == l + 1:
                    rt1, rt2, c, s = _slaev2(d[l - 1], e[l - 1], d[l])
                    wc[l - 1] = c
                    ws[l - 1] = s
                    lasr_b(l, l + 1)
                    d[l - 1] = rt1
                    d[l] = rt2
                    e[l - 1] = _F(0.0)
                    l += 2
                    if l <= lend:
                        continue
                    break
                if jtot == nmaxit:
                    break
                jtot += 1
                g = _F(_F(d[l] - p) / _F(_F(2.0) * e[l - 1]))
                r = _slapy2(g, _F(1.0))
                g = _F(_F(d[m - 1] - p) + _F(e[l - 1] / _F(g + _sign(r, g))))
                s = _F(1.0)
                c = _F(1.0)
                p = _F(0.0)
                for i in range(m - 1, l - 1, -1):
                    f = _F(s * e[i - 1])
                    b = _F(c * e[i - 1])
                    c, s, r = _slartg(g, f)
                    if i != m - 1:
                        e[i] = r
                    g = _F(d[i] - p)
                    r = _F(_F(_F(d[i - 1] - g) * s) + _F(_F(_F(2.0) * c) * b))
                    p = _F(s * r)
                    d[i] = _F(g + p)
                    g = _F(_F(c * r) - b)
                    wc[i - 1] = c
                    ws[i - 1] = -s
                lasr_b(l, m)
                d[l - 1] = _F(d[l - 1] - p)
                e[l - 1] = g
        else:
            while True:  # QR
                m = lend
                if l != lend:
                    for mm in range(l, lend, -1):
                        tst = _F(abs(e[mm - 2]) * abs(e[mm - 2]))
                        if tst <= _F(_F(_F(_EPS2 * abs(d[mm - 1])) *
                                        abs(d[mm - 2])) + _SAFMIN):
                            m = mm
                            break
                if m > lend:
                    e[m - 2] = _F(0.0)
                p = d[l - 1]
                if m == l:
                    d[l - 1] = p
                    l -= 1
                    if l >= lend:
                        continue
                    break
                if m == l - 1:
                    rt1, rt2, c, s = _slaev2(d[l - 2], e[l - 2], d[l - 1])
                    wc[m - 1] = c
                    ws[m - 1] = s
                    lasr_f(m, l)
                    d[l - 2] = rt1
                    d[l - 1] = rt2
                    e[l - 2] = _F(0.0)
                    l -= 2
                    if l >= lend:
                        continue
                    break
                if jtot == nmaxit:
                    break
                jtot += 1
                g = _F(_F(d[l - 2] - p) / _F(_F(2.0) * e[l - 2]))
                r = _slapy2(g, _F(1.0))
                g = _F(_F(d[m - 1] - p) + _F(e[l - 2] / _F(g + _sign(r, g))))
                s = _F(1.0)
                c = _F(1.0)
                p = _F(0.0)
                for i in range(m, l):
                    f = _F(s * e[i - 1])
                    b = _F(c * e[i - 1])
                    c, s, r = _slartg(g, f)
                    if i != m:
                        e[i - 2] = r
                    g = _F(d[i - 1] - p)
                    r = _F(_F(_F(d[i] - g) * s) + _F(_F(_F(2.0) * c) * b))
                    p = _F(s * r)
                    d[i - 1] = _F(g + p)
                    g = _F(_F(c * r) - b)
                    wc[i - 1] = c
                    ws[i - 1] = s
                lasr_f(m, l)
                d[l - 1] = _F(d[l - 1] - p)
                e[l - 2] = g
        if jtot >= nmaxit:
            break
    for ii in range(2, n + 1):
        i = ii - 1
        k = i
        p = d[i - 1]
        for j in range(ii, n + 1):
            if d[j - 1] < p:
                k = j
                p = d[j - 1]
        if k != i:
            d[k - 1] = d[i - 1]
            d[i - 1] = p
            tmp = Z[:, k - 1].copy()
            Z[:, k - 1] = Z[:, i - 1]
            Z[:, i - 1] = tmp
    return d, Z


def _eigh3_batch(covs):
    n = covs.shape[0]
    W = np.empty((n, 3), np.float32)
    V = np.empty((n, 3, 3), np.float32)
    for i in range(n):
        d, e, v2, tau = _ssytrd3(covs[i])
        w, Z = _ssteqr3(d, e)
        if tau != _F(0.0):
            for j in range(3):
                vtz = _F(Z[1, j] + _F(v2 * Z[2, j]))
                tvz = _F(tau * vtz)
                Z[1, j] = _F(Z[1, j] - tvz)
                Z[2, j] = _F(Z[2, j] - _F(v2 * tvz))
        W[i] = w
        V[i] = Z
    return W, V


# ------------------------------------------------------------- weights ----

def _prep_weights(ins):
    """Host-side packing of the model weights into device layouts.

    Strided column-pair scheme (see the layer maps below); conv0/conv1
    biases ride a ones-row in the rhs so the matmul itself adds them.
    """
    w0, w1, w2, w3 = ins["w0"], ins["w1"], ins["w2"], ins["w3"]
    b0, b1 = np.asarray(ins["b0"], np.float32), np.asarray(ins["b1"],
                                                           np.float32)

    d = {}
    # conv0: window rows (c:3, j:9): even cols l = 8q-1+j, odd l = 8q+j;
    # k = j - 2g for output slot g; row 27 = ones -> bias.
    W0 = np.zeros((28, 80), np.float32)
    for c in range(3):
        for j in range(9):
            for g in range(4):
                k = j - 2 * g
                if 0 <= k < 3:
                    for o in range(20):
                        W0[c * 9 + j, g * 20 + o] = w0[o, c, k]
    W0[27, :] = np.tile(b0, 4)
    d["W0"] = W0.astype(BF)

    def s1_rows(with_hl, with_hr):
        rows = [(g * 20, 20, g) for g in range(4)]
        if with_hl:
            rows.append((80, 20, -1))
        if with_hr:
            rows.append((100, 20, 4))
        return rows

    def mk(w, blocks, Ghalf, parity, Cout, shift, colbase=None):
        Cin = w.shape[1]
        K = max(rb + Cin for rb, _, _ in blocks)
        if colbase is None:
            colbase = [g * Cout for g in range(Ghalf)]
        W = np.zeros((K, max(colbase) + Cout), np.float32)
        for rb, _, lrel in blocks:
            for g in range(Ghalf):
                pos = 2 * g + parity
                k = (lrel + shift) - pos + 1
                if 0 <= k < 3:
                    for ci in range(Cin):
                        W[rb + ci, colbase[g] + np.arange(Cout)] = w[:, ci, k]
        return W

    # conv1 output M-order: g0->0, g1->64, g2->96, g3->32 (C1B) so conv2's
    # boundary reads sit at legal rhs bases.
    C1B = [0, 64, 96, 32]

    # baseline-layout W1 blocks, then re-rowed for the s1 layout with the
    # ones row at 80: main [0:80], ones 80, hl [81:101], hr [101:121].
    w1e1_base = mk(w1, s1_rows(True, False), 4, 0, 32, 0, C1B)   # [100,128]
    w1e2 = mk(w1, [(rb, 20, lr + 4) for rb, _, lr in
                   s1_rows(False, False)], 4, 0, 32, 0, C1B)     # [80,128]
    w1o1 = mk(w1, s1_rows(False, False), 4, 1, 32, 0, C1B)       # [80,128]
    w1o2_blocks = ([(g * 20, 20, g + 4) for g in range(4)] +
                   [(80, 20, 1000), (100, 20, 8)])
    w1o2_base = mk(w1, w1o2_blocks, 4, 1, 32, 0, C1B)            # [120,128]
    b1t = np.tile(b1, 4)
    W1e1 = np.zeros((101, 128), np.float32)
    W1e1[0:80] = w1e1_base[0:80]
    W1e1[80] = b1t
    W1e1[81:101] = w1e1_base[80:100]
    W1o2 = np.zeros((121, 128), np.float32)
    W1o2[0:80] = w1o2_base[0:80]
    W1o2[80] = b1t
    W1o2[101:121] = w1o2_base[100:120]
    d["W1e1"] = W1e1.astype(BF)
    d["W1e2"] = w1e2.astype(BF)
    d["W1o1"] = w1o1.astype(BF)
    d["W1o2"] = W1o2.astype(BF)

    # conv2 (G=4, Ghalf=2, Cout=64): stored2 rows (g:4, o:32)->128
    s2_main = [(0, 32, 0), (64, 32, 1), (96, 32, 2), (32, 32, 3)]
    d["W2e1"] = mk(w2, [(0, 32, -1)], 2, 0, 64, 0).astype(BF)
    d["W2e2"] = mk(w2, s2_main, 2, 0, 64, 0).astype(BF)
    d["W2o1"] = mk(w2, s2_main, 2, 1, 64, 0).astype(BF)
    d["W2o2"] = mk(w2, [(0, 32, 4)], 2, 1, 64, 0).astype(BF)

    # conv3 (G=12, Ghalf=6, Cout=20): stored3 rows (g:2, o:64)->128.
    # Window w covers pre-pool pos [12w, 12w+12); MM t reads s3 col 6w+t
    # (l3 = 12w + 2t - 2 + g); M = (h:6, o:20) = 120.
    for t in range(7):
        d[f"W3E{t}"] = mk(w3, [(0, 64, 2 * t - 2), (64, 64, 2 * t - 1)],
                          6, 0, 20, 0).astype(BF)
    for t in range(1, 8):
        d[f"W3O{t}"] = mk(w3, [(0, 64, 2 * t - 2), (64, 64, 2 * t - 1)],
                          6, 1, 20, 0).astype(BF)

    # fc: stored4 rows (h:6, o:20)->120, col w: feature (o, l4 = 6w + h)
    wl0 = ins["wl0"]
    WFC = np.zeros((120, 63 * 100), np.float32)
    for w in range(63):
        for h in range(6):
            l4 = 6 * w + h
            if l4 < 375:
                WFC[h * 20:(h + 1) * 20, w * 100:(w + 1) * 100] = \
                    wl0[:, np.arange(20)[:, None] * 375 + l4].T.reshape(
                        20, 100)
    d["WFC"] = WFC.astype(BF)

    d["B2"] = np.tile(ins["b2"], 2).astype(np.float32)[:, None]   # [128]
    d["B3"] = np.tile(ins["b3"], 6).astype(np.float32)[:, None]   # [120]
    # launch 2
    d["wcT"] = ins["wc"][:, :, 0].T.astype(np.float32).copy()      # [3, 20]
    d["bc"] = ins["bc"].astype(np.float32)[:, None]                # [20, 1]
    w0b = np.zeros((7, 20, 100), np.float32)
    for t in range(7):
        for o in range(20):
            w0b[t, o] = ins["wl0"][:, 7500 + o * 7 + t]
    d["w0bT"] = w0b
    d["bl0"] = ins["bl0"].astype(np.float32)[:, None]              # [100, 1]
    d["wl1T"] = ins["wl1"].T.astype(np.float32).copy()             # [100, 2]
    d["bl1"] = ins["bl1"].astype(np.float32)[:, None]              # [2, 1]
    return d


# ------------------------------------------------------------- launch 1 ----

def _build_launch1():
    nc = bacc.Bacc("TRN2", target_bir_lowering=False, debug=False,
                   num_devices=NCORES)
    dram = {}
    for nm, shape, dt in [
        ("x_winE", [28, NS, 750], BF16), ("x_winO", [28, NS, 750], BF16),
        ("x_f32", [NS, 3, L0], F32),
        ("W0", [28, 80], BF16),
        ("W1e1", [101, 128], BF16), ("W1e2", [80, 128], BF16),
        ("W1o1", [80, 128], BF16), ("W1o2", [121, 128], BF16),
        ("W2e1", [32, 128], BF16), ("W2e2", [128, 128], BF16),
        ("W2o1", [128, 128], BF16), ("W2o2", [32, 128], BF16),
    ] + [(f"W3E{t}", [128, 120], BF16) for t in range(7)] + \
        [(f"W3O{t}", [128, 120], BF16) for t in range(1, 8)] + [
        ("WFC", [120, 6300], BF16),
        ("B2", [128, 1], F32), ("B3", [120, 1], F32),
        ("INIT1", [1, BN, 750], BF16), ("INITZ", [128, BN, 1], BF16),
    ]:
        dram[nm] = nc.dram_tensor(nm, shape, dt, kind="ExternalInput").ap()
    out_p0 = nc.dram_tensor("partial0", [100, NS], F32,
                            kind="ExternalOutput").ap()
    out_mom = nc.dram_tensor("mom", [NS, 9], F32,
                             kind="ExternalOutput").ap()

    with tile.TileContext(nc) as tc:
        with tc.tile_pool(name="wpool", bufs=1) as wp, \
             tc.tile_pool(name="covp", bufs=1) as cvp, \
             tc.tile_pool(name="scrp", bufs=1) as scp, \
             tc.tile_pool(name="xw", bufs=2) as xwp, \
             tc.tile_pool(name="s1p", bufs=1) as s1p, \
             tc.tile_pool(name="s2p", bufs=1) as s2p, \
             tc.tile_pool(name="s3p", bufs=1) as s3p, \
             tc.tile_pool(name="s4p", bufs=1) as s4p, \
             tc.tile_pool(name="tep", bufs=4) as tep, \
             tc.tile_pool(name="ps", bufs=4, space="PSUM") as psp:

            xw_tiles = {}

            def issue_xw(b):
                if b >= NBLK or b in xw_tiles:
                    return
                n0 = b * BN
                te = xwp.tile([28, BN, 750], BF16, tag="xwE")
                nc.sync.dma_start(te[:], dram["x_winE"][:, n0:n0 + BN, :])
                to = xwp.tile([28, BN, 750], BF16, tag="xwO")
                nc.sync.dma_start(to[:], dram["x_winO"][:, n0:n0 + BN, :])
                xw_tiles[b] = (te, to)

            # The SP DMA queue is FIFO and a DMA holds HWDGE ~0.6us each:
            # preload ONLY what conv0(0)/conv1(0) need, trickle the rest
            # into the pipeline steps below via dma_sched.
            momt = cvp.tile([NS, 9], F32, tag="mom")
            p0sb = cvp.tile([100, NS], F32, tag="p0sb")
            s1 = s1p.tile([121, BN, 750], BF16, tag="s1")
            s2 = s2p.tile([128, BN, 377], BF16, tag="s2")
            s3 = s3p.tile([128, BN, 380], BF16, tag="s3")
            s4 = s4p.tile([120, NS, 63], BF16, tag="s4")
            xcv = cvp.tile([NS, 3, L0], F32, tag="xcv")

            Ws = {}
            for nm in (["W1e1", "W1e2", "W1o1", "W1o2", "W0",
                        "W2e2", "W2o1", "W2o2"] +
                       [f"W3E{t}" for t in range(7)] +
                       [f"W3O{t}" for t in range(1, 8)] + ["WFC"]):
                Ws[nm] = wp.tile(list(dram[nm].shape), BF16, name=nm, tag=nm)
            Wpad = wp.tile([64, 128], BF16, name="W2e1", tag="W2e1")
            Ws["W2e1"] = Wpad[32:64]
            B2t = wp.tile([128, 1], F32, tag="B2")
            B3t = wp.tile([120, 1], F32, tag="B3")

            def dma_w(nm):
                nc.sync.dma_start(Ws[nm][:], dram[nm][:])

            # prologue: conv0(0..1) + conv1(0) prerequisites only
            dma_w("W0")
            issue_xw(0)
            issue_xw(1)
            # ones row + hl/hr edges (engines cannot address partition
            # bases off the 0/32/64/96 grid; DMA can)
            nc.sync.dma_start(s1[80:81, :, :], dram["INIT1"][:])
            nc.sync.dma_start(s1[81:101, :, 0:1], dram["INITZ"][0:20])
            nc.sync.dma_start(s1[101:121, :, 749:750], dram["INITZ"][0:20])
            for nm in ["W1e1", "W1e2", "W1o1", "W1o2"]:
                dma_w(nm)

            def dmas_step0():
                nc.sync.dma_start(Wpad[32:64], dram["W2e1"][:])
                for nm in ["W2e2", "W2o1", "W2o2"]:
                    dma_w(nm)
                nc.sync.dma_start(B2t[:], dram["B2"][:])
                nc.sync.dma_start(s2[:, :, 0:1], dram["INITZ"][:])
                nc.sync.dma_start(s2[:, :, 376:377], dram["INITZ"][:])

            def dmas_step1():
                for t in range(7):
                    dma_w(f"W3E{t}")

            def dmas_step2():
                for t in range(1, 8):
                    dma_w(f"W3O{t}")
                nc.sync.dma_start(B3t[:], dram["B3"][:])
                nc.sync.dma_start(s3[:, :, 0:1], dram["INITZ"][:])
                for cz in range(376, 380):
                    nc.sync.dma_start(s3[:, :, cz:cz + 1], dram["INITZ"][:])

            def make_xcv_piece(i):
                c0 = i * 750
                return lambda: nc.sync.dma_start(
                    xcv[:, :, c0:c0 + 750], dram["x_f32"][:, :, c0:c0 + 750])

            def make_wfc_piece(i):
                c0 = i * 3150
                return lambda: nc.sync.dma_start(
                    Ws["WFC"][:, c0:c0 + 3150], dram["WFC"][:, c0:c0 + 3150])

            dma_sched = {0: dmas_step0, 1: dmas_step1, 2: dmas_step2}
            for i in range(8):
                dma_sched[3 + i] = make_xcv_piece(i)
            dma_sched[12] = make_wfc_piece(0)
            dma_sched[14] = make_wfc_piece(1)

            # interleaved covariance moment ops, one per scheduled step
            cov_ops = []
            for c in range(3):
                cov_ops.append(("mean", c))
            for c in range(3):
                cov_ops.append(("sq", c))
            for k, (c, dch) in enumerate([(0, 1), (0, 2), (1, 2)]):
                cov_ops.append(("xy", k, c, dch))
            cov_sched = {13 + 6 * i: op for i, op in enumerate(cov_ops)}

            def emit_cov(op):
                scr = scp.tile([NS, L0], BF16, tag="cscr")
                if op[0] == "mean":
                    c = op[1]
                    nc.scalar.activation(scr[:], xcv[:, c, :], ACTF.Copy,
                                         accum_out=momt[:, c:c + 1])
                elif op[0] == "sq":
                    c = op[1]
                    nc.scalar.activation(scr[:], xcv[:, c, :], ACTF.Square,
                                         accum_out=momt[:, 3 + c:4 + c])
                else:
                    _, k, c, dch = op
                    nc.vector.scalar_tensor_tensor(
                        scr[:], xcv[:, c, :], 1.0, xcv[:, dch, :],
                        AOP.mult, AOP.mult,
                        accum_out=momt[:, 6 + k:7 + k])

            def conv0(p):
                blk, nb = p // 4, (p % 4) * 2
                xwE, xwO = xw_tiles[blk]
                for ch in range(2):
                    c0 = ch * 375
                    psE = psp.tile([128, 2, 512], F32, tag="ps")
                    psO = psp.tile([128, 2, 512], F32, tag="ps")
                    for i in range(2):
                        nc.tensor.matmul(psE[0:80, i, 0:375], Ws["W0"][:],
                                         xwE[:, nb + i, c0:c0 + 375],
                                         start=True, stop=True)
                    for i in range(2):
                        nc.tensor.matmul(psO[0:80, i, 0:375], Ws["W0"][:],
                                         xwO[:, nb + i, c0:c0 + 375],
                                         start=True, stop=True)
                    tE = tep.tile([128, 2, 384], BF16, tag="tE")
                    nc.scalar.activation(tE[0:80, :, 0:375],
                                         psE[0:80, :, 0:375], ACTF.Relu)
                    nc.vector.scalar_tensor_tensor(
                        s1[0:80, nb:nb + 2, c0:c0 + 375],
                        psO[0:80, :, 0:375], 0.0, tE[0:80, :, 0:375],
                        AOP.max, AOP.max)
                # per-pair halo rows for conv1
                nc.sync.dma_start(s1[81:101, nb:nb + 2, 1:750],
                                  s1[60:80, nb:nb + 2, 0:749])
                nc.sync.dma_start(s1[101:121, nb:nb + 2, 0:749],
                                  s1[0:20, nb:nb + 2, 1:750])

            def conv1(p):
                nb = (p % 4) * 2
                psE = psp.tile([128, 2, 512], F32, tag="ps")
                psO = psp.tile([128, 2, 512], F32, tag="ps")
                for i in range(2):
                    n = nb + i
                    nc.tensor.matmul(psE[0:128, i, 0:375], Ws["W1e1"][:],
                                     s1[0:101, n, 0:750:2],
                                     start=True, stop=False)
                    nc.tensor.matmul(psE[0:128, i, 0:375], Ws["W1e2"][:],
                                     s1[0:80, n, 1:750:2],
                                     start=False, stop=True)
                for i in range(2):
                    n = nb + i
                    nc.tensor.matmul(psO[0:128, i, 0:375], Ws["W1o1"][:],
                                     s1[0:80, n, 0:750:2],
                                     start=True, stop=False)
                    nc.tensor.matmul(psO[0:128, i, 0:375], Ws["W1o2"][:],
                                     s1[0:121, n, 1:750:2],
                                     start=False, stop=True)
                tE = tep.tile([128, 2, 384], BF16, tag="tE")
                nc.scalar.activation(tE[0:128, :, 0:375],
                                     psE[0:128, :, 0:375], ACTF.Relu)
                nc.vector.scalar_tensor_tensor(
                    s2[0:128, nb:nb + 2, 1:376],
                    psO[0:128, :, 0:375], 0.0, tE[0:128, :, 0:375],
                    AOP.max, AOP.max)

            def conv2(p):
                nb = (p % 4) * 2
                psE = psp.tile([128, 2, 512], F32, tag="ps")
                psO = psp.tile([128, 2, 512], F32, tag="ps")
                for i in range(2):
                    n = nb + i
                    nc.tensor.matmul(psE[0:128, i, 0:375], Ws["W2e1"],
                                     s2[32:64, n, 0:375],
                                     start=True, stop=False)
                    nc.tensor.matmul(psE[0:128, i, 0:375], Ws["W2e2"][:],
                                     s2[0:128, n, 1:376],
                                     start=False, stop=True)
                for i in range(2):
                    n = nb + i
                    nc.tensor.matmul(psO[0:128, i, 0:375], Ws["W2o1"][:],
                                     s2[0:128, n, 1:376],
                                     start=True, stop=False)
                    nc.tensor.matmul(psO[0:128, i, 0:375], Ws["W2o2"][:],
                                     s2[0:32, n, 2:377],
                                     start=False, stop=True)
                tE = tep.tile([128, 2, 384], BF16, tag="tE")
                nc.scalar.activation(tE[0:128, :, 0:375],
                                     psE[0:128, :, 0:375], ACTF.Relu,
                                     bias=B2t[:])
                nc.vector.scalar_tensor_tensor(
                    s3[0:128, nb:nb + 2, 1:376],
                    psO[0:128, :, 0:375], B2t[:], tE[0:128, :, 0:375],
                    AOP.add, AOP.max)

            def conv3(qd):
                blk, nq = qd // 2, (qd % 2) * 4
                n0 = blk * BN + nq
                psE = psp.tile([128, 2, 512], F32, tag="ps")
                psO = psp.tile([128, 2, 512], F32, tag="ps")
                for t in range(7):
                    nc.tensor.matmul(
                        psE[0:120, 0, 0:252], Ws[f"W3E{t}"][:],
                        s3[0:128, nq:nq + 4, t:t + 373:6],
                        start=(t == 0), stop=(t == 6))
                for t in range(1, 8):
                    nc.tensor.matmul(
                        psO[0:120, 0, 0:252], Ws[f"W3O{t}"][:],
                        s3[0:128, nq:nq + 4, t:t + 373:6],
                        start=(t == 1), stop=(t == 7))
                tE = tep.tile([128, 512], BF16, tag="tE3")
                nc.scalar.activation(tE[0:120, 0:252], psE[0:120, 0, 0:252],
                                     ACTF.Relu, bias=B3t[:])
                nc.vector.scalar_tensor_tensor(
                    s4[0:120, n0:n0 + 4, 0:63],
                    psO[0:120, 0, 0:252].rearrange("p (n l) -> p n l", n=4),
                    B3t[:],
                    tE[0:120, 0:252].rearrange("p (n l) -> p n l", n=4),
                    AOP.add, AOP.max)

            def fc(half):
                c0 = half * 64
                psfc = psp.tile([128, 2, 512], F32, tag="ps")
                for w in range(63):
                    nc.tensor.matmul(
                        psfc[0:100, 0, 0:64],
                        Ws["WFC"][:, w * 100:(w + 1) * 100],
                        s4[:, c0:c0 + 64, w], start=(w == 0), stop=(w == 62))
                nc.scalar.copy(p0sb[:, c0:c0 + 64], psfc[0:100, 0, 0:64])

            # pipeline: conv0(p) | conv1(p-1) | conv2(p-2) |
            #           conv3((p-3)/2 @ odd p) | fc halves at p=35/66
            for p in range(67):
                if p < NPAIR:
                    if p % 4 == 0:
                        issue_xw(p // 4 + 2)
                    conv0(p)
                if 0 <= p - 1 < NPAIR:
                    conv1(p - 1)
                if 0 <= p - 2 < NPAIR:
                    conv2(p - 2)
                if p % 2 == 1 and 0 <= (p - 3) // 2 < 32:
                    conv3((p - 3) // 2)
                if p == 35 or p == 66:
                    fc(0 if p == 35 else 1)
                if p in cov_sched:
                    emit_cov(cov_sched[p])
                if p in dma_sched:
                    dma_sched[p]()

            nc.sync.dma_start(out_p0[:], p0sb[:])
            nc.sync.dma_start(out_mom[:], momt[:])

    nc.compile()
    return nc


# ------------------------------------------------------------- launch 2 ----

def _build_launch2():
    nc = bacc.Bacc("TRN2", target_bir_lowering=False, debug=False,
                   num_devices=NCORES)
    dr = {}
    for nm, shape in [("featsT", [3, 7 * NS]), ("p0T", [100, NS]),
                      ("wcT", [3, 20]), ("bc", [20, 1]),
                      ("w0bT", [7, 20, 100]), ("bl0", [100, 1]),
                      ("wl1T", [100, 2]), ("bl1", [2, 1])]:
        dr[nm] = nc.dram_tensor(nm, shape, F32, kind="ExternalInput").ap()
    out2 = nc.dram_tensor("out2", [2, NS], F32, kind="ExternalOutput").ap()

    with tile.TileContext(nc) as tc:
        with tc.tile_pool(name="w2p", bufs=1) as wp, \
             tc.tile_pool(name="ps2", bufs=2, space="PSUM") as psp:
            fT = wp.tile([3, 7 * NS], F32, tag="fT")
            nc.sync.dma_start(fT[:], dr["featsT"][:])
            p0T = wp.tile([100, NS], F32, tag="p0T")
            nc.sync.dma_start(p0T[:], dr["p0T"][:])
            wcT = wp.tile([3, 20], F32, tag="wcT")
            nc.sync.dma_start(wcT[:], dr["wcT"][:])
            bc = wp.tile([20, 1], F32, tag="bc")
            nc.sync.dma_start(bc[:], dr["bc"][:])
            w0bT = [wp.tile([20, 100], F32, name=f"w0bT{t}", tag=f"w0bT{t}")
                    for t in range(7)]
            for t in range(7):
                nc.sync.dma_start(w0bT[t][:], dr["w0bT"][t])
            bl0 = wp.tile([100, 1], F32, tag="bl0")
            nc.sync.dma_start(bl0[:], dr["bl0"][:])
            wl1T = wp.tile([100, 2], F32, tag="wl1T")
            nc.sync.dma_start(wl1T[:], dr["wl1T"][:])
            bl1 = wp.tile([2, 1], F32, tag="bl1")
            nc.sync.dma_start(bl1[:], dr["bl1"][:])

            # h1 = relu(wc @ feats + bc): [20, (t, n)]
            h1 = wp.tile([20, 7 * NS], F32, tag="h1")
            for half in range(2):
                c0 = half * 448
                ps = psp.tile([32, 448], F32, tag="ph")
                nc.tensor.matmul(ps[0:20, :], wcT[:], fT[:, c0:c0 + 448],
                                 start=True, stop=True)
                nc.scalar.activation(h1[:, c0:c0 + 448], ps[0:20, :],
                                     ACTF.Relu, bias=bc[:])
            # z = relu(p0 + sum_t w0b_t.T @ h1_t + bl0)
            psz = psp.tile([100, NS], F32, tag="pz")
            for t in range(7):
                nc.tensor.matmul(psz[:], w0bT[t][:],
                                 h1[:, t * NS:(t + 1) * NS],
                                 start=(t == 0), stop=(t == 6))
            z = wp.tile([100, NS], F32, tag="z")
            nc.vector.scalar_tensor_tensor(z[:], psz[:], bl0[:], p0T[:],
                                           AOP.add, AOP.add)
            nc.vector.tensor_scalar_max(z[:], z[:], 0.0)
            pso = psp.tile([32, NS], F32, tag="po")
            nc.tensor.matmul(pso[0:2, :], wl1T[:], z[:],
                             start=True, stop=True)
            osb = wp.tile([2, NS], F32, tag="osb")
            nc.vector.tensor_scalar(osb[:], pso[0:2, :], bl1[:], None,
                                    AOP.add)
            nc.sync.dma_start(out2[:], osb[:])

    nc.compile()
    return nc


# --------------------------------------------------------------- kernel ----

def kernel(**inputs):
    ins = {k: np.asarray(v) for k, v in inputs.items()}
    x = ins["x"].astype(np.float32)

    if "l1" not in _CACHE:
        _CACHE["l1"] = _build_launch1()
    if "l2" not in _CACHE:
        _CACHE["l2"] = _build_launch2()
    w = _prep_weights(ins)

    xbf = x.astype(BF)
    xwE = np.zeros((28, x.shape[0], 750), BF)
    xwO = np.zeros((28, x.shape[0], 750), BF)
    for c in range(3):
        for j in range(9):
            # even cols: l = 8q - 1 + j ; odd cols: l = 8q + j
            if j == 0:
                xwE[c * 9 + 0, :, 1:750] = xbf[:, c, 7:5992:8]
            else:
                xwE[c * 9 + j] = xbf[:, c, j - 1::8]
            if j == 8:
                xwO[c * 9 + 8, :, 0:749] = xbf[:, c, 8:6000:8]
            else:
                xwO[c * 9 + j] = xbf[:, c, j::8]
    xwE[27] = 1.0
    xwO[27] = 1.0
    shards = [x[i * NS:(i + 1) * NS] for i in range(NCORES)]
    in1 = []
    for i, sh in enumerate(shards):
        sl = slice(i * NS, (i + 1) * NS)
        m = {"x_winE": np.ascontiguousarray(xwE[:, sl]),
             "x_winO": np.ascontiguousarray(xwO[:, sl]),
             "x_f32": sh}
        for nm in (["W0", "W1e1", "W1e2", "W1o1", "W1o2",
                    "W2e1", "W2e2", "W2o1", "W2o2", "WFC", "B2", "B3"] +
                   [f"W3E{t}" for t in range(7)] +
                   [f"W3O{t}" for t in range(1, 8)]):
            m[nm] = w[nm]
        m["INIT1"] = np.ones((1, BN, 750), BF)
        m["INITZ"] = np.zeros((128, BN, 1), BF)
        in1.append(m)
    t0 = time.time()
    res1 = run_bass_kernel_spmd(_CACHE["l1"], in1, list(range(NCORES)))
    LAST_EXEC_NS[0] = res1.exec_time_ns
    LAST_WALL_S[0] = time.time() - t0

    mom = np.concatenate([res1.results[i]["mom"] for i in range(NCORES)], 0)
    partial0 = np.concatenate(
        [res1.results[i]["partial0"].T for i in range(NCORES)], 0)

    # host: cov assembly (fp32) + LAPACK-clone eigh + global normalizers
    Sx = mom[:, 0:3].astype(np.float32)
    Sxx = mom[:, 3:6].astype(np.float32)
    Sxy = mom[:, 6:9].astype(np.float32)
    L = np.float32(L0)
    cov = np.empty((x.shape[0], 3, 3), np.float32)
    for idx, (c, dch) in enumerate([(0, 1), (0, 2), (1, 2)]):
        v = (Sxy[:, idx] - Sx[:, c] * Sx[:, dch] / L) / np.float32(L0 - 1)
        cov[:, c, dch] = v
        cov[:, dch, c] = v
    for c in range(3):
        cov[:, c, c] = (Sxx[:, c] - Sx[:, c] * Sx[:, c] / L) / np.float32(
            L0 - 1)
    vals, vecs = _eigh3_batch(cov)
    covn = cov / np.abs(cov).max()
    valsn = (vals / vals.max())[..., None]
    feats = np.concatenate([covn, valsn, vecs], axis=-1).astype(np.float32)

    in2 = []
    for i in range(NCORES):
        sl = slice(i * NS, (i + 1) * NS)
        m = {"featsT": np.ascontiguousarray(
                 feats[sl].transpose(1, 2, 0).reshape(3, 7 * NS)),
             "p0T": partial0[sl].T.copy(),
             "wcT": w["wcT"], "bc": w["bc"], "w0bT": w["w0bT"],
             "bl0": w["bl0"], "wl1T": w["wl1T"], "bl1": w["bl1"]}
        in2.append(m)
    t0 = time.time()
    res2 = run_bass_kernel_spmd(_CACHE["l2"], in2, list(range(NCORES)))
    LAST_EXEC_NS[1] = res2.exec_time_ns
    LAST_WALL_S[1] = time.time() - t0

    out = np.concatenate([res2.results[i]["out2"].T for i in range(NCORES)],
                         0).astype(np.float32)
    return (out[:, 0:1], out[:, 1:2])


# revision 16
# speedup vs baseline: 1.1573x; 1.1089x over previous
"""Trainium2 Bass kernel for nn_BAZ_Network (dense CNN + cov/eig head).

Data-parallel over 8 NeuronCores: 128 samples each.

Launch 1 (per core), software-pipelined over 64 sample-pairs:
  conv trunk as G-packed banded-weight matmuls (bf16, fp32 PSUM), with
  conv biases folded into the matmuls via a ones-row in the rhs (conv0,
  conv1).  Postprocess per (E,O) parity pair is two fused ops over a
  2-sample two-PSUM-bank 3D access pattern:
    op1 (Act):  tE = relu(psE + b)           PSUM -> SBUF bf16
    op2 (DVE):  s  = max(psO + b, tE)        = relu(max(E,O)+b), the
                 maxpool, relu, bias and bf16 cast in one instruction.
  Covariance fp32 raw moments (means on Act copy+accum, squares on Act
  Square+accum, cross-products on DVE stt+accum) are interleaved into
  the pair pipeline; the FC contraction of the conv features against
  wl0[:, :7500] runs per-block (125 accumulating matmuls of 8 cols).
  Stage stagger: conv0(p) | conv1(p-1) | conv2(p-2) | conv3 at odd p |
  FC at p%4==2, which hides the halo-DMA and PSUM-evacuation latency.
Host: branch-exact fp32 netlib-LAPACK ssyevd clone for the 3x3 eigh
  (required to reproduce jnp.linalg.eigh eigenvector signs).
Launch 2 (per core): eig-feature head: 1x1 conv (wc) + relu, remaining
  FC columns wl0[:, 7500:], bias+relu, final linear wl1.
"""

import os
import sys
import time
import numpy as np
import ml_dtypes

sys.path.insert(0, "/opt/trn_rl_repo")
os.environ["BASS_NEVER_TRACE"] = "1"

import concourse.bass as bass  # noqa: E402
import concourse.tile as tile  # noqa: E402
import concourse.mybir as mybir  # noqa: E402
from concourse import bacc  # noqa: E402
from concourse.bass_utils import run_bass_kernel_spmd  # noqa: E402

F32 = mybir.dt.float32
BF16 = mybir.dt.bfloat16
AOP = mybir.AluOpType
ACTF = mybir.ActivationFunctionType
BF = ml_dtypes.bfloat16

NCORES = 8
NS = 128          # samples per core
BN = 8            # samples per block
NBLK = NS // BN
NPAIR = NS // 2   # 64 sample-pairs, the pipeline unit
L0 = 6000

LAST_EXEC_NS = [None, None]
LAST_WALL_S = [None, None]
_CACHE = {}


# ---------------------------------------------------------------- eigh ----
# fp32 netlib-LAPACK ssyevd clone for n=3 (jobz='V', uplo='L').
# Matches jaxlib's CPU eigh (LAPACK >= 3.10 slartg) bit-closely: 0/3072
# eigenvector sign mismatches on the problem distribution.

_F = np.float32
_EPS = _F(np.finfo(np.float32).eps) * _F(0.5)
_EPS2 = _EPS * _EPS
_SAFMIN = _F(np.finfo(np.float32).tiny)


def _slapy2(x, y):
    xa, ya = abs(x), abs(y)
    w, z = max(xa, ya), min(xa, ya)
    if z == 0:
        return w
    return _F(w * _F(np.sqrt(_F(_F(1.0) + _F(_F(z / w) * _F(z / w))))))


def _sign(a, b):
    return abs(a) if b >= 0 else -abs(a)


def _slartg(f, g):
    if g == _F(0.0):
        return _F(1.0), _F(0.0), f
    if f == _F(0.0):
        return _F(0.0), _sign(_F(1.0), g), abs(g)
    d = _F(np.sqrt(_F(f * f + g * g)))
    c = _F(abs(f) / d)
    r = _sign(d, f)
    s = _F(g / r)
    return c, s, r


def _slaev2(a, b, c):
    sm = _F(a + c)
    df = _F(a - c)
    adf = abs(df)
    tb = _F(b + b)
    ab = abs(tb)
    acmx, acmn = (a, c) if abs(a) > abs(c) else (c, a)
    if adf > ab:
        t = _F(ab / adf)
        rt = _F(adf * _F(np.sqrt(_F(_F(1.0) + _F(t * t)))))
    elif adf < ab:
        t = _F(adf / ab)
        rt = _F(ab * _F(np.sqrt(_F(_F(1.0) + _F(t * t)))))
    else:
        rt = _F(ab * _F(np.sqrt(_F(2.0))))
    if sm < 0:
        rt1 = _F(_F(0.5) * _F(sm - rt))
        sgn1 = -1
        rt2 = _F(_F(_F(acmx / rt1) * acmn) - _F(_F(b / rt1) * b))
    elif sm > 0:
        rt1 = _F(_F(0.5) * _F(sm + rt))
        sgn1 = 1
        rt2 = _F(_F(_F(acmx / rt1) * acmn) - _F(_F(b / rt1) * b))
    else:
        rt1 = _F(_F(0.5) * rt)
        rt2 = _F(_F(-0.5) * rt)
        sgn1 = 1
    if df >= 0:
        cs = _F(df + rt)
        sgn2 = 1
    else:
        cs = _F(df - rt)
        sgn2 = -1
    acs = abs(cs)
    if acs > ab:
        ct = _F(-tb / cs)
        sn1 = _F(_F(1.0) / _F(np.sqrt(_F(_F(1.0) + _F(ct * ct)))))
        cs1 = _F(ct * sn1)
    else:
        if ab == 0:
            cs1, sn1 = _F(1.0), _F(0.0)
        else:
            tn = _F(-cs / tb)
            cs1 = _F(_F(1.0) / _F(np.sqrt(_F(_F(1.0) + _F(tn * tn)))))
            sn1 = _F(tn * cs1)
    if sgn1 == sgn2:
        cs1, sn1 = -sn1, cs1
    return rt1, rt2, cs1, sn1


def _ssytrd3(A):
    a00, a10, a20 = A[0, 0], A[1, 0], A[2, 0]
    a11, a21, a22 = A[1, 1], A[2, 1], A[2, 2]
    xnorm = abs(a20)
    if xnorm == _F(0.0):
        beta, v2, tau = a10, a20, _F(0.0)
    else:
        beta = -_sign(_slapy2(a10, xnorm), a10)
        tau = _F(_F(beta - a10) / beta)
        v2 = _F(a20 * _F(_F(1.0) / _F(a10 - beta)))
    e0 = beta
    if tau != _F(0.0):
        x0 = _F(_F(tau * a11) + _F(tau * _F(a21 * v2)))
        x1 = _F(_F(tau * a21) + _F(_F(tau * v2) * a22))
        sdot = _F(_F(x0 * _F(1.0)) + _F(x1 * v2))
        alpha = _F(_F(_F(-0.5) * tau) * sdot)
        w0 = _F(x0 + _F(alpha * _F(1.0)))
        w1 = _F(x1 + _F(alpha * v2))
        t1, t2 = -w0, _F(-1.0)
        a11 = _F(_F(a11 + _F(_F(1.0) * t1)) + _F(w0 * t2))
        a21 = _F(_F(a21 + _F(v2 * t1)) + _F(w1 * t2))
        t1b, t2b = -w1, -v2
        a22 = _F(_F(a22 + _F(v2 * t1b)) + _F(w1 * t2b))
    d = np.array([a00, a11, a22], np.float32)
    e = np.array([e0, a21, 0.0], np.float32)
    return d, e, v2, tau


def _ssteqr3(d, e):
    n = 3
    Z = np.eye(3, dtype=np.float32)
    wc = np.zeros(2, np.float32)
    ws = np.zeros(2, np.float32)
    nmaxit, jtot = 90, 0

    def lasr_b(l, m):
        for j in range(m - 1, l - 1, -1):
            c, s = wc[j - 1], ws[j - 1]
            if c != _F(1.0) or s != _F(0.0):
                for i in range(3):
                    t = Z[i, j]
                    Z[i, j] = _F(_F(c * t) - _F(s * Z[i, j - 1]))
                    Z[i, j - 1] = _F(_F(s * t) + _F(c * Z[i, j - 1]))

    def lasr_f(m, l):
        for j in range(m, l):
            c, s = wc[j - 1], ws[j - 1]
            if c != _F(1.0) or s != _F(0.0):
                for i in range(3):
                    t = Z[i, j]
                    Z[i, j] = _F(_F(c * t) - _F(s * Z[i, j - 1]))
                    Z[i, j - 1] = _F(_F(s * t) + _F(c * Z[i, j - 1]))

    l1 = 1
    while True:
        if l1 > n:
            break
        if l1 > 1:
            e[l1 - 2] = _F(0.0)
        m = n
        for mm in range(l1, n):
            tst = abs(e[mm - 1])
            if tst == _F(0.0):
                m = mm
                break
            if tst <= _F(_F(_F(np.sqrt(abs(d[mm - 1]))) *
                            _F(np.sqrt(abs(d[mm])))) * _EPS):
                e[mm - 1] = _F(0.0)
                m = mm
                break
        l = l1
        lend = m
        l1 = m + 1
        if lend == l:
            continue
        if abs(d[lend - 1]) < abs(d[l - 1]):
            lend, l = l, lend
        if lend > l:
            while True:  # QL
                m = lend
                if l != lend:
                    for mm in range(l, lend):
                        tst = _F(abs(e[mm - 1]) * abs(e[mm - 1]))
                        if tst <= _F(_F(_F(_EPS2 * abs(d[mm - 1])) *
                                        abs(d[mm])) + _SAFMIN):
                            m = mm
                            break
                if m < lend:
                    e[m - 1] = _F(0.0)
                p = d[l - 1]
                if m == l:
                    d[l - 1] = p
                    l += 1
                    if l <= lend:
                        continue
                    break
                if m == l + 1:
                    rt1, rt2, c, s = _slaev2(d[l - 1], e[l - 1], d[l])
                    wc[l - 1] = c
                    ws[l - 1] = s
                    lasr_b(l, l + 1)
                    d[l - 1] = rt1
                    d[l] = rt2
                    e[l - 1] = _F(0.0)
                    l += 2
                    if l <= lend:
                        continue
                    break
                if jtot == nmaxit:
                    break
                jtot += 1
                g = _F(_F(d[l] - p) / _F(_F(2.0) * e[l - 1]))
                r = _slapy2(g, _F(1.0))
                g = _F(_F(d[m - 1] - p) + _F(e[l - 1] / _F(g + _sign(r, g))))
                s = _F(1.0)
                c = _F(1.0)
                p = _F(0.0)
                for i in range(m - 1, l - 1, -1):
                    f = _F(s * e[i - 1])
                    b = _F(c * e[i - 1])
                    c, s, r = _slartg(g, f)
                    if i != m - 1:
                        e[i] = r
                    g = _F(d[i] - p)
                    r = _F(_F(_F(d[i - 1] - g) * s) + _F(_F(_F(2.0) * c) * b))
                    p = _F(s * r)
                    d[i] = _F(g + p)
                    g = _F(_F(c * r) - b)
                    wc[i - 1] = c
                    ws[i - 1] = -s
                lasr_b(l, m)
                d[l - 1] = _F(d[l - 1] - p)
                e[l - 1] = g
        else:
            while True:  # QR
                m = lend
                if l != lend:
                    for mm in range(l, lend, -1):
                        tst = _F(abs(e[mm - 2]) * abs(e[mm - 2]))
                        if tst <= _F(_F(_F(_EPS2 * abs(d[mm - 1])) *
                                        abs(d[mm - 2])) + _SAFMIN):
                            m = mm
                            break
                if m > lend:
                    e[m - 2] = _F(0.0)
                p = d[l - 1]
                if m == l:
                    d[l - 1] = p
                    l -= 1
                    if l >= lend:
                        continue
                    break
                if m == l - 1:
                    rt1, rt2, c, s = _slaev2(d[l - 2], e[l - 2], d[l - 1])
                    wc[m - 1] = c
                    ws[m - 1] = s
                    lasr_f(m, l)
                    d[l - 2] = rt1
                    d[l - 1] = rt2
                    e[l - 2] = _F(0.0)
                    l -= 2
                    if l >= lend:
                        continue
                    break
                if jtot == nmaxit:
                    break
                jtot += 1
                g = _F(_F(d[l - 2] - p) / _F(_F(2.0) * e[l - 2]))
                r = _slapy2(g, _F(1.0))
                g = _F(_F(d[m - 1] - p) + _F(e[l - 2] / _F(g + _sign(r, g))))
                s = _F(1.0)
                c = _F(1.0)
                p = _F(0.0)
                for i in range(m, l):
                    f = _F(s * e[i - 1])
                    b = _F(c * e[i - 1])
                    c, s, r = _slartg(g, f)
                    if i != m:
                        e[i - 2] = r
                    g = _F(d[i - 1] - p)
                    r = _F(_F(_F(d[i] - g) * s) + _F(_F(_F(2.0) * c) * b))
                    p = _F(s * r)
                    d[i - 1] = _F(g + p)
                    g = _F(_F(c * r) - b)
                    wc[i - 1] = c
                    ws[i - 1] = s
                lasr_f(m, l)
                d[l - 1] = _F(d[l - 1] - p)
                e[l - 2] = g
        if jtot >= nmaxit:
            break
    for ii in range(2, n + 1):
        i = ii - 1
        k = i
        p = d[i - 1]
        for j in range(ii, n + 1):
            if d[j - 1] < p:
                k = j
                p = d[j - 1]
        if k != i:
            d[k - 1] = d[i - 1]
            d[i - 1] = p
            tmp = Z[:, k - 1].copy()
            Z[:, k - 1] = Z[:, i - 1]
            Z[:, i - 1] = tmp
    return d, Z


def _eigh3_batch(covs):
    n = covs.shape[0]
    W = np.empty((n, 3), np.float32)
    V = np.empty((n, 3, 3), np.float32)
    for i in range(n):
        d, e, v2, tau = _ssytrd3(covs[i])
        w, Z = _ssteqr3(d, e)
        if tau != _F(0.0):
            for j in range(3):
                vtz = _F(Z[1, j] + _F(v2 * Z[2, j]))
                tvz = _F(tau * vtz)
                Z[1, j] = _F(Z[1, j] - tvz)
                Z[2, j] = _F(Z[2, j] - _F(v2 * tvz))
        W[i] = w
        V[i] = Z
    return W, V


# ------------------------------------------------------------- weights ----

def _prep_weights(ins):
    """Host-side packing of the model weights into device layouts.

    Strided column-pair scheme (see the layer maps below); conv0/conv1
    biases ride a ones-row in the rhs so the matmul itself adds them.
    """
    w0, w1, w2, w3 = ins["w0"], ins["w1"], ins["w2"], ins["w3"]
    b0, b1 = np.asarray(ins["b0"], np.float32), np.asarray(ins["b1"],
                                                           np.float32)

    d = {}
    # conv0: window rows (c:3, j:9): even cols l = 8q-1+j, odd l = 8q+j;
    # k = j - 2g for output slot g; row 27 = ones -> bias.
    W0 = np.zeros((28, 80), np.float32)
    for c in range(3):
        for j in range(9):
            for g in range(4):
                k = j - 2 * g
                if 0 <= k < 3:
                    for o in range(20):
                        W0[c * 9 + j, g * 20 + o] = w0[o, c, k]
    W0[27, :] = np.tile(b0, 4)
    d["W0"] = W0.astype(BF)

    def s1_rows(with_hl, with_hr):
        rows = [(g * 20, 20, g) for g in range(4)]
        if with_hl:
            rows.append((80, 20, -1))
        if with_hr:
            rows.append((100, 20, 4))
        return rows

    def mk(w, blocks, Ghalf, parity, Cout, shift, colbase=None):
        Cin = w.shape[1]
        K = max(rb + Cin for rb, _, _ in blocks)
        if colbase is None:
            colbase = [g * Cout for g in range(Ghalf)]
        W = np.zeros((K, max(colbase) + Cout), np.float32)
        for rb, _, lrel in blocks:
            for g in range(Ghalf):
                pos = 2 * g + parity
                k = (lrel + shift) - pos + 1
                if 0 <= k < 3:
                    for ci in range(Cin):
                        W[rb + ci, colbase[g] + np.arange(Cout)] = w[:, ci, k]
        return W

    # conv1 output M-order: g0->0, g1->64, g2->96, g3->32 (C1B) so conv2's
    # boundary reads sit at legal rhs bases.
    C1B = [0, 64, 96, 32]

    # baseline-layout W1 blocks, then re-rowed for the s1 layout with the
    # ones row at 80: main [0:80], ones 80, hl [81:101], hr [101:121].
    w1e1_base = mk(w1, s1_rows(True, False), 4, 0, 32, 0, C1B)   # [100,128]
    w1e2 = mk(w1, [(rb, 20, lr + 4) for rb, _, lr in
                   s1_rows(False, False)], 4, 0, 32, 0, C1B)     # [80,128]
    w1o1 = mk(w1, s1_rows(False, False), 4, 1, 32, 0, C1B)       # [80,128]
    w1o2_blocks = ([(g * 20, 20, g + 4) for g in range(4)] +
                   [(80, 20, 1000), (100, 20, 8)])
    w1o2_base = mk(w1, w1o2_blocks, 4, 1, 32, 0, C1B)            # [120,128]
    b1t = np.tile(b1, 4)
    W1e1 = np.zeros((101, 128), np.float32)
    W1e1[0:80] = w1e1_base[0:80]
    W1e1[80] = b1t
    W1e1[81:101] = w1e1_base[80:100]
    W1o2 = np.zeros((121, 128), np.float32)
    W1o2[0:80] = w1o2_base[0:80]
    W1o2[80] = b1t
    W1o2[101:121] = w1o2_base[100:120]
    d["W1e1"] = W1e1.astype(BF)
    d["W1e2"] = w1e2.astype(BF)
    d["W1o1"] = w1o1.astype(BF)
    d["W1o2"] = W1o2.astype(BF)

    # conv2 (G=4, Ghalf=2, Cout=64): stored2 rows (g:4, o:32)->128
    s2_main = [(0, 32, 0), (64, 32, 1), (96, 32, 2), (32, 32, 3)]
    d["W2e1"] = mk(w2, [(0, 32, -1)], 2, 0, 64, 0).astype(BF)
    d["W2e2"] = mk(w2, s2_main, 2, 0, 64, 0).astype(BF)
    d["W2o1"] = mk(w2, s2_main, 2, 1, 64, 0).astype(BF)
    d["W2o2"] = mk(w2, [(0, 32, 4)], 2, 1, 64, 0).astype(BF)

    # conv3 (G=12, Ghalf=6, Cout=20): stored3 rows (g:2, o:64)->128.
    # Window w covers pre-pool pos [12w, 12w+12); MM t reads s3 col 6w+t
    # (l3 = 12w + 2t - 2 + g); M = (h:6, o:20) = 120.
    for t in range(7):
        d[f"W3E{t}"] = mk(w3, [(0, 64, 2 * t - 2), (64, 64, 2 * t - 1)],
                          6, 0, 20, 0).astype(BF)
    for t in range(1, 8):
        d[f"W3O{t}"] = mk(w3, [(0, 64, 2 * t - 2), (64, 64, 2 * t - 1)],
                          6, 1, 20, 0).astype(BF)

    # fc: stored4 rows (h:6, o:20)->120, col w: feature (o, l4 = 6w + h)
    wl0 = ins["wl0"]
    WFC = np.zeros((120, 63 * 100), np.float32)
    for w in range(63):
        for h in range(6):
            l4 = 6 * w + h
            if l4 < 375:
                WFC[h * 20:(h + 1) * 20, w * 100:(w + 1) * 100] = \
                    wl0[:, np.arange(20)[:, None] * 375 + l4].T.reshape(
                        20, 100)
    d["WFC"] = WFC.astype(BF)

    d["B2"] = np.tile(ins["b2"], 2).astype(np.float32)[:, None]   # [128]
    d["B3"] = np.tile(ins["b3"], 6).astype(np.float32)[:, None]   # [120]
    # launch 2
    d["wcT"] = ins["wc"][:, :, 0].T.astype(np.float32).copy()      # [3, 20]
    d["bc"] = ins["bc"].astype(np.float32)[:, None]                # [20, 1]
    w0b = np.zeros((7, 20, 100), np.float32)
    for t in range(7):
        for o in range(20):
            w0b[t, o] = ins["wl0"][:, 7500 + o * 7 + t]
    d["w0bT"] = w0b
    d["bl0"] = ins["bl0"].astype(np.float32)[:, None]              # [100, 1]
    d["wl1T"] = ins["wl1"].T.astype(np.float32).copy()             # [100, 2]
    d["bl1"] = ins["bl1"].astype(np.float32)[:, None]              # [2, 1]
    return d


# ------------------------------------------------------------- launch 1 ----

def _build_launch1():
    nc = bacc.Bacc("TRN2", target_bir_lowering=False, debug=False,
                   num_devices=NCORES)
    dram = {}
    for nm, shape, dt in [
        ("x_winE", [28, NS, 750], BF16), ("x_winO", [28, NS, 750], BF16),
        ("x_f32", [NS, 3, L0], F32),
        ("W0", [28, 80], BF16),
        ("W1e1", [101, 128], BF16), ("W1e2", [80, 128], BF16),
        ("W1o1", [80, 128], BF16), ("W1o2", [121, 128], BF16),
        ("W2e1", [32, 128], BF16), ("W2e2", [128, 128], BF16),
        ("W2o1", [128, 128], BF16), ("W2o2", [32, 128], BF16),
    ] + [(f"W3E{t}", [128, 120], BF16) for t in range(7)] + \
        [(f"W3O{t}", [128, 120], BF16) for t in range(1, 8)] + [
        ("WFC", [120, 6300], BF16),
        ("B2", [128, 1], F32), ("B3", [120, 1], F32),
        ("INIT1", [1, BN, 750], BF16), ("INITZ", [128, BN, 1], BF16),
    ]:
        dram[nm] = nc.dram_tensor(nm, shape, dt, kind="ExternalInput").ap()
    out_p0 = nc.dram_tensor("partial0", [100, NS], F32,
                            kind="ExternalOutput").ap()
    out_mom = nc.dram_tensor("mom", [NS, 72], F32,
                             kind="ExternalOutput").ap()

    with tile.TileContext(nc) as tc:
        with tc.tile_pool(name="wpool", bufs=1) as wp, \
             tc.tile_pool(name="covp", bufs=1) as cvp, \
             tc.tile_pool(name="scrp", bufs=1) as scp, \
             tc.tile_pool(name="xw", bufs=2) as xwp, \
             tc.tile_pool(name="s1p", bufs=1) as s1p, \
             tc.tile_pool(name="s2p", bufs=1) as s2p, \
             tc.tile_pool(name="s3p", bufs=1) as s3p, \
             tc.tile_pool(name="s4p", bufs=1) as s4p, \
             tc.tile_pool(name="tep", bufs=4) as tep, \
             tc.tile_pool(name="ps", bufs=4, space="PSUM") as psp:

            xw_tiles = {}

            def issue_xw(b):
                if b >= NBLK or b in xw_tiles:
                    return
                n0 = b * BN
                te = xwp.tile([28, BN, 750], BF16, tag="xwE")
                nc.sync.dma_start(te[:], dram["x_winE"][:, n0:n0 + BN, :])
                to = xwp.tile([28, BN, 750], BF16, tag="xwO")
                nc.sync.dma_start(to[:], dram["x_winO"][:, n0:n0 + BN, :])
                xw_tiles[b] = (te, to)

            # The SP DMA queue is FIFO and a DMA holds HWDGE ~0.6us each:
            # preload ONLY what conv0(0)/conv1(0) need, trickle the rest
            # into the pipeline steps below via dma_sched.
            momt = cvp.tile([NS, 72], F32, tag="mom")
            p0sb = cvp.tile([100, NS], F32, tag="p0sb")
            s1 = s1p.tile([121, BN, 750], BF16, tag="s1")
            s2 = s2p.tile([128, BN, 377], BF16, tag="s2")
            s3 = s3p.tile([128, BN, 380], BF16, tag="s3")
            s4 = s4p.tile([120, NS, 63], BF16, tag="s4")
            xcv = cvp.tile([NS, 3, L0], F32, tag="xcv")

            Ws = {}
            for nm in (["W1e1", "W1e2", "W1o1", "W1o2", "W0",
                        "W2e2", "W2o1", "W2o2"] +
                       [f"W3E{t}" for t in range(7)] +
                       [f"W3O{t}" for t in range(1, 8)] + ["WFC"]):
                Ws[nm] = wp.tile(list(dram[nm].shape), BF16, name=nm, tag=nm)
            Wpad = wp.tile([64, 128], BF16, name="W2e1", tag="W2e1")
            Ws["W2e1"] = Wpad[32:64]
            B2t = wp.tile([128, 1], F32, tag="B2")
            B3t = wp.tile([120, 1], F32, tag="B3")

            def dma_w(nm):
                nc.sync.dma_start(Ws[nm][:], dram[nm][:])

            # prologue: conv0(0..1) + conv1(0) prerequisites only
            dma_w("W0")
            issue_xw(0)
            issue_xw(1)
            # ones row + hl/hr edges (engines cannot address partition
            # bases off the 0/32/64/96 grid; DMA can)
            nc.sync.dma_start(s1[80:81, :, :], dram["INIT1"][:])
            nc.sync.dma_start(s1[81:101, :, 0:1], dram["INITZ"][0:20])
            nc.sync.dma_start(s1[101:121, :, 749:750], dram["INITZ"][0:20])
            for nm in ["W1e1", "W1e2", "W1o1", "W1o2"]:
                dma_w(nm)

            def dmas_step0():
                nc.sync.dma_start(Wpad[32:64], dram["W2e1"][:])
                for nm in ["W2e2", "W2o1", "W2o2"]:
                    dma_w(nm)
                nc.sync.dma_start(B2t[:], dram["B2"][:])
                nc.sync.dma_start(s2[:, :, 0:1], dram["INITZ"][:])
                nc.sync.dma_start(s2[:, :, 376:377], dram["INITZ"][:])

            def dmas_step1():
                for t in range(7):
                    dma_w(f"W3E{t}")

            def dmas_step2():
                for t in range(1, 8):
                    dma_w(f"W3O{t}")
                nc.sync.dma_start(B3t[:], dram["B3"][:])
                nc.sync.dma_start(s3[:, :, 0:1], dram["INITZ"][:])
                for cz in range(376, 380):
                    nc.sync.dma_start(s3[:, :, cz:cz + 1], dram["INITZ"][:])

            def make_xcv_piece(i):
                c0 = i * 750
                return lambda: nc.sync.dma_start(
                    xcv[:, :, c0:c0 + 750], dram["x_f32"][:, :, c0:c0 + 750])

            def make_wfc_piece(i):
                c0 = i * 3150
                return lambda: nc.sync.dma_start(
                    Ws["WFC"][:, c0:c0 + 3150], dram["WFC"][:, c0:c0 + 3150])

            dma_sched = {0: dmas_step0, 1: dmas_step1, 2: dmas_step2}
            for i in range(8):
                dma_sched[3 + i] = make_xcv_piece(i)
            dma_sched[12] = make_wfc_piece(0)
            dma_sched[14] = make_wfc_piece(1)

            # covariance moment ops, chunked 8x750 so no single op can
            # head-of-line block the Act/DVE queues; 72 partial sums,
            # host adds the 8 chunks per moment.
            cov_ops = []
            for ch in range(8):
                for c in range(3):
                    cov_ops.append(("mean", c, ch))
                for c in range(3):
                    cov_ops.append(("sq", c, ch))
                for k, (c, dch) in enumerate([(0, 1), (0, 2), (1, 2)]):
                    cov_ops.append(("xy", k, c, dch, ch))
            cov_sched = {}
            for i, op in enumerate(cov_ops):
                cov_sched.setdefault(12 + i // 2, []).append(op)

            def emit_cov(op):
                ch = op[-1]
                c0 = ch * 750
                sl = slice(c0, c0 + 750)
                scr = scp.tile([NS, 750], BF16, tag="cscr")
                if op[0] == "mean":
                    c = op[1]
                    nc.scalar.activation(scr[:], xcv[:, c, sl], ACTF.Copy,
                                         accum_out=momt[:, c * 8 + ch:
                                                        c * 8 + ch + 1])
                elif op[0] == "sq":
                    c = op[1]
                    nc.scalar.activation(scr[:], xcv[:, c, sl], ACTF.Square,
                                         accum_out=momt[:, 24 + c * 8 + ch:
                                                        24 + c * 8 + ch + 1])
                else:
                    _, k, c, dch, ch = op
                    nc.vector.scalar_tensor_tensor(
                        scr[:], xcv[:, c, sl], 1.0, xcv[:, dch, sl],
                        AOP.mult, AOP.mult,
                        accum_out=momt[:, 48 + k * 8 + ch:
                                       48 + k * 8 + ch + 1])

            def conv0(p):
                blk, nb = p // 4, (p % 4) * 2
                xwE, xwO = xw_tiles[blk]
                for ch in range(2):
                    c0 = ch * 375
                    psE = psp.tile([128, 2, 512], F32, tag="ps")
                    psO = psp.tile([128, 2, 512], F32, tag="ps")
                    for i in range(2):
                        nc.tensor.matmul(psE[0:80, i, 0:375], Ws["W0"][:],
                                         xwE[:, nb + i, c0:c0 + 375],
                                         start=True, stop=True)
                    for i in range(2):
                        nc.tensor.matmul(psO[0:80, i, 0:375], Ws["W0"][:],
                                         xwO[:, nb + i, c0:c0 + 375],
                                         start=True, stop=True)
                    tE = tep.tile([128, 2, 384], BF16, tag="tE")
                    nc.scalar.activation(tE[0:80, :, 0:375],
                                         psE[0:80, :, 0:375], ACTF.Relu)
                    nc.vector.scalar_tensor_tensor(
                        s1[0:80, nb:nb + 2, c0:c0 + 375],
                        psO[0:80, :, 0:375], 0.0, tE[0:80, :, 0:375],
                        AOP.max, AOP.max)
                # per-pair halo rows for conv1
                nc.sync.dma_start(s1[81:101, nb:nb + 2, 1:750],
                                  s1[60:80, nb:nb + 2, 0:749])
                nc.sync.dma_start(s1[101:121, nb:nb + 2, 0:749],
                                  s1[0:20, nb:nb + 2, 1:750])

            def conv1(p):
                nb = (p % 4) * 2
                psE = psp.tile([128, 2, 512], F32, tag="ps")
                psO = psp.tile([128, 2, 512], F32, tag="ps")
                for i in range(2):
                    n = nb + i
                    nc.tensor.matmul(psE[0:128, i, 0:375], Ws["W1e1"][:],
                                     s1[0:101, n, 0:750:2],
                                     start=True, stop=False)
                    nc.tensor.matmul(psE[0:128, i, 0:375], Ws["W1e2"][:],
                                     s1[0:80, n, 1:750:2],
                                     start=False, stop=True)
                for i in range(2):
                    n = nb + i
                    nc.tensor.matmul(psO[0:128, i, 0:375], Ws["W1o1"][:],
                                     s1[0:80, n, 0:750:2],
                                     start=True, stop=False)
                    nc.tensor.matmul(psO[0:128, i, 0:375], Ws["W1o2"][:],
                                     s1[0:121, n, 1:750:2],
                                     start=False, stop=True)
                tE = tep.tile([128, 2, 384], BF16, tag="tE")
                nc.scalar.activation(tE[0:128, :, 0:375],
                                     psE[0:128, :, 0:375], ACTF.Relu)
                nc.vector.scalar_tensor_tensor(
                    s2[0:128, nb:nb + 2, 1:376],
                    psO[0:128, :, 0:375], 0.0, tE[0:128, :, 0:375],
                    AOP.max, AOP.max)

            def conv2(p):
                nb = (p % 4) * 2
                psE = psp.tile([128, 2, 512], F32, tag="ps")
                psO = psp.tile([128, 2, 512], F32, tag="ps")
                for i in range(2):
                    n = nb + i
                    nc.tensor.matmul(psE[0:128, i, 0:375], Ws["W2e1"],
                                     s2[32:64, n, 0:375],
                                     start=True, stop=False)
                    nc.tensor.matmul(psE[0:128, i, 0:375], Ws["W2e2"][:],
                                     s2[0:128, n, 1:376],
                                     start=False, stop=True)
                for i in range(2):
                    n = nb + i
                    nc.tensor.matmul(psO[0:128, i, 0:375], Ws["W2o1"][:],
                                     s2[0:128, n, 1:376],
                                     start=True, stop=False)
                    nc.tensor.matmul(psO[0:128, i, 0:375], Ws["W2o2"][:],
                                     s2[0:32, n, 2:377],
                                     start=False, stop=True)
                tE = tep.tile([128, 2, 384], BF16, tag="tE")
                nc.scalar.activation(tE[0:128, :, 0:375],
                                     psE[0:128, :, 0:375], ACTF.Relu,
                                     bias=B2t[:])
                nc.vector.scalar_tensor_tensor(
                    s3[0:128, nb:nb + 2, 1:376],
                    psO[0:128, :, 0:375], B2t[:], tE[0:128, :, 0:375],
                    AOP.add, AOP.max)

            def conv3(qd):
                blk, nq = qd // 2, (qd % 2) * 4
                n0 = blk * BN + nq
                psE = psp.tile([128, 2, 512], F32, tag="ps")
                psO = psp.tile([128, 2, 512], F32, tag="ps")
                for t in range(7):
                    nc.tensor.matmul(
                        psE[0:120, 0, 0:252], Ws[f"W3E{t}"][:],
                        s3[0:128, nq:nq + 4, t:t + 373:6],
                        start=(t == 0), stop=(t == 6))
                for t in range(1, 8):
                    nc.tensor.matmul(
                        psO[0:120, 0, 0:252], Ws[f"W3O{t}"][:],
                        s3[0:128, nq:nq + 4, t:t + 373:6],
                        start=(t == 1), stop=(t == 7))
                tE = tep.tile([128, 512], BF16, tag="tE3")
                nc.scalar.activation(tE[0:120, 0:252], psE[0:120, 0, 0:252],
                                     ACTF.Relu, bias=B3t[:])
                nc.vector.scalar_tensor_tensor(
                    s4[0:120, n0:n0 + 4, 0:63],
                    psO[0:120, 0, 0:252].rearrange("p (n l) -> p n l", n=4),
                    B3t[:],
                    tE[0:120, 0:252].rearrange("p (n l) -> p n l", n=4),
                    AOP.add, AOP.max)

            def fc(half):
                c0 = half * 64
                psfc = psp.tile([128, 2, 512], F32, tag="ps")
                for w in range(63):
                    nc.tensor.matmul(
                        psfc[0:100, 0, 0:64],
                        Ws["WFC"][:, w * 100:(w + 1) * 100],
                        s4[:, c0:c0 + 64, w], start=(w == 0), stop=(w == 62))
                nc.scalar.copy(p0sb[:, c0:c0 + 64], psfc[0:100, 0, 0:64])

            # pipeline: conv0(p) | conv1(p-1) | conv2(p-2) |
            #           conv3((p-3)/2 @ odd p) | fc halves at p=35/66
            for p in range(68):
                if p < NPAIR:
                    if p % 4 == 0:
                        issue_xw(p // 4 + 2)
                    conv0(p)
                if 0 <= p - 1 < NPAIR:
                    conv1(p - 1)
                if 0 <= p - 2 < NPAIR:
                    conv2(p - 2)
                if p % 2 == 0 and 0 <= (p - 4) // 2 < 32:
                    conv3((p - 4) // 2)
                if p == 35 or p == 67:
                    fc(0 if p == 35 else 1)
                for op in cov_sched.get(p, ()):
                    emit_cov(op)
                if p in dma_sched:
                    dma_sched[p]()

            nc.sync.dma_start(out_p0[:], p0sb[:])
            nc.sync.dma_start(out_mom[:], momt[:])

    nc.compile()
    return nc


# ------------------------------------------------------------- launch 2 ----

def _build_launch2():
    nc = bacc.Bacc("TRN2", target_bir_lowering=False, debug=False,
                   num_devices=NCORES)
    dr = {}
    for nm, shape in [("featsT", [3, 7 * NS]), ("p0T", [100, NS]),
                      ("wcT", [3, 20]), ("bc", [20, 1]),
                      ("w0bT", [7, 20, 100]), ("bl0", [100, 1]),
                      ("wl1T", [100, 2]), ("bl1", [2, 1])]:
        dr[nm] = nc.dram_tensor(nm, shape, F32, kind="ExternalInput").ap()
    out2 = nc.dram_tensor("out2", [2, NS], F32, kind="ExternalOutput").ap()

    with tile.TileContext(nc) as tc:
        with tc.tile_pool(name="w2p", bufs=1) as wp, \
             tc.tile_pool(name="ps2", bufs=2, space="PSUM") as psp:
            fT = wp.tile([3, 7 * NS], F32, tag="fT")
            nc.sync.dma_start(fT[:], dr["featsT"][:])
            p0T = wp.tile([100, NS], F32, tag="p0T")
            nc.sync.dma_start(p0T[:], dr["p0T"][:])
            wcT = wp.tile([3, 20], F32, tag="wcT")
            nc.sync.dma_start(wcT[:], dr["wcT"][:])
            bc = wp.tile([20, 1], F32, tag="bc")
            nc.sync.dma_start(bc[:], dr["bc"][:])
            w0bT = [wp.tile([20, 100], F32, name=f"w0bT{t}", tag=f"w0bT{t}")
                    for t in range(7)]
            for t in range(7):
                nc.sync.dma_start(w0bT[t][:], dr["w0bT"][t])
            bl0 = wp.tile([100, 1], F32, tag="bl0")
            nc.sync.dma_start(bl0[:], dr["bl0"][:])
            wl1T = wp.tile([100, 2], F32, tag="wl1T")
            nc.sync.dma_start(wl1T[:], dr["wl1T"][:])
            bl1 = wp.tile([2, 1], F32, tag="bl1")
            nc.sync.dma_start(bl1[:], dr["bl1"][:])

            # h1 = relu(wc @ feats + bc): [20, (t, n)]
            h1 = wp.tile([20, 7 * NS], F32, tag="h1")
            for half in range(2):
                c0 = half * 448
                ps = psp.tile([32, 448], F32, tag="ph")
                nc.tensor.matmul(ps[0:20, :], wcT[:], fT[:, c0:c0 + 448],
                                 start=True, stop=True)
                nc.scalar.activation(h1[:, c0:c0 + 448], ps[0:20, :],
                                     ACTF.Relu, bias=bc[:])
            # z = relu(p0 + sum_t w0b_t.T @ h1_t + bl0)
            psz = psp.tile([100, NS], F32, tag="pz")
            for t in range(7):
                nc.tensor.matmul(psz[:], w0bT[t][:],
                                 h1[:, t * NS:(t + 1) * NS],
                                 start=(t == 0), stop=(t == 6))
            z = wp.tile([100, NS], F32, tag="z")
            nc.vector.scalar_tensor_tensor(z[:], psz[:], bl0[:], p0T[:],
                                           AOP.add, AOP.add)
            nc.vector.tensor_scalar_max(z[:], z[:], 0.0)
            pso = psp.tile([32, NS], F32, tag="po")
            nc.tensor.matmul(pso[0:2, :], wl1T[:], z[:],
                             start=True, stop=True)
            osb = wp.tile([2, NS], F32, tag="osb")
            nc.vector.tensor_scalar(osb[:], pso[0:2, :], bl1[:], None,
                                    AOP.add)
            nc.sync.dma_start(out2[:], osb[:])

    nc.compile()
    return nc


# --------------------------------------------------------------- kernel ----

def kernel(**inputs):
    ins = {k: np.asarray(v) for k, v in inputs.items()}
    x = ins["x"].astype(np.float32)

    if "l1" not in _CACHE:
        _CACHE["l1"] = _build_launch1()
    if "l2" not in _CACHE:
        _CACHE["l2"] = _build_launch2()
    w = _prep_weights(ins)

    xbf = x.astype(BF)
    xwE = np.zeros((28, x.shape[0], 750), BF)
    xwO = np.zeros((28, x.shape[0], 750), BF)
    for c in range(3):
        for j in range(9):
            # even cols: l = 8q - 1 + j ; odd cols: l = 8q + j
            if j == 0:
                xwE[c * 9 + 0, :, 1:750] = xbf[:, c, 7:5992:8]
            else:
                xwE[c * 9 + j] = xbf[:, c, j - 1::8]
            if j == 8:
                xwO[c * 9 + 8, :, 0:749] = xbf[:, c, 8:6000:8]
            else:
                xwO[c * 9 + j] = xbf[:, c, j::8]
    xwE[27] = 1.0
    xwO[27] = 1.0
    shards = [x[i * NS:(i + 1) * NS] for i in range(NCORES)]
    in1 = []
    for i, sh in enumerate(shards):
        sl = slice(i * NS, (i + 1) * NS)
        m = {"x_winE": np.ascontiguousarray(xwE[:, sl]),
             "x_winO": np.ascontiguousarray(xwO[:, sl]),
             "x_f32": sh}
        for nm in (["W0", "W1e1", "W1e2", "W1o1", "W1o2",
                    "W2e1", "W2e2", "W2o1", "W2o2", "WFC", "B2", "B3"] +
                   [f"W3E{t}" for t in range(7)] +
                   [f"W3O{t}" for t in range(1, 8)]):
            m[nm] = w[nm]
        m["INIT1"] = np.ones((1, BN, 750), BF)
        m["INITZ"] = np.zeros((128, BN, 1), BF)
        in1.append(m)
    t0 = time.time()
    res1 = run_bass_kernel_spmd(_CACHE["l1"], in1, list(range(NCORES)))
    LAST_EXEC_NS[0] = res1.exec_time_ns
    LAST_WALL_S[0] = time.time() - t0

    mom72 = np.concatenate([res1.results[i]["mom"] for i in range(NCORES)],
                           0).astype(np.float32)
    mom = mom72.reshape(-1, 9, 8).sum(-1)
    partial0 = np.concatenate(
        [res1.results[i]["partial0"].T for i in range(NCORES)], 0)

    # host: cov assembly (fp32) + LAPACK-clone eigh + global normalizers
    Sx = mom[:, 0:3].astype(np.float32)
    Sxx = mom[:, 3:6].astype(np.float32)
    Sxy = mom[:, 6:9].astype(np.float32)
    L = np.float32(L0)
    cov = np.empty((x.shape[0], 3, 3), np.float32)
    for idx, (c, dch) in enumerate([(0, 1), (0, 2), (1, 2)]):
        v = (Sxy[:, idx] - Sx[:, c] * Sx[:, dch] / L) / np.float32(L0 - 1)
        cov[:, c, dch] = v
        cov[:, dch, c] = v
    for c in range(3):
        cov[:, c, c] = (Sxx[:, c] - Sx[:, c] * Sx[:, c] / L) / np.float32(
            L0 - 1)
    vals, vecs = _eigh3_batch(cov)
    covn = cov / np.abs(cov).max()
    valsn = (vals / vals.max())[..., None]
    feats = np.concatenate([covn, valsn, vecs], axis=-1).astype(np.float32)

    in2 = []
    for i in range(NCORES):
        sl = slice(i * NS, (i + 1) * NS)
        m = {"featsT": np.ascontiguousarray(
                 feats[sl].transpose(1, 2, 0).reshape(3, 7 * NS)),
             "p0T": partial0[sl].T.copy(),
             "wcT": w["wcT"], "bc": w["bc"], "w0bT": w["w0bT"],
             "bl0": w["bl0"], "wl1T": w["wl1T"], "bl1": w["bl1"]}
        in2.append(m)
    t0 = time.time()
    res2 = run_bass_kernel_spmd(_CACHE["l2"], in2, list(range(NCORES)))
    LAST_EXEC_NS[1] = res2.exec_time_ns
    LAST_WALL_S[1] = time.time() - t0

    out = np.concatenate([res2.results[i]["out2"].T for i in range(NCORES)],
                         0).astype(np.float32)
    return (out[:, 0:1], out[:, 1:2])


# revision 17
# speedup vs baseline: 1.1867x; 1.0255x over previous
"""Trainium2 Bass kernel for nn_BAZ_Network (dense CNN + cov/eig head).

Data-parallel over 8 NeuronCores: 128 samples each.

Launch 1 (per core), software-pipelined over 64 sample-pairs:
  conv trunk as G-packed banded-weight matmuls (bf16, fp32 PSUM), with
  conv biases folded into the matmuls via a ones-row in the rhs (conv0,
  conv1).  Postprocess per (E,O) parity pair is two fused ops over a
  2-sample two-PSUM-bank 3D access pattern:
    op1 (Act):  tE = relu(psE + b)           PSUM -> SBUF bf16
    op2 (DVE):  s  = max(psO + b, tE)        = relu(max(E,O)+b), the
                 maxpool, relu, bias and bf16 cast in one instruction.
  Covariance fp32 raw moments (means on Act copy+accum, squares on Act
  Square+accum, cross-products on DVE stt+accum) are interleaved into
  the pair pipeline; the FC contraction of the conv features against
  wl0[:, :7500] runs per-block (125 accumulating matmuls of 8 cols).
  Stage stagger: conv0(p) | conv1(p-1) | conv2(p-2) | conv3 at odd p |
  FC at p%4==2, which hides the halo-DMA and PSUM-evacuation latency.
Host: branch-exact fp32 netlib-LAPACK ssyevd clone for the 3x3 eigh
  (required to reproduce jnp.linalg.eigh eigenvector signs).
Launch 2 (per core): eig-feature head: 1x1 conv (wc) + relu, remaining
  FC columns wl0[:, 7500:], bias+relu, final linear wl1.
"""

import os
import sys
import time
import numpy as np
import ml_dtypes

sys.path.insert(0, "/opt/trn_rl_repo")
os.environ["BASS_NEVER_TRACE"] = "1"

import concourse.bass as bass  # noqa: E402
import concourse.tile as tile  # noqa: E402
import concourse.mybir as mybir  # noqa: E402
from concourse import bacc  # noqa: E402
from concourse.bass_utils import run_bass_kernel_spmd  # noqa: E402

F32 = mybir.dt.float32
BF16 = mybir.dt.bfloat16
AOP = mybir.AluOpType
ACTF = mybir.ActivationFunctionType
BF = ml_dtypes.bfloat16

NCORES = 8
NS = 128          # samples per core
BN = 8            # samples per block
NBLK = NS // BN
NPAIR = NS // 2   # 64 sample-pairs, the pipeline unit
L0 = 6000

LAST_EXEC_NS = [None, None]
LAST_WALL_S = [None, None]
_CACHE = {}


# ---------------------------------------------------------------- eigh ----
# fp32 netlib-LAPACK ssyevd clone for n=3 (jobz='V', uplo='L').
# Matches jaxlib's CPU eigh (LAPACK >= 3.10 slartg) bit-closely: 0/3072
# eigenvector sign mismatches on the problem distribution.

_F = np.float32
_EPS = _F(np.finfo(np.float32).eps) * _F(0.5)
_EPS2 = _EPS * _EPS
_SAFMIN = _F(np.finfo(np.float32).tiny)


def _slapy2(x, y):
    xa, ya = abs(x), abs(y)
    w, z = max(xa, ya), min(xa, ya)
    if z == 0:
        return w
    return _F(w * _F(np.sqrt(_F(_F(1.0) + _F(_F(z / w) * _F(z / w))))))


def _sign(a, b):
    return abs(a) if b >= 0 else -abs(a)


def _slartg(f, g):
    if g == _F(0.0):
        return _F(1.0), _F(0.0), f
    if f == _F(0.0):
        return _F(0.0), _sign(_F(1.0), g), abs(g)
    d = _F(np.sqrt(_F(f * f + g * g)))
    c = _F(abs(f) / d)
    r = _sign(d, f)
    s = _F(g / r)
    return c, s, r


def _slaev2(a, b, c):
    sm = _F(a + c)
    df = _F(a - c)
    adf = abs(df)
    tb = _F(b + b)
    ab = abs(tb)
    acmx, acmn = (a, c) if abs(a) > abs(c) else (c, a)
    if adf > ab:
        t = _F(ab / adf)
        rt = _F(adf * _F(np.sqrt(_F(_F(1.0) + _F(t * t)))))
    elif adf < ab:
        t = _F(adf / ab)
        rt = _F(ab * _F(np.sqrt(_F(_F(1.0) + _F(t * t)))))
    else:
        rt = _F(ab * _F(np.sqrt(_F(2.0))))
    if sm < 0:
        rt1 = _F(_F(0.5) * _F(sm - rt))
        sgn1 = -1
        rt2 = _F(_F(_F(acmx / rt1) * acmn) - _F(_F(b / rt1) * b))
    elif sm > 0:
        rt1 = _F(_F(0.5) * _F(sm + rt))
        sgn1 = 1
        rt2 = _F(_F(_F(acmx / rt1) * acmn) - _F(_F(b / rt1) * b))
    else:
        rt1 = _F(_F(0.5) * rt)
        rt2 = _F(_F(-0.5) * rt)
        sgn1 = 1
    if df >= 0:
        cs = _F(df + rt)
        sgn2 = 1
    else:
        cs = _F(df - rt)
        sgn2 = -1
    acs = abs(cs)
    if acs > ab:
        ct = _F(-tb / cs)
        sn1 = _F(_F(1.0) / _F(np.sqrt(_F(_F(1.0) + _F(ct * ct)))))
        cs1 = _F(ct * sn1)
    else:
        if ab == 0:
            cs1, sn1 = _F(1.0), _F(0.0)
        else:
            tn = _F(-cs / tb)
            cs1 = _F(_F(1.0) / _F(np.sqrt(_F(_F(1.0) + _F(tn * tn)))))
            sn1 = _F(tn * cs1)
    if sgn1 == sgn2:
        cs1, sn1 = -sn1, cs1
    return rt1, rt2, cs1, sn1


def _ssytrd3(A):
    a00, a10, a20 = A[0, 0], A[1, 0], A[2, 0]
    a11, a21, a22 = A[1, 1], A[2, 1], A[2, 2]
    xnorm = abs(a20)
    if xnorm == _F(0.0):
        beta, v2, tau = a10, a20, _F(0.0)
    else:
        beta = -_sign(_slapy2(a10, xnorm), a10)
        tau = _F(_F(beta - a10) / beta)
        v2 = _F(a20 * _F(_F(1.0) / _F(a10 - beta)))
    e0 = beta
    if tau != _F(0.0):
        x0 = _F(_F(tau * a11) + _F(tau * _F(a21 * v2)))
        x1 = _F(_F(tau * a21) + _F(_F(tau * v2) * a22))
        sdot = _F(_F(x0 * _F(1.0)) + _F(x1 * v2))
        alpha = _F(_F(_F(-0.5) * tau) * sdot)
        w0 = _F(x0 + _F(alpha * _F(1.0)))
        w1 = _F(x1 + _F(alpha * v2))
        t1, t2 = -w0, _F(-1.0)
        a11 = _F(_F(a11 + _F(_F(1.0) * t1)) + _F(w0 * t2))
        a21 = _F(_F(a21 + _F(v2 * t1)) + _F(w1 * t2))
        t1b, t2b = -w1, -v2
        a22 = _F(_F(a22 + _F(v2 * t1b)) + _F(w1 * t2b))
    d = np.array([a00, a11, a22], np.float32)
    e = np.array([e0, a21, 0.0], np.float32)
    return d, e, v2, tau


def _ssteqr3(d, e):
    n = 3
    Z = np.eye(3, dtype=np.float32)
    wc = np.zeros(2, np.float32)
    ws = np.zeros(2, np.float32)
    nmaxit, jtot = 90, 0

    def lasr_b(l, m):
        for j in range(m - 1, l - 1, -1):
            c, s = wc[j - 1], ws[j - 1]
            if c != _F(1.0) or s != _F(0.0):
                for i in range(3):
                    t = Z[i, j]
                    Z[i, j] = _F(_F(c * t) - _F(s * Z[i, j - 1]))
                    Z[i, j - 1] = _F(_F(s * t) + _F(c * Z[i, j - 1]))

    def lasr_f(m, l):
        for j in range(m, l):
            c, s = wc[j - 1], ws[j - 1]
            if c != _F(1.0) or s != _F(0.0):
                for i in range(3):
                    t = Z[i, j]
                    Z[i, j] = _F(_F(c * t) - _F(s * Z[i, j - 1]))
                    Z[i, j - 1] = _F(_F(s * t) + _F(c * Z[i, j - 1]))

    l1 = 1
    while True:
        if l1 > n:
            break
        if l1 > 1:
            e[l1 - 2] = _F(0.0)
        m = n
        for mm in range(l1, n):
            tst = abs(e[mm - 1])
            if tst == _F(0.0):
                m = mm
                break
            if tst <= _F(_F(_F(np.sqrt(abs(d[mm - 1]))) *
                            _F(np.sqrt(abs(d[mm])))) * _EPS):
                e[mm - 1] = _F(0.0)
                m = mm
                break
        l = l1
        lend = m
        l1 = m + 1
        if lend == l:
            continue
        if abs(d[lend - 1]) < abs(d[l - 1]):
            lend, l = l, lend
        if lend > l:
            while True:  # QL
                m = lend
                if l != lend:
                    for mm in range(l, lend):
                        tst = _F(abs(e[mm - 1]) * abs(e[mm - 1]))
                        if tst <= _F(_F(_F(_EPS2 * abs(d[mm - 1])) *
                                        abs(d[mm])) + _SAFMIN):
                            m = mm
                            break
                if m < lend:
                    e[m - 1] = _F(0.0)
                p = d[l - 1]
                if m == l:
                    d[l - 1] = p
                    l += 1
                    if l <= lend:
                        continue
                    break
                if m == l + 1:
                    rt1, rt2, c, s = _slaev2(d[l - 1], e[l - 1], d[l])
                    wc[l - 1] = c
                    ws[l - 1] = s
                    lasr_b(l, l + 1)
                    d[l - 1] = rt1
                    d[l] = rt2
                    e[l - 1] = _F(0.0)
                    l += 2
                    if l <= lend:
                        continue
                    break
                if jtot == nmaxit:
                    break
                jtot += 1
                g = _F(_F(d[l] - p) / _F(_F(2.0) * e[l - 1]))
                r = _slapy2(g, _F(1.0))
                g = _F(_F(d[m - 1] - p) + _F(e[l - 1] / _F(g + _sign(r, g))))
                s = _F(1.0)
                c = _F(1.0)
                p = _F(0.0)
                for i in range(m - 1, l - 1, -1):
                    f = _F(s * e[i - 1])
                    b = _F(c * e[i - 1])
                    c, s, r = _slartg(g, f)
                    if i != m - 1:
                        e[i] = r
                    g = _F(d[i] - p)
                    r = _F(_F(_F(d[i - 1] - g) * s) + _F(_F(_F(2.0) * c) * b))
                    p = _F(s * r)
                    d[i] = _F(g + p)
                    g = _F(_F(c * r) - b)
                    wc[i - 1] = c
                    ws[i - 1] = -s
                lasr_b(l, m)
                d[l - 1] = _F(d[l - 1] - p)
                e[l - 1] = g
        else:
            while True:  # QR
                m = lend
                if l != lend:
                    for mm in range(l, lend, -1):
                        tst = _F(abs(e[mm - 2]) * abs(e[mm - 2]))
                        if tst <= _F(_F(_F(_EPS2 * abs(d[mm - 1])) *
                                        abs(d[mm - 2])) + _SAFMIN):
                            m = mm
                            break
                if m > lend:
                    e[m - 2] = _F(0.0)
                p = d[l - 1]
                if m == l:
                    d[l - 1] = p
                    l -= 1
                    if l >= lend:
                        continue
                    break
                if m == l - 1:
                    rt1, rt2, c, s = _slaev2(d[l - 2], e[l - 2], d[l - 1])
                    wc[m - 1] = c
                    ws[m - 1] = s
                    lasr_f(m, l)
                    d[l - 2] = rt1
                    d[l - 1] = rt2
                    e[l - 2] = _F(0.0)
                    l -= 2
                    if l >= lend:
                        continue
                    break
                if jtot == nmaxit:
                    break
                jtot += 1
                g = _F(_F(d[l - 2] - p) / _F(_F(2.0) * e[l - 2]))
                r = _slapy2(g, _F(1.0))
                g = _F(_F(d[m - 1] - p) + _F(e[l - 2] / _F(g + _sign(r, g))))
                s = _F(1.0)
                c = _F(1.0)
                p = _F(0.0)
                for i in range(m, l):
                    f = _F(s * e[i - 1])
                    b = _F(c * e[i - 1])
                    c, s, r = _slartg(g, f)
                    if i != m:
                        e[i - 2] = r
                    g = _F(d[i - 1] - p)
                    r = _F(_F(_F(d[i] - g) * s) + _F(_F(_F(2.0) * c) * b))
                    p = _F(s * r)
                    d[i - 1] = _F(g + p)
                    g = _F(_F(c * r) - b)
                    wc[i - 1] = c
                    ws[i - 1] = s
                lasr_f(m, l)
                d[l - 1] = _F(d[l - 1] - p)
                e[l - 2] = g
        if jtot >= nmaxit:
            break
    for ii in range(2, n + 1):
        i = ii - 1
        k = i
        p = d[i - 1]
        for j in range(ii, n + 1):
            if d[j - 1] < p:
                k = j
                p = d[j - 1]
        if k != i:
            d[k - 1] = d[i - 1]
            d[i - 1] = p
            tmp = Z[:, k - 1].copy()
            Z[:, k - 1] = Z[:, i - 1]
            Z[:, i - 1] = tmp
    return d, Z


def _eigh3_batch(covs):
    n = covs.shape[0]
    W = np.empty((n, 3), np.float32)
    V = np.empty((n, 3, 3), np.float32)
    for i in range(n):
        d, e, v2, tau = _ssytrd3(covs[i])
        w, Z = _ssteqr3(d, e)
        if tau != _F(0.0):
            for j in range(3):
                vtz = _F(Z[1, j] + _F(v2 * Z[2, j]))
                tvz = _F(tau * vtz)
                Z[1, j] = _F(Z[1, j] - tvz)
                Z[2, j] = _F(Z[2, j] - _F(v2 * tvz))
        W[i] = w
        V[i] = Z
    return W, V


# ------------------------------------------------------------- weights ----

def _prep_weights(ins):
    """Host-side packing of the model weights into device layouts.

    Strided column-pair scheme (see the layer maps below); conv0/conv1
    biases ride a ones-row in the rhs so the matmul itself adds them.
    """
    w0, w1, w2, w3 = ins["w0"], ins["w1"], ins["w2"], ins["w3"]
    b0, b1 = np.asarray(ins["b0"], np.float32), np.asarray(ins["b1"],
                                                           np.float32)

    d = {}
    # conv0: window rows (c:3, j:9): even cols l = 8q-1+j, odd l = 8q+j;
    # k = j - 2g for output slot g; row 27 = ones -> bias.
    W0 = np.zeros((28, 80), np.float32)
    for c in range(3):
        for j in range(9):
            for g in range(4):
                k = j - 2 * g
                if 0 <= k < 3:
                    for o in range(20):
                        W0[c * 9 + j, g * 20 + o] = w0[o, c, k]
    W0[27, :] = np.tile(b0, 4)
    d["W0"] = W0.astype(BF)

    def s1_rows(with_hl, with_hr):
        rows = [(g * 20, 20, g) for g in range(4)]
        if with_hl:
            rows.append((80, 20, -1))
        if with_hr:
            rows.append((100, 20, 4))
        return rows

    def mk(w, blocks, Ghalf, parity, Cout, shift, colbase=None):
        Cin = w.shape[1]
        K = max(rb + Cin for rb, _, _ in blocks)
        if colbase is None:
            colbase = [g * Cout for g in range(Ghalf)]
        W = np.zeros((K, max(colbase) + Cout), np.float32)
        for rb, _, lrel in blocks:
            for g in range(Ghalf):
                pos = 2 * g + parity
                k = (lrel + shift) - pos + 1
                if 0 <= k < 3:
                    for ci in range(Cin):
                        W[rb + ci, colbase[g] + np.arange(Cout)] = w[:, ci, k]
        return W

    # conv1 output M-order: g0->0, g1->64, g2->96, g3->32 (C1B) so conv2's
    # boundary reads sit at legal rhs bases.
    C1B = [0, 64, 96, 32]

    # baseline-layout W1 blocks, then re-rowed for the s1 layout with the
    # ones row at 80: main [0:80], ones 80, hl [81:101], hr [101:121].
    w1e1_base = mk(w1, s1_rows(True, False), 4, 0, 32, 0, C1B)   # [100,128]
    w1e2 = mk(w1, [(rb, 20, lr + 4) for rb, _, lr in
                   s1_rows(False, False)], 4, 0, 32, 0, C1B)     # [80,128]
    w1o1 = mk(w1, s1_rows(False, False), 4, 1, 32, 0, C1B)       # [80,128]
    w1o2_blocks = ([(g * 20, 20, g + 4) for g in range(4)] +
                   [(80, 20, 1000), (100, 20, 8)])
    w1o2_base = mk(w1, w1o2_blocks, 4, 1, 32, 0, C1B)            # [120,128]
    b1t = np.tile(b1, 4)
    W1e1 = np.zeros((101, 128), np.float32)
    W1e1[0:80] = w1e1_base[0:80]
    W1e1[80] = b1t
    W1e1[81:101] = w1e1_base[80:100]
    W1o2 = np.zeros((121, 128), np.float32)
    W1o2[0:80] = w1o2_base[0:80]
    W1o2[80] = b1t
    W1o2[101:121] = w1o2_base[100:120]
    d["W1e1"] = W1e1.astype(BF)
    d["W1e2"] = w1e2.astype(BF)
    d["W1o1"] = w1o1.astype(BF)
    d["W1o2"] = W1o2.astype(BF)

    # conv2 (G=4, Ghalf=2, Cout=64): stored2 rows (g:4, o:32)->128
    s2_main = [(0, 32, 0), (64, 32, 1), (96, 32, 2), (32, 32, 3)]
    d["W2e1"] = mk(w2, [(0, 32, -1)], 2, 0, 64, 0).astype(BF)
    d["W2e2"] = mk(w2, s2_main, 2, 0, 64, 0).astype(BF)
    d["W2o1"] = mk(w2, s2_main, 2, 1, 64, 0).astype(BF)
    d["W2o2"] = mk(w2, [(0, 32, 4)], 2, 1, 64, 0).astype(BF)

    # conv3 (G=12, Ghalf=6, Cout=20): stored3 rows (g:2, o:64)->128.
    # Window w covers pre-pool pos [12w, 12w+12); MM t reads s3 col 6w+t
    # (l3 = 12w + 2t - 2 + g); M = (h:6, o:20) = 120.
    for t in range(7):
        d[f"W3E{t}"] = mk(w3, [(0, 64, 2 * t - 2), (64, 64, 2 * t - 1)],
                          6, 0, 20, 0).astype(BF)
    for t in range(1, 8):
        d[f"W3O{t}"] = mk(w3, [(0, 64, 2 * t - 2), (64, 64, 2 * t - 1)],
                          6, 1, 20, 0).astype(BF)

    # fc: stored4 rows (h:6, o:20)->120, col w: feature (o, l4 = 6w + h)
    wl0 = ins["wl0"]
    WFC = np.zeros((120, 63 * 100), np.float32)
    for w in range(63):
        for h in range(6):
            l4 = 6 * w + h
            if l4 < 375:
                WFC[h * 20:(h + 1) * 20, w * 100:(w + 1) * 100] = \
                    wl0[:, np.arange(20)[:, None] * 375 + l4].T.reshape(
                        20, 100)
    d["WFC"] = WFC.astype(BF)

    d["B2"] = np.tile(ins["b2"], 2).astype(np.float32)[:, None]   # [128]
    d["B3"] = np.tile(ins["b3"], 6).astype(np.float32)[:, None]   # [120]
    # launch 2
    d["wcT"] = ins["wc"][:, :, 0].T.astype(np.float32).copy()      # [3, 20]
    d["bc"] = ins["bc"].astype(np.float32)[:, None]                # [20, 1]
    w0b = np.zeros((7, 20, 100), np.float32)
    for t in range(7):
        for o in range(20):
            w0b[t, o] = ins["wl0"][:, 7500 + o * 7 + t]
    d["w0bT"] = w0b
    d["bl0"] = ins["bl0"].astype(np.float32)[:, None]              # [100, 1]
    d["wl1T"] = ins["wl1"].T.astype(np.float32).copy()             # [100, 2]
    d["bl1"] = ins["bl1"].astype(np.float32)[:, None]              # [2, 1]
    return d


# ------------------------------------------------------------- launch 1 ----

def _build_launch1():
    nc = bacc.Bacc("TRN2", target_bir_lowering=False, debug=False,
                   num_devices=NCORES)
    dram = {}
    for nm, shape, dt in [
        ("x_winE", [28, NS, 750], BF16), ("x_winO", [28, NS, 750], BF16),
        ("x_f32", [NS, 3, L0], F32),
        ("W0", [28, 80], BF16),
        ("W1e1", [101, 128], BF16), ("W1e2", [80, 128], BF16),
        ("W1o1", [80, 128], BF16), ("W1o2", [121, 128], BF16),
        ("W2e1", [32, 128], BF16), ("W2e2", [128, 128], BF16),
        ("W2o1", [128, 128], BF16), ("W2o2", [32, 128], BF16),
    ] + [(f"W3E{t}", [128, 120], BF16) for t in range(7)] + \
        [(f"W3O{t}", [128, 120], BF16) for t in range(1, 8)] + [
        ("WFC", [120, 6300], BF16),
        ("B2", [128, 1], F32), ("B3", [120, 1], F32),
        ("INIT1", [1, BN, 750], BF16), ("INITZ", [128, BN, 1], BF16),
    ]:
        dram[nm] = nc.dram_tensor(nm, shape, dt, kind="ExternalInput").ap()
    out_p0 = nc.dram_tensor("partial0", [100, NS], F32,
                            kind="ExternalOutput").ap()
    out_mom = nc.dram_tensor("mom", [NS, 72], F32,
                             kind="ExternalOutput").ap()

    with tile.TileContext(nc) as tc:
        with tc.tile_pool(name="wpool", bufs=1) as wp, \
             tc.tile_pool(name="covp", bufs=1) as cvp, \
             tc.tile_pool(name="scrp", bufs=4) as scp, \
             tc.tile_pool(name="xw", bufs=2) as xwp, \
             tc.tile_pool(name="s1p", bufs=1) as s1p, \
             tc.tile_pool(name="s2p", bufs=1) as s2p, \
             tc.tile_pool(name="s3p", bufs=1) as s3p, \
             tc.tile_pool(name="s4p", bufs=1) as s4p, \
             tc.tile_pool(name="tep", bufs=4) as tep, \
             tc.tile_pool(name="ps", bufs=4, space="PSUM") as psp:

            xw_tiles = {}

            def issue_xw(b):
                if b >= NBLK or b in xw_tiles:
                    return
                n0 = b * BN
                te = xwp.tile([28, BN, 750], BF16, tag="xwE")
                nc.sync.dma_start(te[:], dram["x_winE"][:, n0:n0 + BN, :])
                to = xwp.tile([28, BN, 750], BF16, tag="xwO")
                nc.sync.dma_start(to[:], dram["x_winO"][:, n0:n0 + BN, :])
                xw_tiles[b] = (te, to)

            # The SP DMA queue is FIFO and a DMA holds HWDGE ~0.6us each:
            # preload ONLY what conv0(0)/conv1(0) need, trickle the rest
            # into the pipeline steps below via dma_sched.
            momt = cvp.tile([NS, 72], F32, tag="mom")
            p0sb = cvp.tile([100, NS], F32, tag="p0sb")
            s1 = s1p.tile([121, BN, 750], BF16, tag="s1")
            s2 = s2p.tile([128, BN, 377], BF16, tag="s2")
            s3 = s3p.tile([128, BN, 380], BF16, tag="s3")
            s4 = s4p.tile([120, NS, 63], BF16, tag="s4")
            xcv = cvp.tile([NS, 3, L0], F32, tag="xcv")

            Ws = {}
            for nm in (["W1e1", "W1e2", "W1o1", "W1o2", "W0",
                        "W2e2", "W2o1", "W2o2"] +
                       [f"W3E{t}" for t in range(7)] +
                       [f"W3O{t}" for t in range(1, 8)] + ["WFC"]):
                Ws[nm] = wp.tile(list(dram[nm].shape), BF16, name=nm, tag=nm)
            Wpad = wp.tile([64, 128], BF16, name="W2e1", tag="W2e1")
            Ws["W2e1"] = Wpad[32:64]
            B2t = wp.tile([128, 1], F32, tag="B2")
            B3t = wp.tile([120, 1], F32, tag="B3")

            def dma_w(nm):
                nc.sync.dma_start(Ws[nm][:], dram[nm][:])

            # prologue: conv0(0..1) + conv1(0) prerequisites only
            dma_w("W0")
            issue_xw(0)
            issue_xw(1)
            # ones row + hl/hr edges (engines cannot address partition
            # bases off the 0/32/64/96 grid; DMA can)
            nc.sync.dma_start(s1[80:81, :, :], dram["INIT1"][:])
            nc.sync.dma_start(s1[81:101, :, 0:1], dram["INITZ"][0:20])
            nc.sync.dma_start(s1[101:121, :, 749:750], dram["INITZ"][0:20])
            for nm in ["W1e1", "W1e2", "W1o1", "W1o2"]:
                dma_w(nm)

            def dmas_step0():
                nc.sync.dma_start(Wpad[32:64], dram["W2e1"][:])
                for nm in ["W2e2", "W2o1", "W2o2"]:
                    dma_w(nm)
                nc.sync.dma_start(B2t[:], dram["B2"][:])
                nc.sync.dma_start(s2[:, :, 0:1], dram["INITZ"][:])
                nc.sync.dma_start(s2[:, :, 376:377], dram["INITZ"][:])

            def dmas_step1():
                for t in range(7):
                    dma_w(f"W3E{t}")

            def dmas_step2():
                for t in range(1, 8):
                    dma_w(f"W3O{t}")
                nc.sync.dma_start(B3t[:], dram["B3"][:])
                nc.sync.dma_start(s3[:, :, 0:1], dram["INITZ"][:])
                for cz in range(376, 380):
                    nc.sync.dma_start(s3[:, :, cz:cz + 1], dram["INITZ"][:])

            def make_xcv_piece(i):
                c0 = i * 750
                return lambda: nc.sync.dma_start(
                    xcv[:, :, c0:c0 + 750], dram["x_f32"][:, :, c0:c0 + 750])

            def make_wfc_piece(i):
                c0 = i * 3150
                return lambda: nc.sync.dma_start(
                    Ws["WFC"][:, c0:c0 + 3150], dram["WFC"][:, c0:c0 + 3150])

            dma_sched = {0: dmas_step0, 1: dmas_step1, 2: dmas_step2}
            for i in range(8):
                dma_sched[3 + i] = make_xcv_piece(i)
            dma_sched[12] = make_wfc_piece(0)
            dma_sched[14] = make_wfc_piece(1)

            # covariance moment ops, chunked 8x750 so no single op can
            # head-of-line block the Act/DVE queues; 72 partial sums,
            # host adds the 8 chunks per moment.
            cov_ops = []
            for ch in range(8):
                for c in range(3):
                    cov_ops.append(("mean", c, ch))
                for c in range(3):
                    cov_ops.append(("sq", c, ch))
                for k, (c, dch) in enumerate([(0, 1), (0, 2), (1, 2)]):
                    cov_ops.append(("xy", k, c, dch, ch))
            cov_sched = {}
            for i, op in enumerate(cov_ops):
                cov_sched.setdefault(12 + i // 2, []).append(op)

            def emit_cov(op):
                ch = op[-1]
                c0 = ch * 750
                sl = slice(c0, c0 + 750)
                scr = scp.tile([NS, 750], BF16, tag="cscr")
                if op[0] == "mean":
                    c = op[1]
                    nc.scalar.activation(scr[:], xcv[:, c, sl], ACTF.Copy,
                                         accum_out=momt[:, c * 8 + ch:
                                                        c * 8 + ch + 1])
                elif op[0] == "sq":
                    c = op[1]
                    nc.scalar.activation(scr[:], xcv[:, c, sl], ACTF.Square,
                                         accum_out=momt[:, 24 + c * 8 + ch:
                                                        24 + c * 8 + ch + 1])
                else:
                    _, k, c, dch, ch = op
                    nc.vector.scalar_tensor_tensor(
                        scr[:], xcv[:, c, sl], 1.0, xcv[:, dch, sl],
                        AOP.mult, AOP.mult,
                        accum_out=momt[:, 48 + k * 8 + ch:
                                       48 + k * 8 + ch + 1])

            def conv0(p):
                blk, nb = p // 4, (p % 4) * 2
                xwE, xwO = xw_tiles[blk]
                for ch in range(2):
                    c0 = ch * 375
                    psE = psp.tile([128, 2, 512], F32, tag="ps")
                    psO = psp.tile([128, 2, 512], F32, tag="ps")
                    for i in range(2):
                        nc.tensor.matmul(psE[0:80, i, 0:375], Ws["W0"][:],
                                         xwE[:, nb + i, c0:c0 + 375],
                                         start=True, stop=True)
                    for i in range(2):
                        nc.tensor.matmul(psO[0:80, i, 0:375], Ws["W0"][:],
                                         xwO[:, nb + i, c0:c0 + 375],
                                         start=True, stop=True)
                    tE = tep.tile([128, 2, 384], BF16, tag="tE")
                    nc.scalar.activation(tE[0:80, :, 0:375],
                                         psE[0:80, :, 0:375], ACTF.Relu)
                    nc.vector.scalar_tensor_tensor(
                        s1[0:80, nb:nb + 2, c0:c0 + 375],
                        psO[0:80, :, 0:375], 0.0, tE[0:80, :, 0:375],
                        AOP.max, AOP.max)
                # per-pair halo rows for conv1
                nc.sync.dma_start(s1[81:101, nb:nb + 2, 1:750],
                                  s1[60:80, nb:nb + 2, 0:749])
                nc.sync.dma_start(s1[101:121, nb:nb + 2, 0:749],
                                  s1[0:20, nb:nb + 2, 1:750])

            def conv1(p):
                nb = (p % 4) * 2
                psE = psp.tile([128, 2, 512], F32, tag="ps")
                psO = psp.tile([128, 2, 512], F32, tag="ps")
                for i in range(2):
                    n = nb + i
                    nc.tensor.matmul(psE[0:128, i, 0:375], Ws["W1e1"][:],
                                     s1[0:101, n, 0:750:2],
                                     start=True, stop=False)
                    nc.tensor.matmul(psE[0:128, i, 0:375], Ws["W1e2"][:],
                                     s1[0:80, n, 1:750:2],
                                     start=False, stop=True)
                for i in range(2):
                    n = nb + i
                    nc.tensor.matmul(psO[0:128, i, 0:375], Ws["W1o1"][:],
                                     s1[0:80, n, 0:750:2],
                                     start=True, stop=False)
                    nc.tensor.matmul(psO[0:128, i, 0:375], Ws["W1o2"][:],
                                     s1[0:121, n, 1:750:2],
                                     start=False, stop=True)
                tE = tep.tile([128, 2, 384], BF16, tag="tE")
                nc.scalar.activation(tE[0:128, :, 0:375],
                                     psE[0:128, :, 0:375], ACTF.Relu)
                nc.vector.scalar_tensor_tensor(
                    s2[0:128, nb:nb + 2, 1:376],
                    psO[0:128, :, 0:375], 0.0, tE[0:128, :, 0:375],
                    AOP.max, AOP.max)

            def conv2(p):
                nb = (p % 4) * 2
                psE = psp.tile([128, 2, 512], F32, tag="ps")
                psO = psp.tile([128, 2, 512], F32, tag="ps")
                for i in range(2):
                    n = nb + i
                    nc.tensor.matmul(psE[0:128, i, 0:375], Ws["W2e1"],
                                     s2[32:64, n, 0:375],
                                     start=True, stop=False)
                    nc.tensor.matmul(psE[0:128, i, 0:375], Ws["W2e2"][:],
                                     s2[0:128, n, 1:376],
                                     start=False, stop=True)
                for i in range(2):
                    n = nb + i
                    nc.tensor.matmul(psO[0:128, i, 0:375], Ws["W2o1"][:],
                                     s2[0:128, n, 1:376],
                                     start=True, stop=False)
                    nc.tensor.matmul(psO[0:128, i, 0:375], Ws["W2o2"][:],
                                     s2[0:32, n, 2:377],
                                     start=False, stop=True)
                tE = tep.tile([128, 2, 384], BF16, tag="tE")
                nc.scalar.activation(tE[0:128, :, 0:375],
                                     psE[0:128, :, 0:375], ACTF.Relu,
                                     bias=B2t[:])
                nc.vector.scalar_tensor_tensor(
                    s3[0:128, nb:nb + 2, 1:376],
                    psO[0:128, :, 0:375], B2t[:], tE[0:128, :, 0:375],
                    AOP.add, AOP.max)

            def conv3(qd):
                blk, nq = qd // 2, (qd % 2) * 4
                n0 = blk * BN + nq
                psE = psp.tile([128, 2, 512], F32, tag="ps")
                psO = psp.tile([128, 2, 512], F32, tag="ps")
                for t in range(7):
                    nc.tensor.matmul(
                        psE[0:120, 0, 0:252], Ws[f"W3E{t}"][:],
                        s3[0:128, nq:nq + 4, t:t + 373:6],
                        start=(t == 0), stop=(t == 6))
                for t in range(1, 8):
                    nc.tensor.matmul(
                        psO[0:120, 0, 0:252], Ws[f"W3O{t}"][:],
                        s3[0:128, nq:nq + 4, t:t + 373:6],
                        start=(t == 1), stop=(t == 7))
                tE = tep.tile([128, 512], BF16, tag="tE3")
                nc.scalar.activation(tE[0:120, 0:252], psE[0:120, 0, 0:252],
                                     ACTF.Relu, bias=B3t[:])
                nc.vector.scalar_tensor_tensor(
                    s4[0:120, n0:n0 + 4, 0:63],
                    psO[0:120, 0, 0:252].rearrange("p (n l) -> p n l", n=4),
                    B3t[:],
                    tE[0:120, 0:252].rearrange("p (n l) -> p n l", n=4),
                    AOP.add, AOP.max)

            def fc(half):
                c0 = half * 64
                psfc = psp.tile([128, 2, 512], F32, tag="ps")
                for w in range(63):
                    nc.tensor.matmul(
                        psfc[0:100, 0, 0:64],
                        Ws["WFC"][:, w * 100:(w + 1) * 100],
                        s4[:, c0:c0 + 64, w], start=(w == 0), stop=(w == 62))
                nc.scalar.copy(p0sb[:, c0:c0 + 64], psfc[0:100, 0, 0:64])

            # pipeline: conv0(p) | conv1(p-1) | conv2(p-2) |
            #           conv3((p-3)/2 @ odd p) | fc halves at p=35/66
            for p in range(68):
                if p < NPAIR:
                    if p % 4 == 0:
                        issue_xw(p // 4 + 2)
                    conv0(p)
                if 0 <= p - 1 < NPAIR:
                    conv1(p - 1)
                if 0 <= p - 2 < NPAIR:
                    conv2(p - 2)
                if p % 2 == 0 and 0 <= (p - 4) // 2 < 32:
                    conv3((p - 4) // 2)
                if p == 35 or p == 67:
                    fc(0 if p == 35 else 1)
                for op in cov_sched.get(p, ()):
                    emit_cov(op)
                if p in dma_sched:
                    dma_sched[p]()

            nc.sync.dma_start(out_p0[:], p0sb[:])
            nc.sync.dma_start(out_mom[:], momt[:])

    nc.compile()
    return nc


# ------------------------------------------------------------- launch 2 ----

def _build_launch2():
    nc = bacc.Bacc("TRN2", target_bir_lowering=False, debug=False,
                   num_devices=NCORES)
    dr = {}
    for nm, shape in [("featsT", [3, 7 * NS]), ("p0T", [100, NS]),
                      ("wcT", [3, 20]), ("bc", [20, 1]),
                      ("w0bT", [7, 20, 100]), ("bl0", [100, 1]),
                      ("wl1T", [100, 2]), ("bl1", [2, 1])]:
        dr[nm] = nc.dram_tensor(nm, shape, F32, kind="ExternalInput").ap()
    out2 = nc.dram_tensor("out2", [2, NS], F32, kind="ExternalOutput").ap()

    with tile.TileContext(nc) as tc:
        with tc.tile_pool(name="w2p", bufs=1) as wp, \
             tc.tile_pool(name="ps2", bufs=2, space="PSUM") as psp:
            fT = wp.tile([3, 7 * NS], F32, tag="fT")
            nc.sync.dma_start(fT[:], dr["featsT"][:])
            p0T = wp.tile([100, NS], F32, tag="p0T")
            nc.sync.dma_start(p0T[:], dr["p0T"][:])
            wcT = wp.tile([3, 20], F32, tag="wcT")
            nc.sync.dma_start(wcT[:], dr["wcT"][:])
            bc = wp.tile([20, 1], F32, tag="bc")
            nc.sync.dma_start(bc[:], dr["bc"][:])
            w0bT = [wp.tile([20, 100], F32, name=f"w0bT{t}", tag=f"w0bT{t}")
                    for t in range(7)]
            for t in range(7):
                nc.sync.dma_start(w0bT[t][:], dr["w0bT"][t])
            bl0 = wp.tile([100, 1], F32, tag="bl0")
            nc.sync.dma_start(bl0[:], dr["bl0"][:])
            wl1T = wp.tile([100, 2], F32, tag="wl1T")
            nc.sync.dma_start(wl1T[:], dr["wl1T"][:])
            bl1 = wp.tile([2, 1], F32, tag="bl1")
            nc.sync.dma_start(bl1[:], dr["bl1"][:])

            # h1 = relu(wc @ feats + bc): [20, (t, n)]
            h1 = wp.tile([20, 7 * NS], F32, tag="h1")
            for half in range(2):
                c0 = half * 448
                ps = psp.tile([32, 448], F32, tag="ph")
                nc.tensor.matmul(ps[0:20, :], wcT[:], fT[:, c0:c0 + 448],
                                 start=True, stop=True)
                nc.scalar.activation(h1[:, c0:c0 + 448], ps[0:20, :],
                                     ACTF.Relu, bias=bc[:])
            # z = relu(p0 + sum_t w0b_t.T @ h1_t + bl0)
            psz = psp.tile([100, NS], F32, tag="pz")
            for t in range(7):
                nc.tensor.matmul(psz[:], w0bT[t][:],
                                 h1[:, t * NS:(t + 1) * NS],
                                 start=(t == 0), stop=(t == 6))
            z = wp.tile([100, NS], F32, tag="z")
            nc.vector.scalar_tensor_tensor(z[:], psz[:], bl0[:], p0T[:],
                                           AOP.add, AOP.add)
            nc.vector.tensor_scalar_max(z[:], z[:], 0.0)
            pso = psp.tile([32, NS], F32, tag="po")
            nc.tensor.matmul(pso[0:2, :], wl1T[:], z[:],
                             start=True, stop=True)
            osb = wp.tile([2, NS], F32, tag="osb")
            nc.vector.tensor_scalar(osb[:], pso[0:2, :], bl1[:], None,
                                    AOP.add)
            nc.sync.dma_start(out2[:], osb[:])

    nc.compile()
    return nc


# --------------------------------------------------------------- kernel ----

def kernel(**inputs):
    ins = {k: np.asarray(v) for k, v in inputs.items()}
    x = ins["x"].astype(np.float32)

    if "l1" not in _CACHE:
        _CACHE["l1"] = _build_launch1()
    if "l2" not in _CACHE:
        _CACHE["l2"] = _build_launch2()
    w = _prep_weights(ins)

    xbf = x.astype(BF)
    xwE = np.zeros((28, x.shape[0], 750), BF)
    xwO = np.zeros((28, x.shape[0], 750), BF)
    for c in range(3):
        for j in range(9):
            # even cols: l = 8q - 1 + j ; odd cols: l = 8q + j
            if j == 0:
                xwE[c * 9 + 0, :, 1:750] = xbf[:, c, 7:5992:8]
            else:
                xwE[c * 9 + j] = xbf[:, c, j - 1::8]
            if j == 8:
                xwO[c * 9 + 8, :, 0:749] = xbf[:, c, 8:6000:8]
            else:
                xwO[c * 9 + j] = xbf[:, c, j::8]
    xwE[27] = 1.0
    xwO[27] = 1.0
    shards = [x[i * NS:(i + 1) * NS] for i in range(NCORES)]
    in1 = []
    for i, sh in enumerate(shards):
        sl = slice(i * NS, (i + 1) * NS)
        m = {"x_winE": np.ascontiguousarray(xwE[:, sl]),
             "x_winO": np.ascontiguousarray(xwO[:, sl]),
             "x_f32": sh}
        for nm in (["W0", "W1e1", "W1e2", "W1o1", "W1o2",
                    "W2e1", "W2e2", "W2o1", "W2o2", "WFC", "B2", "B3"] +
                   [f"W3E{t}" for t in range(7)] +
                   [f"W3O{t}" for t in range(1, 8)]):
            m[nm] = w[nm]
        m["INIT1"] = np.ones((1, BN, 750), BF)
        m["INITZ"] = np.zeros((128, BN, 1), BF)
        in1.append(m)
    t0 = time.time()
    res1 = run_bass_kernel_spmd(_CACHE["l1"], in1, list(range(NCORES)))
    LAST_EXEC_NS[0] = res1.exec_time_ns
    LAST_WALL_S[0] = time.time() - t0

    mom72 = np.concatenate([res1.results[i]["mom"] for i in range(NCORES)],
                           0).astype(np.float32)
    mom = mom72.reshape(-1, 9, 8).sum(-1)
    partial0 = np.concatenate(
        [res1.results[i]["partial0"].T for i in range(NCORES)], 0)

    # host: cov assembly (fp32) + LAPACK-clone eigh + global normalizers
    Sx = mom[:, 0:3].astype(np.float32)
    Sxx = mom[:, 3:6].astype(np.float32)
    Sxy = mom[:, 6:9].astype(np.float32)
    L = np.float32(L0)
    cov = np.empty((x.shape[0], 3, 3), np.float32)
    for idx, (c, dch) in enumerate([(0, 1), (0, 2), (1, 2)]):
        v = (Sxy[:, idx] - Sx[:, c] * Sx[:, dch] / L) / np.float32(L0 - 1)
        cov[:, c, dch] = v
        cov[:, dch, c] = v
    for c in range(3):
        cov[:, c, c] = (Sxx[:, c] - Sx[:, c] * Sx[:, c] / L) / np.float32(
            L0 - 1)
    vals, vecs = _eigh3_batch(cov)
    covn = cov / np.abs(cov).max()
    valsn = (vals / vals.max())[..., None]
    feats = np.concatenate([covn, valsn, vecs], axis=-1).astype(np.float32)

    in2 = []
    for i in range(NCORES):
        sl = slice(i * NS, (i + 1) * NS)
        m = {"featsT": np.ascontiguousarray(
                 feats[sl].transpose(1, 2, 0).reshape(3, 7 * NS)),
             "p0T": partial0[sl].T.copy(),
             "wcT": w["wcT"], "bc": w["bc"], "w0bT": w["w0bT"],
             "bl0": w["bl0"], "wl1T": w["wl1T"], "bl1": w["bl1"]}
        in2.append(m)
    t0 = time.time()
    res2 = run_bass_kernel_spmd(_CACHE["l2"], in2, list(range(NCORES)))
    LAST_EXEC_NS[1] = res2.exec_time_ns
    LAST_WALL_S[1] = time.time() - t0

    out = np.concatenate([res2.results[i]["out2"].T for i in range(NCORES)],
                         0).astype(np.float32)
    return (out[:, 0:1], out[:, 1:2])


# revision 18
# speedup vs baseline: 1.1964x; 1.0082x over previous
"""Trainium2 Bass kernel for nn_BAZ_Network (dense CNN + cov/eig head).

Data-parallel over 8 NeuronCores: 128 samples each.

Launch 1 (per core), software-pipelined over 64 sample-pairs:
  conv trunk as G-packed banded-weight matmuls (bf16, fp32 PSUM), with
  conv biases folded into the matmuls via a ones-row in the rhs (conv0,
  conv1).  Postprocess per (E,O) parity pair is two fused ops over a
  2-sample two-PSUM-bank 3D access pattern:
    op1 (Act):  tE = relu(psE + b)           PSUM -> SBUF bf16
    op2 (DVE):  s  = max(psO + b, tE)        = relu(max(E,O)+b), the
                 maxpool, relu, bias and bf16 cast in one instruction.
  Covariance fp32 raw moments (means on Act copy+accum, squares on Act
  Square+accum, cross-products on DVE stt+accum) are interleaved into
  the pair pipeline; the FC contraction of the conv features against
  wl0[:, :7500] runs per-block (125 accumulating matmuls of 8 cols).
  Stage stagger: conv0(p) | conv1(p-1) | conv2(p-2) | conv3 at odd p |
  FC at p%4==2, which hides the halo-DMA and PSUM-evacuation latency.
Host: branch-exact fp32 netlib-LAPACK ssyevd clone for the 3x3 eigh
  (required to reproduce jnp.linalg.eigh eigenvector signs).
Launch 2 (per core): eig-feature head: 1x1 conv (wc) + relu, remaining
  FC columns wl0[:, 7500:], bias+relu, final linear wl1.
"""

import os
import sys
import time
import numpy as np
import ml_dtypes

sys.path.insert(0, "/opt/trn_rl_repo")
os.environ["BASS_NEVER_TRACE"] = "1"

import concourse.bass as bass  # noqa: E402
import concourse.tile as tile  # noqa: E402
import concourse.mybir as mybir  # noqa: E402
from concourse import bacc  # noqa: E402
from concourse.bass_utils import run_bass_kernel_spmd  # noqa: E402

F32 = mybir.dt.float32
BF16 = mybir.dt.bfloat16
AOP = mybir.AluOpType
ACTF = mybir.ActivationFunctionType
BF = ml_dtypes.bfloat16

NCORES = 8
NS = 128          # samples per core
BN = 8            # samples per block
NBLK = NS // BN
NPAIR = NS // 2   # 64 sample-pairs, the pipeline unit
L0 = 6000

LAST_EXEC_NS = [None, None]
LAST_WALL_S = [None, None]
_CACHE = {}


# ---------------------------------------------------------------- eigh ----
# fp32 netlib-LAPACK ssyevd clone for n=3 (jobz='V', uplo='L').
# Matches jaxlib's CPU eigh (LAPACK >= 3.10 slartg) bit-closely: 0/3072
# eigenvector sign mismatches on the problem distribution.

_F = np.float32
_EPS = _F(np.finfo(np.float32).eps) * _F(0.5)
_EPS2 = _EPS * _EPS
_SAFMIN = _F(np.finfo(np.float32).tiny)


def _slapy2(x, y):
    xa, ya = abs(x), abs(y)
    w, z = max(xa, ya), min(xa, ya)
    if z == 0:
        return w
    return _F(w * _F(np.sqrt(_F(_F(1.0) + _F(_F(z / w) * _F(z / w))))))


def _sign(a, b):
    return abs(a) if b >= 0 else -abs(a)


def _slartg(f, g):
    if g == _F(0.0):
        return _F(1.0), _F(0.0), f
    if f == _F(0.0):
        return _F(0.0), _sign(_F(1.0), g), abs(g)
    d = _F(np.sqrt(_F(f * f + g * g)))
    c = _F(abs(f) / d)
    r = _sign(d, f)
    s = _F(g / r)
    return c, s, r


def _slaev2(a, b, c):
    sm = _F(a + c)
    df = _F(a - c)
    adf = abs(df)
    tb = _F(b + b)
    ab = abs(tb)
    acmx, acmn = (a, c) if abs(a) > abs(c) else (c, a)
    if adf > ab:
        t = _F(ab / adf)
        rt = _F(adf * _F(np.sqrt(_F(_F(1.0) + _F(t * t)))))
    elif adf < ab:
        t = _F(adf / ab)
        rt = _F(ab * _F(np.sqrt(_F(_F(1.0) + _F(t * t)))))
    else:
        rt = _F(ab * _F(np.sqrt(_F(2.0))))
    if sm < 0:
        rt1 = _F(_F(0.5) * _F(sm - rt))
        sgn1 = -1
        rt2 = _F(_F(_F(acmx / rt1) * acmn) - _F(_F(b / rt1) * b))
    elif sm > 0:
        rt1 = _F(_F(0.5) * _F(sm + rt))
        sgn1 = 1
        rt2 = _F(_F(_F(acmx / rt1) * acmn) - _F(_F(b / rt1) * b))
    else:
        rt1 = _F(_F(0.5) * rt)
        rt2 = _F(_F(-0.5) * rt)
        sgn1 = 1
    if df >= 0:
        cs = _F(df + rt)
        sgn2 = 1
    else:
        cs = _F(df - rt)
        sgn2 = -1
    acs = abs(cs)
    if acs > ab:
        ct = _F(-tb / cs)
        sn1 = _F(_F(1.0) / _F(np.sqrt(_F(_F(1.0) + _F(ct * ct)))))
        cs1 = _F(ct * sn1)
    else:
        if ab == 0:
            cs1, sn1 = _F(1.0), _F(0.0)
        else:
            tn = _F(-cs / tb)
            cs1 = _F(_F(1.0) / _F(np.sqrt(_F(_F(1.0) + _F(tn * tn)))))
            sn1 = _F(tn * cs1)
    if sgn1 == sgn2:
        cs1, sn1 = -sn1, cs1
    return rt1, rt2, cs1, sn1


def _ssytrd3(A):
    a00, a10, a20 = A[0, 0], A[1, 0], A[2, 0]
    a11, a21, a22 = A[1, 1], A[2, 1], A[2, 2]
    xnorm = abs(a20)
    if xnorm == _F(0.0):
        beta, v2, tau = a10, a20, _F(0.0)
    else:
        beta = -_sign(_slapy2(a10, xnorm), a10)
        tau = _F(_F(beta - a10) / beta)
        v2 = _F(a20 * _F(_F(1.0) / _F(a10 - beta)))
    e0 = beta
    if tau != _F(0.0):
        x0 = _F(_F(tau * a11) + _F(tau * _F(a21 * v2)))
        x1 = _F(_F(tau * a21) + _F(_F(tau * v2) * a22))
        sdot = _F(_F(x0 * _F(1.0)) + _F(x1 * v2))
        alpha = _F(_F(_F(-0.5) * tau) * sdot)
        w0 = _F(x0 + _F(alpha * _F(1.0)))
        w1 = _F(x1 + _F(alpha * v2))
        t1, t2 = -w0, _F(-1.0)
        a11 = _F(_F(a11 + _F(_F(1.0) * t1)) + _F(w0 * t2))
        a21 = _F(_F(a21 + _F(v2 * t1)) + _F(w1 * t2))
        t1b, t2b = -w1, -v2
        a22 = _F(_F(a22 + _F(v2 * t1b)) + _F(w1 * t2b))
    d = np.array([a00, a11, a22], np.float32)
    e = np.array([e0, a21, 0.0], np.float32)
    return d, e, v2, tau


def _ssteqr3(d, e):
    n = 3
    Z = np.eye(3, dtype=np.float32)
    wc = np.zeros(2, np.float32)
    ws = np.zeros(2, np.float32)
    nmaxit, jtot = 90, 0

    def lasr_b(l, m):
        for j in range(m - 1, l - 1, -1):
            c, s = wc[j - 1], ws[j - 1]
            if c != _F(1.0) or s != _F(0.0):
                for i in range(3):
                    t = Z[i, j]
                    Z[i, j] = _F(_F(c * t) - _F(s * Z[i, j - 1]))
                    Z[i, j - 1] = _F(_F(s * t) + _F(c * Z[i, j - 1]))

    def lasr_f(m, l):
        for j in range(m, l):
            c, s = wc[j - 1], ws[j - 1]
            if c != _F(1.0) or s != _F(0.0):
                for i in range(3):
                    t = Z[i, j]
                    Z[i, j] = _F(_F(c * t) - _F(s * Z[i, j - 1]))
                    Z[i, j - 1] = _F(_F(s * t) + _F(c * Z[i, j - 1]))

    l1 = 1
    while True:
        if l1 > n:
            break
        if l1 > 1:
            e[l1 - 2] = _F(0.0)
        m = n
        for mm in range(l1, n):
            tst = abs(e[mm - 1])
            if tst == _F(0.0):
                m = mm
                break
            if tst <= _F(_F(_F(np.sqrt(abs(d[mm - 1]))) *
                            _F(np.sqrt(abs(d[mm])))) * _EPS):
                e[mm - 1] = _F(0.0)
                m = mm
                break
        l = l1
        lend = m
        l1 = m + 1
        if lend == l:
            continue
        if abs(d[lend - 1]) < abs(d[l - 1]):
            lend, l = l, lend
        if lend > l:
            while True:  # QL
                m = lend
                if l != lend:
                    for mm in range(l, lend):
                        tst = _F(abs(e[mm - 1]) * abs(e[mm - 1]))
                        if tst <= _F(_F(_F(_EPS2 * abs(d[mm - 1])) *
                                        abs(d[mm])) + _SAFMIN):
                            m = mm
                            break
                if m < lend:
                    e[m - 1] = _F(0.0)
                p = d[l - 1]
                if m == l:
                    d[l - 1] = p
                    l += 1
                    if l <= lend:
                        continue
                    break
                if m == l + 1:
                    rt1, rt2, c, s = _slaev2(d[l - 1], e[l - 1], d[l])
                    wc[l - 1] = c
                    ws[l - 1] = s
                    lasr_b(l, l + 1)
                    d[l - 1] = rt1
                    d[l] = rt2
                    e[l - 1] = _F(0.0)
                    l += 2
                    if l <= lend:
                        continue
                    break
                if jtot == nmaxit:
                    break
                jtot += 1
                g = _F(_F(d[l] - p) / _F(_F(2.0) * e[l - 1]))
                r = _slapy2(g, _F(1.0))
                g = _F(_F(d[m - 1] - p) + _F(e[l - 1] / _F(g + _sign(r, g))))
                s = _F(1.0)
                c = _F(1.0)
                p = _F(0.0)
                for i in range(m - 1, l - 1, -1):
                    f = _F(s * e[i - 1])
                    b = _F(c * e[i - 1])
                    c, s, r = _slartg(g, f)
                    if i != m - 1:
                        e[i] = r
                    g = _F(d[i] - p)
                    r = _F(_F(_F(d[i - 1] - g) * s) + _F(_F(_F(2.0) * c) * b))
                    p = _F(s * r)
                    d[i] = _F(g + p)
                    g = _F(_F(c * r) - b)
                    wc[i - 1] = c
                    ws[i - 1] = -s
                lasr_b(l, m)
                d[l - 1] = _F(d[l - 1] - p)
                e[l - 1] = g
        else:
            while True:  # QR
                m = lend
                if l != lend:
                    for mm in range(l, lend, -1):
                        tst = _F(abs(e[mm - 2]) * abs(e[mm - 2]))
                        if tst <= _F(_F(_F(_EPS2 * abs(d[mm - 1])) *
                                        abs(d[mm - 2])) + _SAFMIN):
                            m = mm
                            break
                if m > lend:
                    e[m - 2] = _F(0.0)
                p = d[l - 1]
                if m == l:
                    d[l - 1] = p
                    l -= 1
                    if l >= lend:
                        continue
                    break
                if m == l - 1:
                    rt1, rt2, c, s = _slaev2(d[l - 2], e[l - 2], d[l - 1])
                    wc[m - 1] = c
                    ws[m - 1] = s
                    lasr_f(m, l)
                    d[l - 2] = rt1
                    d[l - 1] = rt2
                    e[l - 2] = _F(0.0)
                    l -= 2
                    if l >= lend:
                        continue
                    break
                if jtot == nmaxit:
                    break
                jtot += 1
                g = _F(_F(d[l - 2] - p) / _F(_F(2.0) * e[l - 2]))
                r = _slapy2(g, _F(1.0))
                g = _F(_F(d[m - 1] - p) + _F(e[l - 2] / _F(g + _sign(r, g))))
                s = _F(1.0)
                c = _F(1.0)
                p = _F(0.0)
                for i in range(m, l):
                    f = _F(s * e[i - 1])
                    b = _F(c * e[i - 1])
                    c, s, r = _slartg(g, f)
                    if i != m:
                        e[i - 2] = r
                    g = _F(d[i - 1] - p)
                    r = _F(_F(_F(d[i] - g) * s) + _F(_F(_F(2.0) * c) * b))
                    p = _F(s * r)
                    d[i - 1] = _F(g + p)
                    g = _F(_F(c * r) - b)
                    wc[i - 1] = c
                    ws[i - 1] = s
                lasr_f(m, l)
                d[l - 1] = _F(d[l - 1] - p)
                e[l - 2] = g
        if jtot >= nmaxit:
            break
    for ii in range(2, n + 1):
        i = ii - 1
        k = i
        p = d[i - 1]
        for j in range(ii, n + 1):
            if d[j - 1] < p:
                k = j
                p = d[j - 1]
        if k != i:
            d[k - 1] = d[i - 1]
            d[i - 1] = p
            tmp = Z[:, k - 1].copy()
            Z[:, k - 1] = Z[:, i - 1]
            Z[:, i - 1] = tmp
    return d, Z


def _eigh3_batch(covs):
    n = covs.shape[0]
    W = np.empty((n, 3), np.float32)
    V = np.empty((n, 3, 3), np.float32)
    for i in range(n):
        d, e, v2, tau = _ssytrd3(covs[i])
        w, Z = _ssteqr3(d, e)
        if tau != _F(0.0):
            for j in range(3):
                vtz = _F(Z[1, j] + _F(v2 * Z[2, j]))
                tvz = _F(tau * vtz)
                Z[1, j] = _F(Z[1, j] - tvz)
                Z[2, j] = _F(Z[2, j] - _F(v2 * tvz))
        W[i] = w
        V[i] = Z
    return W, V


# ------------------------------------------------------------- weights ----

def _prep_weights(ins):
    """Host-side packing of the model weights into device layouts.

    Strided column-pair scheme (see the layer maps below); conv0/conv1
    biases ride a ones-row in the rhs so the matmul itself adds them.
    """
    w0, w1, w2, w3 = ins["w0"], ins["w1"], ins["w2"], ins["w3"]
    b0, b1 = np.asarray(ins["b0"], np.float32), np.asarray(ins["b1"],
                                                           np.float32)

    d = {}
    # conv0: window rows (c:3, j:9): even cols l = 8q-1+j, odd l = 8q+j;
    # k = j - 2g for output slot g; row 27 = ones -> bias.
    W0 = np.zeros((28, 80), np.float32)
    for c in range(3):
        for j in range(9):
            for g in range(4):
                k = j - 2 * g
                if 0 <= k < 3:
                    for o in range(20):
                        W0[c * 9 + j, g * 20 + o] = w0[o, c, k]
    W0[27, :] = np.tile(b0, 4)
    d["W0"] = W0.astype(BF)

    def s1_rows(with_hl, with_hr):
        rows = [(g * 20, 20, g) for g in range(4)]
        if with_hl:
            rows.append((80, 20, -1))
        if with_hr:
            rows.append((100, 20, 4))
        return rows

    def mk(w, blocks, Ghalf, parity, Cout, shift, colbase=None):
        Cin = w.shape[1]
        K = max(rb + Cin for rb, _, _ in blocks)
        if colbase is None:
            colbase = [g * Cout for g in range(Ghalf)]
        W = np.zeros((K, max(colbase) + Cout), np.float32)
        for rb, _, lrel in blocks:
            for g in range(Ghalf):
                pos = 2 * g + parity
                k = (lrel + shift) - pos + 1
                if 0 <= k < 3:
                    for ci in range(Cin):
                        W[rb + ci, colbase[g] + np.arange(Cout)] = w[:, ci, k]
        return W

    # conv1 output M-order: g0->0, g1->64, g2->96, g3->32 (C1B) so conv2's
    # boundary reads sit at legal rhs bases.
    C1B = [0, 64, 96, 32]

    # baseline-layout W1 blocks, then re-rowed for the s1 layout with the
    # ones row at 80: main [0:80], ones 80, hl [81:101], hr [101:121].
    w1e1_base = mk(w1, s1_rows(True, False), 4, 0, 32, 0, C1B)   # [100,128]
    w1e2 = mk(w1, [(rb, 20, lr + 4) for rb, _, lr in
                   s1_rows(False, False)], 4, 0, 32, 0, C1B)     # [80,128]
    w1o1 = mk(w1, s1_rows(False, False), 4, 1, 32, 0, C1B)       # [80,128]
    w1o2_blocks = ([(g * 20, 20, g + 4) for g in range(4)] +
                   [(80, 20, 1000), (100, 20, 8)])
    w1o2_base = mk(w1, w1o2_blocks, 4, 1, 32, 0, C1B)            # [120,128]
    b1t = np.tile(b1, 4)
    W1e1 = np.zeros((101, 128), np.float32)
    W1e1[0:80] = w1e1_base[0:80]
    W1e1[80] = b1t
    W1e1[81:101] = w1e1_base[80:100]
    W1o2 = np.zeros((121, 128), np.float32)
    W1o2[0:80] = w1o2_base[0:80]
    W1o2[80] = b1t
    W1o2[101:121] = w1o2_base[100:120]
    d["W1e1"] = W1e1.astype(BF)
    d["W1e2"] = w1e2.astype(BF)
    d["W1o1"] = w1o1.astype(BF)
    d["W1o2"] = W1o2.astype(BF)

    # conv2 (G=4, Ghalf=2, Cout=64): stored2 rows (g:4, o:32)->128
    s2_main = [(0, 32, 0), (64, 32, 1), (96, 32, 2), (32, 32, 3)]
    d["W2e1"] = mk(w2, [(0, 32, -1)], 2, 0, 64, 0).astype(BF)
    d["W2e2"] = mk(w2, s2_main, 2, 0, 64, 0).astype(BF)
    d["W2o1"] = mk(w2, s2_main, 2, 1, 64, 0).astype(BF)
    d["W2o2"] = mk(w2, [(0, 32, 4)], 2, 1, 64, 0).astype(BF)

    # conv3 (G=12, Ghalf=6, Cout=20): stored3 rows (g:2, o:64)->128.
    # Window w covers pre-pool pos [12w, 12w+12); MM t reads s3 col 6w+t
    # (l3 = 12w + 2t - 2 + g); M = (h:6, o:20) = 120.
    for t in range(7):
        d[f"W3E{t}"] = mk(w3, [(0, 64, 2 * t - 2), (64, 64, 2 * t - 1)],
                          6, 0, 20, 0).astype(BF)
    for t in range(1, 8):
        d[f"W3O{t}"] = mk(w3, [(0, 64, 2 * t - 2), (64, 64, 2 * t - 1)],
                          6, 1, 20, 0).astype(BF)

    # fc: stored4 rows (h:6, o:20)->120, col w: feature (o, l4 = 6w + h)
    wl0 = ins["wl0"]
    WFC = np.zeros((120, 63 * 100), np.float32)
    for w in range(63):
        for h in range(6):
            l4 = 6 * w + h
            if l4 < 375:
                WFC[h * 20:(h + 1) * 20, w * 100:(w + 1) * 100] = \
                    wl0[:, np.arange(20)[:, None] * 375 + l4].T.reshape(
                        20, 100)
    d["WFC"] = WFC.astype(BF)

    d["B2"] = np.tile(ins["b2"], 2).astype(np.float32)[:, None]   # [128]
    d["B3"] = np.tile(ins["b3"], 6).astype(np.float32)[:, None]   # [120]
    # launch 2
    d["wcT"] = ins["wc"][:, :, 0].T.astype(np.float32).copy()      # [3, 20]
    d["bc"] = ins["bc"].astype(np.float32)[:, None]                # [20, 1]
    w0b = np.zeros((7, 20, 100), np.float32)
    for t in range(7):
        for o in range(20):
            w0b[t, o] = ins["wl0"][:, 7500 + o * 7 + t]
    d["w0bT"] = w0b
    d["bl0"] = ins["bl0"].astype(np.float32)[:, None]              # [100, 1]
    d["wl1T"] = ins["wl1"].T.astype(np.float32).copy()             # [100, 2]
    d["bl1"] = ins["bl1"].astype(np.float32)[:, None]              # [2, 1]
    return d


# ------------------------------------------------------------- launch 1 ----

def _build_launch1():
    nc = bacc.Bacc("TRN2", target_bir_lowering=False, debug=False,
                   num_devices=NCORES)
    dram = {}
    for nm, shape, dt in [
        ("x_winE", [28, NS, 750], BF16), ("x_winO", [28, NS, 750], BF16),
        ("x_f32", [NS, 3, L0], F32),
        ("W0", [28, 80], BF16),
        ("W1e1", [101, 128], BF16), ("W1e2", [80, 128], BF16),
        ("W1o1", [80, 128], BF16), ("W1o2", [121, 128], BF16),
        ("W2e1", [32, 128], BF16), ("W2e2", [128, 128], BF16),
        ("W2o1", [128, 128], BF16), ("W2o2", [32, 128], BF16),
    ] + [(f"W3E{t}", [128, 120], BF16) for t in range(7)] + \
        [(f"W3O{t}", [128, 120], BF16) for t in range(1, 8)] + [
        ("WFC", [120, 6300], BF16),
        ("B2", [128, 1], F32), ("B3", [120, 1], F32),
        ("INIT1", [1, BN, 750], BF16), ("INITZ", [128, BN, 1], BF16),
    ]:
        dram[nm] = nc.dram_tensor(nm, shape, dt, kind="ExternalInput").ap()
    out_p0 = nc.dram_tensor("partial0", [100, NS], F32,
                            kind="ExternalOutput").ap()
    out_mom = nc.dram_tensor("mom", [NS, 72], F32,
                             kind="ExternalOutput").ap()

    with tile.TileContext(nc) as tc:
        with tc.tile_pool(name="wpool", bufs=1) as wp, \
             tc.tile_pool(name="covp", bufs=1) as cvp, \
             tc.tile_pool(name="scrp", bufs=4) as scp, \
             tc.tile_pool(name="xw", bufs=2) as xwp, \
             tc.tile_pool(name="s1p", bufs=1) as s1p, \
             tc.tile_pool(name="s2p", bufs=1) as s2p, \
             tc.tile_pool(name="s3p", bufs=1) as s3p, \
             tc.tile_pool(name="s4p", bufs=1) as s4p, \
             tc.tile_pool(name="tep", bufs=4) as tep, \
             tc.tile_pool(name="ps", bufs=4, space="PSUM") as psp:

            xw_tiles = {}

            def issue_xw(b):
                if b >= NBLK or b in xw_tiles:
                    return
                n0 = b * BN
                te = xwp.tile([28, BN, 750], BF16, tag="xwE")
                nc.sync.dma_start(te[:], dram["x_winE"][:, n0:n0 + BN, :])
                to = xwp.tile([28, BN, 750], BF16, tag="xwO")
                nc.sync.dma_start(to[:], dram["x_winO"][:, n0:n0 + BN, :])
                xw_tiles[b] = (te, to)

            # The SP DMA queue is FIFO and a DMA holds HWDGE ~0.6us each:
            # preload ONLY what conv0(0)/conv1(0) need, trickle the rest
            # into the pipeline steps below via dma_sched.
            momt = cvp.tile([NS, 72], F32, tag="mom")
            p0sb = cvp.tile([100, NS], F32, tag="p0sb")
            s1 = s1p.tile([121, BN, 750], BF16, tag="s1")
            s2 = s2p.tile([128, BN, 377], BF16, tag="s2")
            s3 = s3p.tile([128, BN, 380], BF16, tag="s3")
            s4 = s4p.tile([120, NS, 63], BF16, tag="s4")
            xcv = cvp.tile([NS, 3, L0], F32, tag="xcv")

            Ws = {}
            for nm in (["W1e1", "W1e2", "W1o1", "W1o2", "W0",
                        "W2e2", "W2o1", "W2o2"] +
                       [f"W3E{t}" for t in range(7)] +
                       [f"W3O{t}" for t in range(1, 8)] + ["WFC"]):
                Ws[nm] = wp.tile(list(dram[nm].shape), BF16, name=nm, tag=nm)
            Wpad = wp.tile([64, 128], BF16, name="W2e1", tag="W2e1")
            Ws["W2e1"] = Wpad[32:64]
            B2t = wp.tile([128, 1], F32, tag="B2")
            B3t = wp.tile([120, 1], F32, tag="B3")

            def dma_w(nm):
                nc.sync.dma_start(Ws[nm][:], dram[nm][:])

            # prologue: conv0(0..1) + conv1(0) prerequisites only
            dma_w("W0")
            issue_xw(0)
            issue_xw(1)
            # ones row + hl/hr edges (engines cannot address partition
            # bases off the 0/32/64/96 grid; DMA can); conv1 prereqs ride
            # the otherwise-idle Act HWDGE queue in parallel
            nc.scalar.dma_start(s1[80:81, :, :], dram["INIT1"][:])
            nc.scalar.dma_start(s1[81:101, :, 0:1], dram["INITZ"][0:20])
            nc.scalar.dma_start(s1[101:121, :, 749:750], dram["INITZ"][0:20])
            for nm in ["W1e1", "W1e2", "W1o1", "W1o2"]:
                nc.scalar.dma_start(Ws[nm][:], dram[nm][:])

            def dmas_step0():
                nc.sync.dma_start(Wpad[32:64], dram["W2e1"][:])
                for nm in ["W2e2", "W2o1", "W2o2"]:
                    dma_w(nm)
                nc.sync.dma_start(B2t[:], dram["B2"][:])
                nc.sync.dma_start(s2[:, :, 0:1], dram["INITZ"][:])
                nc.sync.dma_start(s2[:, :, 376:377], dram["INITZ"][:])

            def dmas_step1():
                for t in range(7):
                    dma_w(f"W3E{t}")

            def dmas_step2():
                for t in range(1, 8):
                    dma_w(f"W3O{t}")
                nc.sync.dma_start(B3t[:], dram["B3"][:])
                nc.sync.dma_start(s3[:, :, 0:1], dram["INITZ"][:])
                for cz in range(376, 380):
                    nc.sync.dma_start(s3[:, :, cz:cz + 1], dram["INITZ"][:])

            def make_xcv_piece(i):
                c0 = i * 750
                return lambda: nc.sync.dma_start(
                    xcv[:, :, c0:c0 + 750], dram["x_f32"][:, :, c0:c0 + 750])

            def make_wfc_piece(i):
                c0 = i * 3150
                return lambda: nc.sync.dma_start(
                    Ws["WFC"][:, c0:c0 + 3150], dram["WFC"][:, c0:c0 + 3150])

            dma_sched = {0: dmas_step0, 1: dmas_step1, 2: dmas_step2}
            for i in range(8):
                dma_sched[3 + i] = make_xcv_piece(i)
            dma_sched[12] = make_wfc_piece(0)
            dma_sched[14] = make_wfc_piece(1)

            # covariance moment ops, chunked 8x750 so no single op can
            # head-of-line block the Act/DVE queues; 72 partial sums,
            # host adds the 8 chunks per moment.
            cov_ops = []
            for ch in range(8):
                for c in range(3):
                    cov_ops.append(("mean", c, ch))
                    cov_ops.append(("sq", c, ch))
                for k, (c, dch) in enumerate([(0, 1), (0, 2), (1, 2)]):
                    cov_ops.append(("xy", k, c, dch, ch))
            # Act gets exactly one ~1us cov op per step, DVE one per two
            # steps: never more than the pipeline's per-step engine slack.
            act_ops = [op for op in cov_ops if op[0] != "xy"]
            dve_ops = [op for op in cov_ops if op[0] == "xy"]
            cov_sched = {}
            for i, op in enumerate(act_ops):
                cov_sched.setdefault(6 + i, []).append(op)
            for i, op in enumerate(dve_ops):
                cov_sched.setdefault(6 + 2 * i, []).append(op)

            def emit_cov(op):
                ch = op[-1]
                c0 = ch * 750
                sl = slice(c0, c0 + 750)
                scr = scp.tile([NS, 750], BF16, tag="cscr")
                if op[0] == "mean":
                    c = op[1]
                    nc.scalar.activation(scr[:], xcv[:, c, sl], ACTF.Copy,
                                         accum_out=momt[:, c * 8 + ch:
                                                        c * 8 + ch + 1])
                elif op[0] == "sq":
                    c = op[1]
                    nc.scalar.activation(scr[:], xcv[:, c, sl], ACTF.Square,
                                         accum_out=momt[:, 24 + c * 8 + ch:
                                                        24 + c * 8 + ch + 1])
                else:
                    _, k, c, dch, ch = op
                    nc.vector.scalar_tensor_tensor(
                        scr[:], xcv[:, c, sl], 1.0, xcv[:, dch, sl],
                        AOP.mult, AOP.mult,
                        accum_out=momt[:, 48 + k * 8 + ch:
                                       48 + k * 8 + ch + 1])

            def conv0(p):
                blk, nb = p // 4, (p % 4) * 2
                xwE, xwO = xw_tiles[blk]
                for ch in range(2):
                    c0 = ch * 375
                    psE = psp.tile([128, 2, 512], F32, tag="ps")
                    psO = psp.tile([128, 2, 512], F32, tag="ps")
                    for i in range(2):
                        nc.tensor.matmul(psE[0:80, i, 0:375], Ws["W0"][:],
                                         xwE[:, nb + i, c0:c0 + 375],
                                         start=True, stop=True)
                    for i in range(2):
                        nc.tensor.matmul(psO[0:80, i, 0:375], Ws["W0"][:],
                                         xwO[:, nb + i, c0:c0 + 375],
                                         start=True, stop=True)
                    tE = tep.tile([128, 2, 384], BF16, tag="tE")
                    nc.scalar.activation(tE[0:80, :, 0:375],
                                         psE[0:80, :, 0:375], ACTF.Relu)
                    nc.vector.scalar_tensor_tensor(
                        s1[0:80, nb:nb + 2, c0:c0 + 375],
                        psO[0:80, :, 0:375], 0.0, tE[0:80, :, 0:375],
                        AOP.max, AOP.max)
                # per-pair halo rows for conv1
                nc.sync.dma_start(s1[81:101, nb:nb + 2, 1:750],
                                  s1[60:80, nb:nb + 2, 0:749])
                nc.sync.dma_start(s1[101:121, nb:nb + 2, 0:749],
                                  s1[0:20, nb:nb + 2, 1:750])

            def conv1(p):
                nb = (p % 4) * 2
                psE = psp.tile([128, 2, 512], F32, tag="ps")
                psO = psp.tile([128, 2, 512], F32, tag="ps")
                for i in range(2):
                    n = nb + i
                    nc.tensor.matmul(psE[0:128, i, 0:375], Ws["W1e1"][:],
                                     s1[0:101, n, 0:750:2],
                                     start=True, stop=False)
                    nc.tensor.matmul(psE[0:128, i, 0:375], Ws["W1e2"][:],
                                     s1[0:80, n, 1:750:2],
                                     start=False, stop=True)
                for i in range(2):
                    n = nb + i
                    nc.tensor.matmul(psO[0:128, i, 0:375], Ws["W1o1"][:],
                                     s1[0:80, n, 0:750:2],
                                     start=True, stop=False)
                    nc.tensor.matmul(psO[0:128, i, 0:375], Ws["W1o2"][:],
                                     s1[0:121, n, 1:750:2],
                                     start=False, stop=True)
                tE = tep.tile([128, 2, 384], BF16, tag="tE")
                nc.scalar.activation(tE[0:128, :, 0:375],
                                     psE[0:128, :, 0:375], ACTF.Relu)
                nc.vector.scalar_tensor_tensor(
                    s2[0:128, nb:nb + 2, 1:376],
                    psO[0:128, :, 0:375], 0.0, tE[0:128, :, 0:375],
                    AOP.max, AOP.max)

            def conv2(p):
                nb = (p % 4) * 2
                psE = psp.tile([128, 2, 512], F32, tag="ps")
                psO = psp.tile([128, 2, 512], F32, tag="ps")
                for i in range(2):
                    n = nb + i
                    nc.tensor.matmul(psE[0:128, i, 0:375], Ws["W2e1"],
                                     s2[32:64, n, 0:375],
                                     start=True, stop=False)
                    nc.tensor.matmul(psE[0:128, i, 0:375], Ws["W2e2"][:],
                                     s2[0:128, n, 1:376],
                                     start=False, stop=True)
                for i in range(2):
                    n = nb + i
                    nc.tensor.matmul(psO[0:128, i, 0:375], Ws["W2o1"][:],
                                     s2[0:128, n, 1:376],
                                     start=True, stop=False)
                    nc.tensor.matmul(psO[0:128, i, 0:375], Ws["W2o2"][:],
                                     s2[0:32, n, 2:377],
                                     start=False, stop=True)
                tE = tep.tile([128, 2, 384], BF16, tag="tE")
                nc.scalar.activation(tE[0:128, :, 0:375],
                                     psE[0:128, :, 0:375], ACTF.Relu,
                                     bias=B2t[:])
                nc.vector.scalar_tensor_tensor(
                    s3[0:128, nb:nb + 2, 1:376],
                    psO[0:128, :, 0:375], B2t[:], tE[0:128, :, 0:375],
                    AOP.add, AOP.max)

            def conv3(qd):
                blk, nq = qd // 2, (qd % 2) * 4
                n0 = blk * BN + nq
                psE = psp.tile([128, 2, 512], F32, tag="ps")
                psO = psp.tile([128, 2, 512], F32, tag="ps")
                for t in range(7):
                    nc.tensor.matmul(
                        psE[0:120, 0, 0:252], Ws[f"W3E{t}"][:],
                        s3[0:128, nq:nq + 4, t:t + 373:6],
                        start=(t == 0), stop=(t == 6))
                for t in range(1, 8):
                    nc.tensor.matmul(
                        psO[0:120, 0, 0:252], Ws[f"W3O{t}"][:],
                        s3[0:128, nq:nq + 4, t:t + 373:6],
                        start=(t == 1), stop=(t == 7))
                tE = tep.tile([128, 512], BF16, tag="tE3")
                nc.scalar.activation(tE[0:120, 0:252], psE[0:120, 0, 0:252],
                                     ACTF.Relu, bias=B3t[:])
                nc.vector.scalar_tensor_tensor(
                    s4[0:120, n0:n0 + 4, 0:63],
                    psO[0:120, 0, 0:252].rearrange("p (n l) -> p n l", n=4),
                    B3t[:],
                    tE[0:120, 0:252].rearrange("p (n l) -> p n l", n=4),
                    AOP.add, AOP.max)

            def fc(half):
                c0 = half * 64
                psfc = psp.tile([128, 2, 512], F32, tag="ps")
                for w in range(63):
                    nc.tensor.matmul(
                        psfc[0:100, 0, 0:64],
                        Ws["WFC"][:, w * 100:(w + 1) * 100],
                        s4[:, c0:c0 + 64, w], start=(w == 0), stop=(w == 62))
                nc.scalar.copy(p0sb[:, c0:c0 + 64], psfc[0:100, 0, 0:64])

            # pipeline: conv0(p) | conv1(p-1) | conv2(p-2) |
            #           conv3((p-3)/2 @ odd p) | fc halves at p=35/66
            for p in range(68):
                if p < NPAIR:
                    if p % 4 == 0:
                        issue_xw(p // 4 + 2)
                    conv0(p)
                if 0 <= p - 1 < NPAIR:
                    conv1(p - 1)
                if 0 <= p - 2 < NPAIR:
                    conv2(p - 2)
                if p % 2 == 0 and 0 <= (p - 4) // 2 < 32:
                    conv3((p - 4) // 2)
                if p == 35 or p == 67:
                    fc(0 if p == 35 else 1)
                for op in cov_sched.get(p, ()):
                    emit_cov(op)
                if p in dma_sched:
                    dma_sched[p]()

            nc.sync.dma_start(out_p0[:], p0sb[:])
            nc.sync.dma_start(out_mom[:], momt[:])

    nc.compile()
    return nc


# ------------------------------------------------------------- launch 2 ----

def _build_launch2():
    nc = bacc.Bacc("TRN2", target_bir_lowering=False, debug=False,
                   num_devices=NCORES)
    dr = {}
    for nm, shape in [("featsT", [3, 7 * NS]), ("p0T", [100, NS]),
                      ("wcT", [3, 20]), ("bc", [20, 1]),
                      ("w0bT", [7, 20, 100]), ("bl0", [100, 1]),
                      ("wl1T", [100, 2]), ("bl1", [2, 1])]:
        dr[nm] = nc.dram_tensor(nm, shape, F32, kind="ExternalInput").ap()
    out2 = nc.dram_tensor("out2", [2, NS], F32, kind="ExternalOutput").ap()

    with tile.TileContext(nc) as tc:
        with tc.tile_pool(name="w2p", bufs=1) as wp, \
             tc.tile_pool(name="ps2", bufs=2, space="PSUM") as psp:
            fT = wp.tile([3, 7 * NS], F32, tag="fT")
            nc.sync.dma_start(fT[:], dr["featsT"][:])
            p0T = wp.tile([100, NS], F32, tag="p0T")
            nc.sync.dma_start(p0T[:], dr["p0T"][:])
            wcT = wp.tile([3, 20], F32, tag="wcT")
            nc.sync.dma_start(wcT[:], dr["wcT"][:])
            bc = wp.tile([20, 1], F32, tag="bc")
            nc.sync.dma_start(bc[:], dr["bc"][:])
            w0bT = [wp.tile([20, 100], F32, name=f"w0bT{t}", tag=f"w0bT{t}")
                    for t in range(7)]
            for t in range(7):
                nc.sync.dma_start(w0bT[t][:], dr["w0bT"][t])
            bl0 = wp.tile([100, 1], F32, tag="bl0")
            nc.sync.dma_start(bl0[:], dr["bl0"][:])
            wl1T = wp.tile([100, 2], F32, tag="wl1T")
            nc.sync.dma_start(wl1T[:], dr["wl1T"][:])
            bl1 = wp.tile([2, 1], F32, tag="bl1")
            nc.sync.dma_start(bl1[:], dr["bl1"][:])

            # h1 = relu(wc @ feats + bc): [20, (t, n)]
            h1 = wp.tile([20, 7 * NS], F32, tag="h1")
            for half in range(2):
                c0 = half * 448
                ps = psp.tile([32, 448], F32, tag="ph")
                nc.tensor.matmul(ps[0:20, :], wcT[:], fT[:, c0:c0 + 448],
                                 start=True, stop=True)
                nc.scalar.activation(h1[:, c0:c0 + 448], ps[0:20, :],
                                     ACTF.Relu, bias=bc[:])
            # z = relu(p0 + sum_t w0b_t.T @ h1_t + bl0)
            psz = psp.tile([100, NS], F32, tag="pz")
            for t in range(7):
                nc.tensor.matmul(psz[:], w0bT[t][:],
                                 h1[:, t * NS:(t + 1) * NS],
                                 start=(t == 0), stop=(t == 6))
            z = wp.tile([100, NS], F32, tag="z")
            nc.vector.scalar_tensor_tensor(z[:], psz[:], bl0[:], p0T[:],
                                           AOP.add, AOP.add)
            nc.vector.tensor_scalar_max(z[:], z[:], 0.0)
            pso = psp.tile([32, NS], F32, tag="po")
            nc.tensor.matmul(pso[0:2, :], wl1T[:], z[:],
                             start=True, stop=True)
            osb = wp.tile([2, NS], F32, tag="osb")
            nc.vector.tensor_scalar(osb[:], pso[0:2, :], bl1[:], None,
                                    AOP.add)
            nc.sync.dma_start(out2[:], osb[:])

    nc.compile()
    return nc


# --------------------------------------------------------------- kernel ----

def kernel(**inputs):
    ins = {k: np.asarray(v) for k, v in inputs.items()}
    x = ins["x"].astype(np.float32)

    if "l1" not in _CACHE:
        _CACHE["l1"] = _build_launch1()
    if "l2" not in _CACHE:
        _CACHE["l2"] = _build_launch2()
    w = _prep_weights(ins)

    xbf = x.astype(BF)
    xwE = np.zeros((28, x.shape[0], 750), BF)
    xwO = np.zeros((28, x.shape[0], 750), BF)
    for c in range(3):
        for j in range(9):
            # even cols: l = 8q - 1 + j ; odd cols: l = 8q + j
            if j == 0:
                xwE[c * 9 + 0, :, 1:750] = xbf[:, c, 7:5992:8]
            else:
                xwE[c * 9 + j] = xbf[:, c, j - 1::8]
            if j == 8:
                xwO[c * 9 + 8, :, 0:749] = xbf[:, c, 8:6000:8]
            else:
                xwO[c * 9 + j] = xbf[:, c, j::8]
    xwE[27] = 1.0
    xwO[27] = 1.0
    shards = [x[i * NS:(i + 1) * NS] for i in range(NCORES)]
    in1 = []
    for i, sh in enumerate(shards):
        sl = slice(i * NS, (i + 1) * NS)
        m = {"x_winE": np.ascontiguousarray(xwE[:, sl]),
             "x_winO": np.ascontiguousarray(xwO[:, sl]),
             "x_f32": sh}
        for nm in (["W0", "W1e1", "W1e2", "W1o1", "W1o2",
                    "W2e1", "W2e2", "W2o1", "W2o2", "WFC", "B2", "B3"] +
                   [f"W3E{t}" for t in range(7)] +
                   [f"W3O{t}" for t in range(1, 8)]):
            m[nm] = w[nm]
        m["INIT1"] = np.ones((1, BN, 750), BF)
        m["INITZ"] = np.zeros((128, BN, 1), BF)
        in1.append(m)
    t0 = time.time()
    res1 = run_bass_kernel_spmd(_CACHE["l1"], in1, list(range(NCORES)))
    LAST_EXEC_NS[0] = res1.exec_time_ns
    LAST_WALL_S[0] = time.time() - t0

    mom72 = np.concatenate([res1.results[i]["mom"] for i in range(NCORES)],
                           0).astype(np.float32)
    mom = mom72.reshape(-1, 9, 8).sum(-1)
    partial0 = np.concatenate(
        [res1.results[i]["partial0"].T for i in range(NCORES)], 0)

    # host: cov assembly (fp32) + LAPACK-clone eigh + global normalizers
    Sx = mom[:, 0:3].astype(np.float32)
    Sxx = mom[:, 3:6].astype(np.float32)
    Sxy = mom[:, 6:9].astype(np.float32)
    L = np.float32(L0)
    cov = np.empty((x.shape[0], 3, 3), np.float32)
    for idx, (c, dch) in enumerate([(0, 1), (0, 2), (1, 2)]):
        v = (Sxy[:, idx] - Sx[:, c] * Sx[:, dch] / L) / np.float32(L0 - 1)
        cov[:, c, dch] = v
        cov[:, dch, c] = v
    for c in range(3):
        cov[:, c, c] = (Sxx[:, c] - Sx[:, c] * Sx[:, c] / L) / np.float32(
            L0 - 1)
    vals, vecs = _eigh3_batch(cov)
    covn = cov / np.abs(cov).max()
    valsn = (vals / vals.max())[..., None]
    feats = np.concatenate([covn, valsn, vecs], axis=-1).astype(np.float32)

    in2 = []
    for i in range(NCORES):
        sl = slice(i * NS, (i + 1) * NS)
        m = {"featsT": np.ascontiguousarray(
                 feats[sl].transpose(1, 2, 0).reshape(3, 7 * NS)),
             "p0T": partial0[sl].T.copy(),
             "wcT": w["wcT"], "bc": w["bc"], "w0bT": w["w0bT"],
             "bl0": w["bl0"], "wl1T": w["wl1T"], "bl1": w["bl1"]}
        in2.append(m)
    t0 = time.time()
    res2 = run_bass_kernel_spmd(_CACHE["l2"], in2, list(range(NCORES)))
    LAST_EXEC_NS[1] = res2.exec_time_ns
    LAST_WALL_S[1] = time.time() - t0

    out = np.concatenate([res2.results[i]["out2"].T for i in range(NCORES)],
                         0).astype(np.float32)
    return (out[:, 0:1], out[:, 1:2])


# revision 19
# speedup vs baseline: 1.2189x; 1.0188x over previous
"""Trainium2 Bass kernel for nn_BAZ_Network (dense CNN + cov/eig head).

Data-parallel over 8 NeuronCores: 128 samples each.

Launch 1 (per core), software-pipelined over 64 sample-pairs:
  conv trunk as G-packed banded-weight matmuls (bf16, fp32 PSUM), with
  conv biases folded into the matmuls via a ones-row in the rhs (conv0,
  conv1).  Postprocess per (E,O) parity pair is two fused ops over a
  2-sample two-PSUM-bank 3D access pattern:
    op1 (Act):  tE = relu(psE + b)           PSUM -> SBUF bf16
    op2 (DVE):  s  = max(psO + b, tE)        = relu(max(E,O)+b), the
                 maxpool, relu, bias and bf16 cast in one instruction.
  Covariance fp32 raw moments (means on Act copy+accum, squares on Act
  Square+accum, cross-products on DVE stt+accum) are interleaved into
  the pair pipeline; the FC contraction of the conv features against
  wl0[:, :7500] runs per-block (125 accumulating matmuls of 8 cols).
  Stage stagger: conv0(p) | conv1(p-1) | conv2(p-2) | conv3 at odd p |
  FC at p%4==2, which hides the halo-DMA and PSUM-evacuation latency.
Host: branch-exact fp32 netlib-LAPACK ssyevd clone for the 3x3 eigh
  (required to reproduce jnp.linalg.eigh eigenvector signs).
Launch 2 (per core): eig-feature head: 1x1 conv (wc) + relu, remaining
  FC columns wl0[:, 7500:], bias+relu, final linear wl1.
"""

import os
import sys
import time
import numpy as np
import ml_dtypes

sys.path.insert(0, "/opt/trn_rl_repo")
os.environ["BASS_NEVER_TRACE"] = "1"

import concourse.bass as bass  # noqa: E402
import concourse.tile as tile  # noqa: E402
import concourse.mybir as mybir  # noqa: E402
from concourse import bacc  # noqa: E402
from concourse.bass_utils import run_bass_kernel_spmd  # noqa: E402

F32 = mybir.dt.float32
BF16 = mybir.dt.bfloat16
AOP = mybir.AluOpType
ACTF = mybir.ActivationFunctionType
BF = ml_dtypes.bfloat16

NCORES = 8
NS = 128          # samples per core
BN = 8            # samples per block
NBLK = NS // BN
NPAIR = NS // 2   # 64 sample-pairs, the pipeline unit
L0 = 6000

LAST_EXEC_NS = [None, None]
LAST_WALL_S = [None, None]
_CACHE = {}


# ---------------------------------------------------------------- eigh ----
# fp32 netlib-LAPACK ssyevd clone for n=3 (jobz='V', uplo='L').
# Matches jaxlib's CPU eigh (LAPACK >= 3.10 slartg) bit-closely: 0/3072
# eigenvector sign mismatches on the problem distribution.

_F = np.float32
_EPS = _F(np.finfo(np.float32).eps) * _F(0.5)
_EPS2 = _EPS * _EPS
_SAFMIN = _F(np.finfo(np.float32).tiny)


def _slapy2(x, y):
    xa, ya = abs(x), abs(y)
    w, z = max(xa, ya), min(xa, ya)
    if z == 0:
        return w
    return _F(w * _F(np.sqrt(_F(_F(1.0) + _F(_F(z / w) * _F(z / w))))))


def _sign(a, b):
    return abs(a) if b >= 0 else -abs(a)


def _slartg(f, g):
    if g == _F(0.0):
        return _F(1.0), _F(0.0), f
    if f == _F(0.0):
        return _F(0.0), _sign(_F(1.0), g), abs(g)
    d = _F(np.sqrt(_F(f * f + g * g)))
    c = _F(abs(f) / d)
    r = _sign(d, f)
    s = _F(g / r)
    return c, s, r


def _slaev2(a, b, c):
    sm = _F(a + c)
    df = _F(a - c)
    adf = abs(df)
    tb = _F(b + b)
    ab = abs(tb)
    acmx, acmn = (a, c) if abs(a) > abs(c) else (c, a)
    if adf > ab:
        t = _F(ab / adf)
        rt = _F(adf * _F(np.sqrt(_F(_F(1.0) + _F(t * t)))))
    elif adf < ab:
        t = _F(adf / ab)
        rt = _F(ab * _F(np.sqrt(_F(_F(1.0) + _F(t * t)))))
    else:
        rt = _F(ab * _F(np.sqrt(_F(2.0))))
    if sm < 0:
        rt1 = _F(_F(0.5) * _F(sm - rt))
        sgn1 = -1
        rt2 = _F(_F(_F(acmx / rt1) * acmn) - _F(_F(b / rt1) * b))
    elif sm > 0:
        rt1 = _F(_F(0.5) * _F(sm + rt))
        sgn1 = 1
        rt2 = _F(_F(_F(acmx / rt1) * acmn) - _F(_F(b / rt1) * b))
    else:
        rt1 = _F(_F(0.5) * rt)
        rt2 = _F(_F(-0.5) * rt)
        sgn1 = 1
    if df >= 0:
        cs = _F(df + rt)
        sgn2 = 1
    else:
        cs = _F(df - rt)
        sgn2 = -1
    acs = abs(cs)
    if acs > ab:
        ct = _F(-tb / cs)
        sn1 = _F(_F(1.0) / _F(np.sqrt(_F(_F(1.0) + _F(ct * ct)))))
        cs1 = _F(ct * sn1)
    else:
        if ab == 0:
            cs1, sn1 = _F(1.0), _F(0.0)
        else:
            tn = _F(-cs / tb)
            cs1 = _F(_F(1.0) / _F(np.sqrt(_F(_F(1.0) + _F(tn * tn)))))
            sn1 = _F(tn * cs1)
    if sgn1 == sgn2:
        cs1, sn1 = -sn1, cs1
    return rt1, rt2, cs1, sn1


def _ssytrd3(A):
    a00, a10, a20 = A[0, 0], A[1, 0], A[2, 0]
    a11, a21, a22 = A[1, 1], A[2, 1], A[2, 2]
    xnorm = abs(a20)
    if xnorm == _F(0.0):
        beta, v2, tau = a10, a20, _F(0.0)
    else:
        beta = -_sign(_slapy2(a10, xnorm), a10)
        tau = _F(_F(beta - a10) / beta)
        v2 = _F(a20 * _F(_F(1.0) / _F(a10 - beta)))
    e0 = beta
    if tau != _F(0.0):
        x0 = _F(_F(tau * a11) + _F(tau * _F(a21 * v2)))
        x1 = _F(_F(tau * a21) + _F(_F(tau * v2) * a22))
        sdot = _F(_F(x0 * _F(1.0)) + _F(x1 * v2))
        alpha = _F(_F(_F(-0.5) * tau) * sdot)
        w0 = _F(x0 + _F(alpha * _F(1.0)))
        w1 = _F(x1 + _F(alpha * v2))
        t1, t2 = -w0, _F(-1.0)
        a11 = _F(_F(a11 + _F(_F(1.0) * t1)) + _F(w0 * t2))
        a21 = _F(_F(a21 + _F(v2 * t1)) + _F(w1 * t2))
        t1b, t2b = -w1, -v2
        a22 = _F(_F(a22 + _F(v2 * t1b)) + _F(w1 * t2b))
    d = np.array([a00, a11, a22], np.float32)
    e = np.array([e0, a21, 0.0], np.float32)
    return d, e, v2, tau


def _ssteqr3(d, e):
    n = 3
    Z = np.eye(3, dtype=np.float32)
    wc = np.zeros(2, np.float32)
    ws = np.zeros(2, np.float32)
    nmaxit, jtot = 90, 0

    def lasr_b(l, m):
        for j in range(m - 1, l - 1, -1):
            c, s = wc[j - 1], ws[j - 1]
            if c != _F(1.0) or s != _F(0.0):
                for i in range(3):
                    t = Z[i, j]
                    Z[i, j] = _F(_F(c * t) - _F(s * Z[i, j - 1]))
                    Z[i, j - 1] = _F(_F(s * t) + _F(c * Z[i, j - 1]))

    def lasr_f(m, l):
        for j in range(m, l):
            c, s = wc[j - 1], ws[j - 1]
            if c != _F(1.0) or s != _F(0.0):
                for i in range(3):
                    t = Z[i, j]
                    Z[i, j] = _F(_F(c * t) - _F(s * Z[i, j - 1]))
                    Z[i, j - 1] = _F(_F(s * t) + _F(c * Z[i, j - 1]))

    l1 = 1
    while True:
        if l1 > n:
            break
        if l1 > 1:
            e[l1 - 2] = _F(0.0)
        m = n
        for mm in range(l1, n):
            tst = abs(e[mm - 1])
            if tst == _F(0.0):
                m = mm
                break
            if tst <= _F(_F(_F(np.sqrt(abs(d[mm - 1]))) *
                            _F(np.sqrt(abs(d[mm])))) * _EPS):
                e[mm - 1] = _F(0.0)
                m = mm
                break
        l = l1
        lend = m
        l1 = m + 1
        if lend == l:
            continue
        if abs(d[lend - 1]) < abs(d[l - 1]):
            lend, l = l, lend
        if lend > l:
            while True:  # QL
                m = lend
                if l != lend:
                    for mm in range(l, lend):
                        tst = _F(abs(e[mm - 1]) * abs(e[mm - 1]))
                        if tst <= _F(_F(_F(_EPS2 * abs(d[mm - 1])) *
                                        abs(d[mm])) + _SAFMIN):
                            m = mm
                            break
                if m < lend:
                    e[m - 1] = _F(0.0)
                p = d[l - 1]
                if m == l:
                    d[l - 1] = p
                    l += 1
                    if l <= lend:
                        continue
                    break
                if m == l + 1:
                    rt1, rt2, c, s = _slaev2(d[l - 1], e[l - 1], d[l])
                    wc[l - 1] = c
                    ws[l - 1] = s
                    lasr_b(l, l + 1)
                    d[l - 1] = rt1
                    d[l] = rt2
                    e[l - 1] = _F(0.0)
                    l += 2
                    if l <= lend:
                        continue
                    break
                if jtot == nmaxit:
                    break
                jtot += 1
                g = _F(_F(d[l] - p) / _F(_F(2.0) * e[l - 1]))
                r = _slapy2(g, _F(1.0))
                g = _F(_F(d[m - 1] - p) + _F(e[l - 1] / _F(g + _sign(r, g))))
                s = _F(1.0)
                c = _F(1.0)
                p = _F(0.0)
                for i in range(m - 1, l - 1, -1):
                    f = _F(s * e[i - 1])
                    b = _F(c * e[i - 1])
                    c, s, r = _slartg(g, f)
                    if i != m - 1:
                        e[i] = r
                    g = _F(d[i] - p)
                    r = _F(_F(_F(d[i - 1] - g) * s) + _F(_F(_F(2.0) * c) * b))
                    p = _F(s * r)
                    d[i] = _F(g + p)
                    g = _F(_F(c * r) - b)
                    wc[i - 1] = c
                    ws[i - 1] = -s
                lasr_b(l, m)
                d[l - 1] = _F(d[l - 1] - p)
                e[l - 1] = g
        else:
            while True:  # QR
                m = lend
                if l != lend:
                    for mm in range(l, lend, -1):
                        tst = _F(abs(e[mm - 2]) * abs(e[mm - 2]))
                        if tst <= _F(_F(_F(_EPS2 * abs(d[mm - 1])) *
                                        abs(d[mm - 2])) + _SAFMIN):
                            m = mm
                            break
                if m > lend:
                    e[m - 2] = _F(0.0)
                p = d[l - 1]
                if m == l:
                    d[l - 1] = p
                    l -= 1
                    if l >= lend:
                        continue
                    break
                if m == l - 1:
                    rt1, rt2, c, s = _slaev2(d[l - 2], e[l - 2], d[l - 1])
                    wc[m - 1] = c
                    ws[m - 1] = s
                    lasr_f(m, l)
                    d[l - 2] = rt1
                    d[l - 1] = rt2
                    e[l - 2] = _F(0.0)
                    l -= 2
                    if l >= lend:
                        continue
                    break
                if jtot == nmaxit:
                    break
                jtot += 1
                g = _F(_F(d[l - 2] - p) / _F(_F(2.0) * e[l - 2]))
                r = _slapy2(g, _F(1.0))
                g = _F(_F(d[m - 1] - p) + _F(e[l - 2] / _F(g + _sign(r, g))))
                s = _F(1.0)
                c = _F(1.0)
                p = _F(0.0)
                for i in range(m, l):
                    f = _F(s * e[i - 1])
                    b = _F(c * e[i - 1])
                    c, s, r = _slartg(g, f)
                    if i != m:
                        e[i - 2] = r
                    g = _F(d[i - 1] - p)
                    r = _F(_F(_F(d[i] - g) * s) + _F(_F(_F(2.0) * c) * b))
                    p = _F(s * r)
                    d[i - 1] = _F(g + p)
                    g = _F(_F(c * r) - b)
                    wc[i - 1] = c
                    ws[i - 1] = s
                lasr_f(m, l)
                d[l - 1] = _F(d[l - 1] - p)
                e[l - 2] = g
        if jtot >= nmaxit:
            break
    for ii in range(2, n + 1):
        i = ii - 1
        k = i
        p = d[i - 1]
        for j in range(ii, n + 1):
            if d[j - 1] < p:
                k = j
                p = d[j - 1]
        if k != i:
            d[k - 1] = d[i - 1]
            d[i - 1] = p
            tmp = Z[:, k - 1].copy()
            Z[:, k - 1] = Z[:, i - 1]
            Z[:, i - 1] = tmp
    return d, Z


def _eigh3_batch(covs):
    n = covs.shape[0]
    W = np.empty((n, 3), np.float32)
    V = np.empty((n, 3, 3), np.float32)
    for i in range(n):
        d, e, v2, tau = _ssytrd3(covs[i])
        w, Z = _ssteqr3(d, e)
        if tau != _F(0.0):
            for j in range(3):
                vtz = _F(Z[1, j] + _F(v2 * Z[2, j]))
                tvz = _F(tau * vtz)
                Z[1, j] = _F(Z[1, j] - tvz)
                Z[2, j] = _F(Z[2, j] - _F(v2 * tvz))
        W[i] = w
        V[i] = Z
    return W, V


# ------------------------------------------------------------- weights ----

def _prep_weights(ins):
    """Host-side packing of the model weights into device layouts.

    Strided column-pair scheme (see the layer maps below); conv0/conv1
    biases ride a ones-row in the rhs so the matmul itself adds them.
    """
    w0, w1, w2, w3 = ins["w0"], ins["w1"], ins["w2"], ins["w3"]
    b0, b1 = np.asarray(ins["b0"], np.float32), np.asarray(ins["b1"],
                                                           np.float32)

    d = {}
    # conv0: window rows (c:3, j:9): even cols l = 8q-1+j, odd l = 8q+j;
    # k = j - 2g for output slot g; row 27 = ones -> bias.
    W0 = np.zeros((28, 80), np.float32)
    for c in range(3):
        for j in range(9):
            for g in range(4):
                k = j - 2 * g
                if 0 <= k < 3:
                    for o in range(20):
                        W0[c * 9 + j, g * 20 + o] = w0[o, c, k]
    W0[27, :] = np.tile(b0, 4)
    d["W0"] = W0.astype(BF)

    def s1_rows(with_hl, with_hr):
        rows = [(g * 20, 20, g) for g in range(4)]
        if with_hl:
            rows.append((80, 20, -1))
        if with_hr:
            rows.append((100, 20, 4))
        return rows

    def mk(w, blocks, Ghalf, parity, Cout, shift, colbase=None):
        Cin = w.shape[1]
        K = max(rb + Cin for rb, _, _ in blocks)
        if colbase is None:
            colbase = [g * Cout for g in range(Ghalf)]
        W = np.zeros((K, max(colbase) + Cout), np.float32)
        for rb, _, lrel in blocks:
            for g in range(Ghalf):
                pos = 2 * g + parity
                k = (lrel + shift) - pos + 1
                if 0 <= k < 3:
                    for ci in range(Cin):
                        W[rb + ci, colbase[g] + np.arange(Cout)] = w[:, ci, k]
        return W

    # conv1 output M-order: g0->0, g1->64, g2->96, g3->32 (C1B) so conv2's
    # boundary reads sit at legal rhs bases.
    C1B = [0, 64, 96, 32]

    # baseline-layout W1 blocks, then re-rowed for the s1 layout with the
    # ones row at 80: main [0:80], ones 80, hl [81:101], hr [101:121].
    w1e1_base = mk(w1, s1_rows(True, False), 4, 0, 32, 0, C1B)   # [100,128]
    w1e2 = mk(w1, [(rb, 20, lr + 4) for rb, _, lr in
                   s1_rows(False, False)], 4, 0, 32, 0, C1B)     # [80,128]
    w1o1 = mk(w1, s1_rows(False, False), 4, 1, 32, 0, C1B)       # [80,128]
    w1o2_blocks = ([(g * 20, 20, g + 4) for g in range(4)] +
                   [(80, 20, 1000), (100, 20, 8)])
    w1o2_base = mk(w1, w1o2_blocks, 4, 1, 32, 0, C1B)            # [120,128]
    b1t = np.tile(b1, 4)
    W1e1 = np.zeros((101, 128), np.float32)
    W1e1[0:80] = w1e1_base[0:80]
    W1e1[80] = b1t
    W1e1[81:101] = w1e1_base[80:100]
    W1o2 = np.zeros((121, 128), np.float32)
    W1o2[0:80] = w1o2_base[0:80]
    W1o2[80] = b1t
    W1o2[101:121] = w1o2_base[100:120]
    d["W1e1"] = W1e1.astype(BF)
    d["W1e2"] = w1e2.astype(BF)
    d["W1o1"] = w1o1.astype(BF)
    d["W1o2"] = W1o2.astype(BF)

    # conv2 (G=4, Ghalf=2, Cout=64): stored2 rows (g:4, o:32)->128
    s2_main = [(0, 32, 0), (64, 32, 1), (96, 32, 2), (32, 32, 3)]
    d["W2e1"] = mk(w2, [(0, 32, -1)], 2, 0, 64, 0).astype(BF)
    d["W2e2"] = mk(w2, s2_main, 2, 0, 64, 0).astype(BF)
    d["W2o1"] = mk(w2, s2_main, 2, 1, 64, 0).astype(BF)
    d["W2o2"] = mk(w2, [(0, 32, 4)], 2, 1, 64, 0).astype(BF)

    # conv3 (G=12, Ghalf=6, Cout=20): stored3 rows (g:2, o:64)->128.
    # Window w covers pre-pool pos [12w, 12w+12); MM t reads s3 col 6w+t
    # (l3 = 12w + 2t - 2 + g); M = (h:6, o:20) = 120.
    w3e = np.zeros((128, 7 * 120), np.float32)
    w3o = np.zeros((128, 7 * 120), np.float32)
    for t in range(7):
        w3e[:, t * 120:(t + 1) * 120] = mk(
            w3, [(0, 64, 2 * t - 2), (64, 64, 2 * t - 1)], 6, 0, 20, 0)
        w3o[:, t * 120:(t + 1) * 120] = mk(
            w3, [(0, 64, 2 * t), (64, 64, 2 * t + 1)], 6, 1, 20, 0)
    d["W3E"] = w3e.astype(BF)
    d["W3O"] = w3o.astype(BF)

    # fc: stored4 rows (h:6, o:20)->120, col w: feature (o, l4 = 6w + h)
    wl0 = ins["wl0"]
    WFC = np.zeros((120, 63 * 100), np.float32)
    for w in range(63):
        for h in range(6):
            l4 = 6 * w + h
            if l4 < 375:
                WFC[h * 20:(h + 1) * 20, w * 100:(w + 1) * 100] = \
                    wl0[:, np.arange(20)[:, None] * 375 + l4].T.reshape(
                        20, 100)
    d["WFC"] = WFC.astype(BF)

    d["B2"] = np.tile(ins["b2"], 2).astype(np.float32)[:, None]   # [128]
    d["B3"] = np.tile(ins["b3"], 6).astype(np.float32)[:, None]   # [120]
    # launch 2
    d["wcT"] = ins["wc"][:, :, 0].T.astype(np.float32).copy()      # [3, 20]
    d["bc"] = ins["bc"].astype(np.float32)[:, None]                # [20, 1]
    w0b = np.zeros((7, 20, 100), np.float32)
    for t in range(7):
        for o in range(20):
            w0b[t, o] = ins["wl0"][:, 7500 + o * 7 + t]
    d["w0bT"] = w0b
    d["bl0"] = ins["bl0"].astype(np.float32)[:, None]              # [100, 1]
    d["wl1T"] = ins["wl1"].T.astype(np.float32).copy()             # [100, 2]
    d["bl1"] = ins["bl1"].astype(np.float32)[:, None]              # [2, 1]
    return d


# ------------------------------------------------------------- launch 1 ----

def _build_launch1():
    nc = bacc.Bacc("TRN2", target_bir_lowering=False, debug=False,
                   num_devices=NCORES)
    dram = {}
    for nm, shape, dt in [
        ("x_winE", [28, NS, 750], BF16), ("x_winO", [28, NS, 750], BF16),
        ("x_f32", [NS, 3, L0], F32),
        ("W0", [28, 80], BF16),
        ("W1e1", [101, 128], BF16), ("W1e2", [80, 128], BF16),
        ("W1o1", [80, 128], BF16), ("W1o2", [121, 128], BF16),
        ("W2e1", [32, 128], BF16), ("W2e2", [128, 128], BF16),
        ("W2o1", [128, 128], BF16), ("W2o2", [32, 128], BF16),
        ("W3E", [128, 840], BF16), ("W3O", [128, 840], BF16),
        ("WFC", [120, 6300], BF16),
        ("B2", [128, 1], F32), ("B3", [120, 1], F32),
        ("INIT1", [1, BN, 750], BF16), ("INITZ", [128, BN, 1], BF16),
    ]:
        dram[nm] = nc.dram_tensor(nm, shape, dt, kind="ExternalInput").ap()
    out_p0 = nc.dram_tensor("partial0", [100, NS], F32,
                            kind="ExternalOutput").ap()
    out_mom = nc.dram_tensor("mom", [NS, 72], F32,
                             kind="ExternalOutput").ap()

    with tile.TileContext(nc) as tc:
        with tc.tile_pool(name="wpool", bufs=1) as wp, \
             tc.tile_pool(name="covp", bufs=1) as cvp, \
             tc.tile_pool(name="scrp", bufs=4) as scp, \
             tc.tile_pool(name="xw", bufs=2) as xwp, \
             tc.tile_pool(name="s1p", bufs=1) as s1p, \
             tc.tile_pool(name="s2p", bufs=1) as s2p, \
             tc.tile_pool(name="s3p", bufs=1) as s3p, \
             tc.tile_pool(name="s4p", bufs=1) as s4p, \
             tc.tile_pool(name="tep", bufs=4) as tep, \
             tc.tile_pool(name="ps", bufs=4, space="PSUM") as psp:

            xw_tiles = {}

            def issue_xw(b):
                if b >= NBLK or b in xw_tiles:
                    return
                n0 = b * BN
                te = xwp.tile([28, BN, 750], BF16, tag="xwE")
                nc.sync.dma_start(te[:], dram["x_winE"][:, n0:n0 + BN, :])
                to = xwp.tile([28, BN, 750], BF16, tag="xwO")
                nc.sync.dma_start(to[:], dram["x_winO"][:, n0:n0 + BN, :])
                xw_tiles[b] = (te, to)

            # The SP DMA queue is FIFO and a DMA holds HWDGE ~0.6us each:
            # preload ONLY what conv0(0)/conv1(0) need, trickle the rest
            # into the pipeline steps below via dma_sched.
            momt = cvp.tile([NS, 72], F32, tag="mom")
            p0sb = cvp.tile([100, NS], F32, tag="p0sb")
            s1 = s1p.tile([121, BN, 750], BF16, tag="s1")
            s2 = s2p.tile([128, BN, 377], BF16, tag="s2")
            s3 = s3p.tile([128, BN, 380], BF16, tag="s3")
            s4 = s4p.tile([120, NS, 63], BF16, tag="s4")
            xcv = cvp.tile([NS, 3, L0], F32, tag="xcv")

            Ws = {}
            for nm in ["W1e1", "W1e2", "W1o1", "W1o2", "W0",
                       "W2e2", "W2o1", "W2o2", "W3E", "W3O", "WFC"]:
                Ws[nm] = wp.tile(list(dram[nm].shape), BF16, name=nm, tag=nm)
            Wpad = wp.tile([64, 128], BF16, name="W2e1", tag="W2e1")
            Ws["W2e1"] = Wpad[32:64]
            B2t = wp.tile([128, 1], F32, tag="B2")
            B3t = wp.tile([120, 1], F32, tag="B3")

            def dma_w(nm):
                nc.sync.dma_start(Ws[nm][:], dram[nm][:])

            # prologue: conv0(0..1) + conv1(0) prerequisites only
            dma_w("W0")
            issue_xw(0)
            issue_xw(1)
            # ones row + hl/hr edges (engines cannot address partition
            # bases off the 0/32/64/96 grid; DMA can); conv1 prereqs ride
            # the otherwise-idle Act HWDGE queue in parallel
            nc.scalar.dma_start(s1[80:81, :, :], dram["INIT1"][:])
            nc.scalar.dma_start(s1[81:101, :, 0:1], dram["INITZ"][0:20])
            nc.scalar.dma_start(s1[101:121, :, 749:750], dram["INITZ"][0:20])
            for nm in ["W1e1", "W1e2", "W1o1", "W1o2"]:
                nc.scalar.dma_start(Ws[nm][:], dram[nm][:])

            def dmas_step0():
                nc.sync.dma_start(Wpad[32:64], dram["W2e1"][:])
                for nm in ["W2e2", "W2o1", "W2o2"]:
                    dma_w(nm)
                nc.sync.dma_start(B2t[:], dram["B2"][:])
                nc.sync.dma_start(s2[:, :, 0:1], dram["INITZ"][:])
                nc.sync.dma_start(s2[:, :, 376:377], dram["INITZ"][:])

            def dmas_step1():
                dma_w("W3E")

            def dmas_step2():
                dma_w("W3O")
                nc.sync.dma_start(B3t[:], dram["B3"][:])
                nc.sync.dma_start(s3[:, :, 0:1], dram["INITZ"][:])
                for cz in range(376, 380):
                    nc.sync.dma_start(s3[:, :, cz:cz + 1], dram["INITZ"][:])

            def make_xcv_piece(i):
                c0 = i * 375
                return lambda: nc.sync.dma_start(
                    xcv[:, :, c0:c0 + 375], dram["x_f32"][:, :, c0:c0 + 375])

            def make_wfc_piece(i):
                c0 = i * 1575
                return lambda: nc.sync.dma_start(
                    Ws["WFC"][:, c0:c0 + 1575], dram["WFC"][:, c0:c0 + 1575])

            dma_sched = {0: dmas_step0, 1: dmas_step1, 2: dmas_step2}
            for i in range(16):
                dma_sched[3 + i] = make_xcv_piece(i)
            for i in range(4):
                dma_sched[20 + 2 * i] = make_wfc_piece(i)

            # covariance moment ops, chunked 8x750 so no single op can
            # head-of-line block the Act/DVE queues; 72 partial sums,
            # host adds the 8 chunks per moment.
            cov_ops = []
            for ch in range(8):
                for c in range(3):
                    cov_ops.append(("mean", c, ch))
                    cov_ops.append(("sq", c, ch))
                for k, (c, dch) in enumerate([(0, 1), (0, 2), (1, 2)]):
                    cov_ops.append(("xy", k, c, dch, ch))
            # Act gets exactly one ~1us cov op per step, DVE one per two
            # steps: never more than the pipeline's per-step engine slack.
            act_ops = [op for op in cov_ops if op[0] != "xy"]
            dve_ops = [op for op in cov_ops if op[0] == "xy"]
            cov_sched = {}
            for i, op in enumerate(act_ops):
                cov_sched.setdefault(6 + i, []).append(op)
            for i, op in enumerate(dve_ops):
                cov_sched.setdefault(6 + 2 * i, []).append(op)

            def emit_cov(op):
                ch = op[-1]
                c0 = ch * 750
                sl = slice(c0, c0 + 750)
                scr = scp.tile([NS, 750], BF16, tag="cscr")
                if op[0] == "mean":
                    c = op[1]
                    nc.scalar.activation(scr[:], xcv[:, c, sl], ACTF.Copy,
                                         accum_out=momt[:, c * 8 + ch:
                                                        c * 8 + ch + 1])
                elif op[0] == "sq":
                    c = op[1]
                    nc.scalar.activation(scr[:], xcv[:, c, sl], ACTF.Square,
                                         accum_out=momt[:, 24 + c * 8 + ch:
                                                        24 + c * 8 + ch + 1])
                else:
                    _, k, c, dch, ch = op
                    nc.vector.scalar_tensor_tensor(
                        scr[:], xcv[:, c, sl], 1.0, xcv[:, dch, sl],
                        AOP.mult, AOP.mult,
                        accum_out=momt[:, 48 + k * 8 + ch:
                                       48 + k * 8 + ch + 1])

            def conv0(p):
                blk, nb = p // 4, (p % 4) * 2
                xwE, xwO = xw_tiles[blk]
                for ch in range(2):
                    c0 = ch * 375
                    psE = psp.tile([128, 2, 512], F32, tag="ps")
                    psO = psp.tile([128, 2, 512], F32, tag="ps")
                    for i in range(2):
                        nc.tensor.matmul(psE[0:80, i, 0:375], Ws["W0"][:],
                                         xwE[:, nb + i, c0:c0 + 375],
                                         start=True, stop=True)
                    for i in range(2):
                        nc.tensor.matmul(psO[0:80, i, 0:375], Ws["W0"][:],
                                         xwO[:, nb + i, c0:c0 + 375],
                                         start=True, stop=True)
                    tE = tep.tile([128, 2, 384], BF16, tag="tE")
                    nc.scalar.activation(tE[0:80, :, 0:375],
                                         psE[0:80, :, 0:375], ACTF.Relu)
                    nc.vector.scalar_tensor_tensor(
                        s1[0:80, nb:nb + 2, c0:c0 + 375],
                        psO[0:80, :, 0:375], 0.0, tE[0:80, :, 0:375],
                        AOP.max, AOP.max)
                # per-pair halo rows for conv1
                nc.sync.dma_start(s1[81:101, nb:nb + 2, 1:750],
                                  s1[60:80, nb:nb + 2, 0:749])
                nc.sync.dma_start(s1[101:121, nb:nb + 2, 0:749],
                                  s1[0:20, nb:nb + 2, 1:750])

            def conv1(p):
                nb = (p % 4) * 2
                psE = psp.tile([128, 2, 512], F32, tag="ps")
                psO = psp.tile([128, 2, 512], F32, tag="ps")
                for i in range(2):
                    n = nb + i
                    nc.tensor.matmul(psE[0:128, i, 0:375], Ws["W1e1"][:],
                                     s1[0:101, n, 0:750:2],
                                     start=True, stop=False)
                    nc.tensor.matmul(psE[0:128, i, 0:375], Ws["W1e2"][:],
                                     s1[0:80, n, 1:750:2],
                                     start=False, stop=True)
                for i in range(2):
                    n = nb + i
                    nc.tensor.matmul(psO[0:128, i, 0:375], Ws["W1o1"][:],
                                     s1[0:80, n, 0:750:2],
                                     start=True, stop=False)
                    nc.tensor.matmul(psO[0:128, i, 0:375], Ws["W1o2"][:],
                                     s1[0:121, n, 1:750:2],
                                     start=False, stop=True)
                tE = tep.tile([128, 2, 384], BF16, tag="tE")
                nc.scalar.activation(tE[0:128, :, 0:375],
                                     psE[0:128, :, 0:375], ACTF.Relu)
                nc.vector.scalar_tensor_tensor(
                    s2[0:128, nb:nb + 2, 1:376],
                    psO[0:128, :, 0:375], 0.0, tE[0:128, :, 0:375],
                    AOP.max, AOP.max)

            def conv2(p):
                nb = (p % 4) * 2
                psE = psp.tile([128, 2, 512], F32, tag="ps")
                psO = psp.tile([128, 2, 512], F32, tag="ps")
                for i in range(2):
                    n = nb + i
                    nc.tensor.matmul(psE[0:128, i, 0:375], Ws["W2e1"],
                                     s2[32:64, n, 0:375],
                                     start=True, stop=False)
                    nc.tensor.matmul(psE[0:128, i, 0:375], Ws["W2e2"][:],
                                     s2[0:128, n, 1:376],
                                     start=False, stop=True)
                for i in range(2):
                    n = nb + i
                    nc.tensor.matmul(psO[0:128, i, 0:375], Ws["W2o1"][:],
                                     s2[0:128, n, 1:376],
                                     start=True, stop=False)
                    nc.tensor.matmul(psO[0:128, i, 0:375], Ws["W2o2"][:],
                                     s2[0:32, n, 2:377],
                                     start=False, stop=True)
                tE = tep.tile([128, 2, 384], BF16, tag="tE")
                nc.scalar.activation(tE[0:128, :, 0:375],
                                     psE[0:128, :, 0:375], ACTF.Relu,
                                     bias=B2t[:])
                nc.vector.scalar_tensor_tensor(
                    s3[0:128, nb:nb + 2, 1:376],
                    psO[0:128, :, 0:375], B2t[:], tE[0:128, :, 0:375],
                    AOP.add, AOP.max)

            def conv3(qd):
                blk, nq = qd // 2, (qd % 2) * 4
                n0 = blk * BN + nq
                psE = psp.tile([128, 2, 512], F32, tag="ps")
                psO = psp.tile([128, 2, 512], F32, tag="ps")
                for t in range(7):
                    nc.tensor.matmul(
                        psE[0:120, 0, 0:252],
                        Ws["W3E"][:, t * 120:(t + 1) * 120],
                        s3[0:128, nq:nq + 4, t:t + 373:6],
                        start=(t == 0), stop=(t == 6))
                for t in range(7):
                    nc.tensor.matmul(
                        psO[0:120, 0, 0:252],
                        Ws["W3O"][:, t * 120:(t + 1) * 120],
                        s3[0:128, nq:nq + 4, t + 1:t + 374:6],
                        start=(t == 0), stop=(t == 6))
                tE = tep.tile([128, 512], BF16, tag="tE3")
                nc.scalar.activation(tE[0:120, 0:252], psE[0:120, 0, 0:252],
                                     ACTF.Relu, bias=B3t[:])
                nc.vector.scalar_tensor_tensor(
                    s4[0:120, n0:n0 + 4, 0:63],
                    psO[0:120, 0, 0:252].rearrange("p (n l) -> p n l", n=4),
                    B3t[:],
                    tE[0:120, 0:252].rearrange("p (n l) -> p n l", n=4),
                    AOP.add, AOP.max)

            def fc(half):
                c0 = half * 64
                psfc = psp.tile([128, 2, 512], F32, tag="ps")
                for w in range(63):
                    nc.tensor.matmul(
                        psfc[0:100, 0, 0:64],
                        Ws["WFC"][:, w * 100:(w + 1) * 100],
                        s4[:, c0:c0 + 64, w], start=(w == 0), stop=(w == 62))
                nc.scalar.copy(p0sb[:, c0:c0 + 64], psfc[0:100, 0, 0:64])

            # pipeline: conv0(p) | conv1(p-1) | conv2(p-2) |
            #           conv3((p-3)/2 @ odd p) | fc halves at p=35/66
            for p in range(68):
                if p < NPAIR:
                    if p % 4 == 0:
                        issue_xw(p // 4 + 2)
                    conv0(p)
                if 0 <= p - 1 < NPAIR:
                    conv1(p - 1)
                if 0 <= p - 2 < NPAIR:
                    conv2(p - 2)
                if p % 2 == 0 and 0 <= (p - 4) // 2 < 32:
                    conv3((p - 4) // 2)
                if p == 35 or p == 67:
                    fc(0 if p == 35 else 1)
                for op in cov_sched.get(p, ()):
                    emit_cov(op)
                if p in dma_sched:
                    dma_sched[p]()

            nc.sync.dma_start(out_p0[:], p0sb[:])
            nc.sync.dma_start(out_mom[:], momt[:])

    nc.compile()
    return nc


# ------------------------------------------------------------- launch 2 ----

def _build_launch2():
    nc = bacc.Bacc("TRN2", target_bir_lowering=False, debug=False,
                   num_devices=NCORES)
    dr = {}
    for nm, shape in [("featsT", [3, 7 * NS]), ("p0T", [100, NS]),
                      ("wcT", [3, 20]), ("bc", [20, 1]),
                      ("w0bT", [7, 20, 100]), ("bl0", [100, 1]),
                      ("wl1T", [100, 2]), ("bl1", [2, 1])]:
        dr[nm] = nc.dram_tensor(nm, shape, F32, kind="ExternalInput").ap()
    out2 = nc.dram_tensor("out2", [2, NS], F32, kind="ExternalOutput").ap()

    with tile.TileContext(nc) as tc:
        with tc.tile_pool(name="w2p", bufs=1) as wp, \
             tc.tile_pool(name="ps2", bufs=2, space="PSUM") as psp:
            fT = wp.tile([3, 7 * NS], F32, tag="fT")
            nc.sync.dma_start(fT[:], dr["featsT"][:])
            p0T = wp.tile([100, NS], F32, tag="p0T")
            nc.sync.dma_start(p0T[:], dr["p0T"][:])
            wcT = wp.tile([3, 20], F32, tag="wcT")
            nc.sync.dma_start(wcT[:], dr["wcT"][:])
            bc = wp.tile([20, 1], F32, tag="bc")
            nc.sync.dma_start(bc[:], dr["bc"][:])
            w0bT = [wp.tile([20, 100], F32, name=f"w0bT{t}", tag=f"w0bT{t}")
                    for t in range(7)]
            for t in range(7):
                nc.sync.dma_start(w0bT[t][:], dr["w0bT"][t])
            bl0 = wp.tile([100, 1], F32, tag="bl0")
            nc.sync.dma_start(bl0[:], dr["bl0"][:])
            wl1T = wp.tile([100, 2], F32, tag="wl1T")
            nc.sync.dma_start(wl1T[:], dr["wl1T"][:])
            bl1 = wp.tile([2, 1], F32, tag="bl1")
            nc.sync.dma_start(bl1[:], dr["bl1"][:])

            # h1 = relu(wc @ feats + bc): [20, (t, n)]
            h1 = wp.tile([20, 7 * NS], F32, tag="h1")
            for half in range(2):
                c0 = half * 448
                ps = psp.tile([32, 448], F32, tag="ph")
                nc.tensor.matmul(ps[0:20, :], wcT[:], fT[:, c0:c0 + 448],
                                 start=True, stop=True)
                nc.scalar.activation(h1[:, c0:c0 + 448], ps[0:20, :],
                                     ACTF.Relu, bias=bc[:])
            # z = relu(p0 + sum_t w0b_t.T @ h1_t + bl0)
            psz = psp.tile([100, NS], F32, tag="pz")
            for t in range(7):
                nc.tensor.matmul(psz[:], w0bT[t][:],
                                 h1[:, t * NS:(t + 1) * NS],
                                 start=(t == 0), stop=(t == 6))
            z = wp.tile([100, NS], F32, tag="z")
            nc.vector.scalar_tensor_tensor(z[:], psz[:], bl0[:], p0T[:],
                                           AOP.add, AOP.add)
            nc.vector.tensor_scalar_max(z[:], z[:], 0.0)
            pso = psp.tile([32, NS], F32, tag="po")
            nc.tensor.matmul(pso[0:2, :], wl1T[:], z[:],
                             start=True, stop=True)
            osb = wp.tile([2, NS], F32, tag="osb")
            nc.vector.tensor_scalar(osb[:], pso[0:2, :], bl1[:], None,
                                    AOP.add)
            nc.sync.dma_start(out2[:], osb[:])

    nc.compile()
    return nc


# --------------------------------------------------------------- kernel ----

def kernel(**inputs):
    ins = {k: np.asarray(v) for k, v in inputs.items()}
    x = ins["x"].astype(np.float32)

    if "l1" not in _CACHE:
        _CACHE["l1"] = _build_launch1()
    if "l2" not in _CACHE:
        _CACHE["l2"] = _build_launch2()
    w = _prep_weights(ins)

    xbf = x.astype(BF)
    xwE = np.zeros((28, x.shape[0], 750), BF)
    xwO = np.zeros((28, x.shape[0], 750), BF)
    for c in range(3):
        for j in range(9):
            # even cols: l = 8q - 1 + j ; odd cols: l = 8q + j
            if j == 0:
                xwE[c * 9 + 0, :, 1:750] = xbf[:, c, 7:5992:8]
            else:
                xwE[c * 9 + j] = xbf[:, c, j - 1::8]
            if j == 8:
                xwO[c * 9 + 8, :, 0:749] = xbf[:, c, 8:6000:8]
            else:
                xwO[c * 9 + j] = xbf[:, c, j::8]
    xwE[27] = 1.0
    xwO[27] = 1.0
    shards = [x[i * NS:(i + 1) * NS] for i in range(NCORES)]
    in1 = []
    for i, sh in enumerate(shards):
        sl = slice(i * NS, (i + 1) * NS)
        m = {"x_winE": np.ascontiguousarray(xwE[:, sl]),
             "x_winO": np.ascontiguousarray(xwO[:, sl]),
             "x_f32": sh}
        for nm in ["W0", "W1e1", "W1e2", "W1o1", "W1o2",
                   "W2e1", "W2e2", "W2o1", "W2o2", "W3E", "W3O",
                   "WFC", "B2", "B3"]:
            m[nm] = w[nm]
        m["INIT1"] = np.ones((1, BN, 750), BF)
        m["INITZ"] = np.zeros((128, BN, 1), BF)
        in1.append(m)
    t0 = time.time()
    res1 = run_bass_kernel_spmd(_CACHE["l1"], in1, list(range(NCORES)))
    LAST_EXEC_NS[0] = res1.exec_time_ns
    LAST_WALL_S[0] = time.time() - t0

    mom72 = np.concatenate([res1.results[i]["mom"] for i in range(NCORES)],
                           0).astype(np.float32)
    mom = mom72.reshape(-1, 9, 8).sum(-1)
    partial0 = np.concatenate(
        [res1.results[i]["partial0"].T for i in range(NCORES)], 0)

    # host: cov assembly (fp32) + LAPACK-clone eigh + global normalizers
    Sx = mom[:, 0:3].astype(np.float32)
    Sxx = mom[:, 3:6].astype(np.float32)
    Sxy = mom[:, 6:9].astype(np.float32)
    L = np.float32(L0)
    cov = np.empty((x.shape[0], 3, 3), np.float32)
    for idx, (c, dch) in enumerate([(0, 1), (0, 2), (1, 2)]):
        v = (Sxy[:, idx] - Sx[:, c] * Sx[:, dch] / L) / np.float32(L0 - 1)
        cov[:, c, dch] = v
        cov[:, dch, c] = v
    for c in range(3):
        cov[:, c, c] = (Sxx[:, c] - Sx[:, c] * Sx[:, c] / L) / np.float32(
            L0 - 1)
    vals, vecs = _eigh3_batch(cov)
    covn = cov / np.abs(cov).max()
    valsn = (vals / vals.max())[..., None]
    feats = np.concatenate([covn, valsn, vecs], axis=-1).astype(np.float32)

    in2 = []
    for i in range(NCORES):
        sl = slice(i * NS, (i + 1) * NS)
        m = {"featsT": np.ascontiguousarray(
                 feats[sl].transpose(1, 2, 0).reshape(3, 7 * NS)),
             "p0T": partial0[sl].T.copy(),
             "wcT": w["wcT"], "bc": w["bc"], "w0bT": w["w0bT"],
             "bl0": w["bl0"], "wl1T": w["wl1T"], "bl1": w["bl1"]}
        in2.append(m)
    t0 = time.time()
    res2 = run_bass_kernel_spmd(_CACHE["l2"], in2, list(range(NCORES)))
    LAST_EXEC_NS[1] = res2.exec_time_ns
    LAST_WALL_S[1] = time.time() - t0

    out = np.concatenate([res2.results[i]["out2"].T for i in range(NCORES)],
                         0).astype(np.float32)
    return (out[:, 0:1], out[:, 1:2])


# revision 20
# speedup vs baseline: 1.2208x; 1.0016x over previous
"""Trainium2 Bass kernel for nn_BAZ_Network (dense CNN + cov/eig head).

Data-parallel over 8 NeuronCores: 128 samples each.

Launch 1 (per core), software-pipelined over 64 sample-pairs:
  conv trunk as G-packed banded-weight matmuls (bf16, fp32 PSUM), with
  conv biases folded into the matmuls via a ones-row in the rhs (conv0,
  conv1).  Postprocess per (E,O) parity pair is two fused ops over a
  2-sample two-PSUM-bank 3D access pattern:
    op1 (Act):  tE = relu(psE + b)           PSUM -> SBUF bf16
    op2 (DVE):  s  = max(psO + b, tE)        = relu(max(E,O)+b), the
                 maxpool, relu, bias and bf16 cast in one instruction.
  Covariance fp32 raw moments (means on Act copy+accum, squares on Act
  Square+accum, cross-products on DVE stt+accum) are interleaved into
  the pair pipeline; the FC contraction of the conv features against
  wl0[:, :7500] runs per-block (125 accumulating matmuls of 8 cols).
  Stage stagger: conv0(p) | conv1(p-1) | conv2(p-2) | conv3 at odd p |
  FC at p%4==2, which hides the halo-DMA and PSUM-evacuation latency.
Host: branch-exact fp32 netlib-LAPACK ssyevd clone for the 3x3 eigh
  (required to reproduce jnp.linalg.eigh eigenvector signs).
Launch 2 (per core): eig-feature head: 1x1 conv (wc) + relu, remaining
  FC columns wl0[:, 7500:], bias+relu, final linear wl1.
"""

import os
import sys
import time
import numpy as np
import ml_dtypes

sys.path.insert(0, "/opt/trn_rl_repo")
os.environ["BASS_NEVER_TRACE"] = "1"

import concourse.bass as bass  # noqa: E402
import concourse.tile as tile  # noqa: E402
import concourse.mybir as mybir  # noqa: E402
from concourse import bacc  # noqa: E402
from concourse.bass_utils import run_bass_kernel_spmd  # noqa: E402

F32 = mybir.dt.float32
BF16 = mybir.dt.bfloat16
AOP = mybir.AluOpType
ACTF = mybir.ActivationFunctionType
BF = ml_dtypes.bfloat16

NCORES = 8
NS = 128          # samples per core
BN = 8            # samples per block
NBLK = NS // BN
NPAIR = NS // 2   # 64 sample-pairs, the pipeline unit
L0 = 6000

LAST_EXEC_NS = [None, None]
LAST_WALL_S = [None, None]
_CACHE = {}


# ---------------------------------------------------------------- eigh ----
# fp32 netlib-LAPACK ssyevd clone for n=3 (jobz='V', uplo='L').
# Matches jaxlib's CPU eigh (LAPACK >= 3.10 slartg) bit-closely: 0/3072
# eigenvector sign mismatches on the problem distribution.

_F = np.float32
_EPS = _F(np.finfo(np.float32).eps) * _F(0.5)
_EPS2 = _EPS * _EPS
_SAFMIN = _F(np.finfo(np.float32).tiny)


def _slapy2(x, y):
    xa, ya = abs(x), abs(y)
    w, z = max(xa, ya), min(xa, ya)
    if z == 0:
        return w
    return _F(w * _F(np.sqrt(_F(_F(1.0) + _F(_F(z / w) * _F(z / w))))))


def _sign(a, b):
    return abs(a) if b >= 0 else -abs(a)


def _slartg(f, g):
    if g == _F(0.0):
        return _F(1.0), _F(0.0), f
    if f == _F(0.0):
        return _F(0.0), _sign(_F(1.0), g), abs(g)
    d = _F(np.sqrt(_F(f * f + g * g)))
    c = _F(abs(f) / d)
    r = _sign(d, f)
    s = _F(g / r)
    return c, s, r


def _slaev2(a, b, c):
    sm = _F(a + c)
    df = _F(a - c)
    adf = abs(df)
    tb = _F(b + b)
    ab = abs(tb)
    acmx, acmn = (a, c) if abs(a) > abs(c) else (c, a)
    if adf > ab:
        t = _F(ab / adf)
        rt = _F(adf * _F(np.sqrt(_F(_F(1.0) + _F(t * t)))))
    elif adf < ab:
        t = _F(adf / ab)
        rt = _F(ab * _F(np.sqrt(_F(_F(1.0) + _F(t * t)))))
    else:
        rt = _F(ab * _F(np.sqrt(_F(2.0))))
    if sm < 0:
        rt1 = _F(_F(0.5) * _F(sm - rt))
        sgn1 = -1
        rt2 = _F(_F(_F(acmx / rt1) * acmn) - _F(_F(b / rt1) * b))
    elif sm > 0:
        rt1 = _F(_F(0.5) * _F(sm + rt))
        sgn1 = 1
        rt2 = _F(_F(_F(acmx / rt1) * acmn) - _F(_F(b / rt1) * b))
    else:
        rt1 = _F(_F(0.5) * rt)
        rt2 = _F(_F(-0.5) * rt)
        sgn1 = 1
    if df >= 0:
        cs = _F(df + rt)
        sgn2 = 1
    else:
        cs = _F(df - rt)
        sgn2 = -1
    acs = abs(cs)
    if acs > ab:
        ct = _F(-tb / cs)
        sn1 = _F(_F(1.0) / _F(np.sqrt(_F(_F(1.0) + _F(ct * ct)))))
        cs1 = _F(ct * sn1)
    else:
        if ab == 0:
            cs1, sn1 = _F(1.0), _F(0.0)
        else:
            tn = _F(-cs / tb)
            cs1 = _F(_F(1.0) / _F(np.sqrt(_F(_F(1.0) + _F(tn * tn)))))
            sn1 = _F(tn * cs1)
    if sgn1 == sgn2:
        cs1, sn1 = -sn1, cs1
    return rt1, rt2, cs1, sn1


def _ssytrd3(A):
    a00, a10, a20 = A[0, 0], A[1, 0], A[2, 0]
    a11, a21, a22 = A[1, 1], A[2, 1], A[2, 2]
    xnorm = abs(a20)
    if xnorm == _F(0.0):
        beta, v2, tau = a10, a20, _F(0.0)
    else:
        beta = -_sign(_slapy2(a10, xnorm), a10)
        tau = _F(_F(beta - a10) / beta)
        v2 = _F(a20 * _F(_F(1.0) / _F(a10 - beta)))
    e0 = beta
    if tau != _F(0.0):
        x0 = _F(_F(tau * a11) + _F(tau * _F(a21 * v2)))
        x1 = _F(_F(tau * a21) + _F(_F(tau * v2) * a22))
        sdot = _F(_F(x0 * _F(1.0)) + _F(x1 * v2))
        alpha = _F(_F(_F(-0.5) * tau) * sdot)
        w0 = _F(x0 + _F(alpha * _F(1.0)))
        w1 = _F(x1 + _F(alpha * v2))
        t1, t2 = -w0, _F(-1.0)
        a11 = _F(_F(a11 + _F(_F(1.0) * t1)) + _F(w0 * t2))
        a21 = _F(_F(a21 + _F(v2 * t1)) + _F(w1 * t2))
        t1b, t2b = -w1, -v2
        a22 = _F(_F(a22 + _F(v2 * t1b)) + _F(w1 * t2b))
    d = np.array([a00, a11, a22], np.float32)
    e = np.array([e0, a21, 0.0], np.float32)
    return d, e, v2, tau


def _ssteqr3(d, e):
    n = 3
    Z = np.eye(3, dtype=np.float32)
    wc = np.zeros(2, np.float32)
    ws = np.zeros(2, np.float32)
    nmaxit, jtot = 90, 0

    def lasr_b(l, m):
        for j in range(m - 1, l - 1, -1):
            c, s = wc[j - 1], ws[j - 1]
            if c != _F(1.0) or s != _F(0.0):
                for i in range(3):
                    t = Z[i, j]
                    Z[i, j] = _F(_F(c * t) - _F(s * Z[i, j - 1]))
                    Z[i, j - 1] = _F(_F(s * t) + _F(c * Z[i, j - 1]))

    def lasr_f(m, l):
        for j in range(m, l):
            c, s = wc[j - 1], ws[j - 1]
            if c != _F(1.0) or s != _F(0.0):
                for i in range(3):
                    t = Z[i, j]
                    Z[i, j] = _F(_F(c * t) - _F(s * Z[i, j - 1]))
                    Z[i, j - 1] = _F(_F(s * t) + _F(c * Z[i, j - 1]))

    l1 = 1
    while True:
        if l1 > n:
            break
        if l1 > 1:
            e[l1 - 2] = _F(0.0)
        m = n
        for mm in range(l1, n):
            tst = abs(e[mm - 1])
            if tst == _F(0.0):
                m = mm
                break
            if tst <= _F(_F(_F(np.sqrt(abs(d[mm - 1]))) *
                            _F(np.sqrt(abs(d[mm])))) * _EPS):
                e[mm - 1] = _F(0.0)
                m = mm
                break
        l = l1
        lend = m
        l1 = m + 1
        if lend == l:
            continue
        if abs(d[lend - 1]) < abs(d[l - 1]):
            lend, l = l, lend
        if lend > l:
            while True:  # QL
                m = lend
                if l != lend:
                    for mm in range(l, lend):
                        tst = _F(abs(e[mm - 1]) * abs(e[mm - 1]))
                        if tst <= _F(_F(_F(_EPS2 * abs(d[mm - 1])) *
                                        abs(d[mm])) + _SAFMIN):
                            m = mm
                            break
                if m < lend:
                    e[m - 1] = _F(0.0)
                p = d[l - 1]
                if m == l:
                    d[l - 1] = p
                    l += 1
                    if l <= lend:
                        continue
                    break
                if m == l + 1:
                    rt1, rt2, c, s = _slaev2(d[l - 1], e[l - 1], d[l])
                    wc[l - 1] = c
                    ws[l - 1] = s
                    lasr_b(l, l + 1)
                    d[l - 1] = rt1
                    d[l] = rt2
                    e[l - 1] = _F(0.0)
                    l += 2
                    if l <= lend:
                        continue
                    break
                if jtot == nmaxit:
                    break
                jtot += 1
                g = _F(_F(d[l] - p) / _F(_F(2.0) * e[l - 1]))
                r = _slapy2(g, _F(1.0))
                g = _F(_F(d[m - 1] - p) + _F(e[l - 1] / _F(g + _sign(r, g))))
                s = _F(1.0)
                c = _F(1.0)
                p = _F(0.0)
                for i in range(m - 1, l - 1, -1):
                    f = _F(s * e[i - 1])
                    b = _F(c * e[i - 1])
                    c, s, r = _slartg(g, f)
                    if i != m - 1:
                        e[i] = r
                    g = _F(d[i] - p)
                    r = _F(_F(_F(d[i - 1] - g) * s) + _F(_F(_F(2.0) * c) * b))
                    p = _F(s * r)
                    d[i] = _F(g + p)
                    g = _F(_F(c * r) - b)
                    wc[i - 1] = c
                    ws[i - 1] = -s
                lasr_b(l, m)
                d[l - 1] = _F(d[l - 1] - p)
                e[l - 1] = g
        else:
            while True:  # QR
                m = lend
                if l != lend:
                    for mm in range(l, lend, -1):
                        tst = _F(abs(e[mm - 2]) * abs(e[mm - 2]))
                        if tst <= _F(_F(_F(_EPS2 * abs(d[mm - 1])) *
                                        abs(d[mm - 2])) + _SAFMIN):
                            m = mm
                            break
                if m > lend:
                    e[m - 2] = _F(0.0)
                p = d[l - 1]
                if m == l:
                    d[l - 1] = p
                    l -= 1
                    if l >= lend:
                        continue
                    break
                if m == l - 1:
                    rt1, rt2, c, s = _slaev2(d[l - 2], e[l - 2], d[l - 1])
                    wc[m - 1] = c
                    ws[m - 1] = s
                    lasr_f(m, l)
                    d[l - 2] = rt1
                    d[l - 1] = rt2
                    e[l - 2] = _F(0.0)
                    l -= 2
                    if l >= lend:
                        continue
                    break
                if jtot == nmaxit:
                    break
                jtot += 1
                g = _F(_F(d[l - 2] - p) / _F(_F(2.0) * e[l - 2]))
                r = _slapy2(g, _F(1.0))
                g = _F(_F(d[m - 1] - p) + _F(e[l - 2] / _F(g + _sign(r, g))))
                s = _F(1.0)
                c = _F(1.0)
                p = _F(0.0)
                for i in range(m, l):
                    f = _F(s * e[i - 1])
                    b = _F(c * e[i - 1])
                    c, s, r = _slartg(g, f)
                    if i != m:
                        e[i - 2] = r
                    g = _F(d[i - 1] - p)
                    r = _F(_F(_F(d[i] - g) * s) + _F(_F(_F(2.0) * c) * b))
                    p = _F(s * r)
                    d[i - 1] = _F(g + p)
                    g = _F(_F(c * r) - b)
                    wc[i - 1] = c
                    ws[i - 1] = s
                lasr_f(m, l)
                d[l - 1] = _F(d[l - 1] - p)
                e[l - 2] = g
        if jtot >= nmaxit:
            break
    for ii in range(2, n + 1):
        i = ii - 1
        k = i
        p = d[i - 1]
        for j in range(ii, n + 1):
            if d[j - 1] < p:
                k = j
                p = d[j - 1]
        if k != i:
            d[k - 1] = d[i - 1]
            d[i - 1] = p
            tmp = Z[:, k - 1].copy()
            Z[:, k - 1] = Z[:, i - 1]
            Z[:, i - 1] = tmp
    return d, Z


def _eigh3_batch(covs):
    n = covs.shape[0]
    W = np.empty((n, 3), np.float32)
    V = np.empty((n, 3, 3), np.float32)
    for i in range(n):
        d, e, v2, tau = _ssytrd3(covs[i])
        w, Z = _ssteqr3(d, e)
        if tau != _F(0.0):
            for j in range(3):
                vtz = _F(Z[1, j] + _F(v2 * Z[2, j]))
                tvz = _F(tau * vtz)
                Z[1, j] = _F(Z[1, j] - tvz)
                Z[2, j] = _F(Z[2, j] - _F(v2 * tvz))
        W[i] = w
        V[i] = Z
    return W, V


# ------------------------------------------------------------- weights ----

def _prep_weights(ins):
    """Host-side packing of the model weights into device layouts.

    Strided column-pair scheme (see the layer maps below); conv0/conv1
    biases ride a ones-row in the rhs so the matmul itself adds them.
    """
    w0, w1, w2, w3 = ins["w0"], ins["w1"], ins["w2"], ins["w3"]
    b0, b1 = np.asarray(ins["b0"], np.float32), np.asarray(ins["b1"],
                                                           np.float32)

    d = {}
    # conv0: window rows (c:3, j:9): even cols l = 8q-1+j, odd l = 8q+j;
    # k = j - 2g for output slot g; row 27 = ones -> bias.
    W0 = np.zeros((28, 80), np.float32)
    for c in range(3):
        for j in range(9):
            for g in range(4):
                k = j - 2 * g
                if 0 <= k < 3:
                    for o in range(20):
                        W0[c * 9 + j, g * 20 + o] = w0[o, c, k]
    W0[27, :] = np.tile(b0, 4)
    d["W0"] = W0.astype(BF)

    def s1_rows(with_hl, with_hr):
        rows = [(g * 20, 20, g) for g in range(4)]
        if with_hl:
            rows.append((80, 20, -1))
        if with_hr:
            rows.append((100, 20, 4))
        return rows

    def mk(w, blocks, Ghalf, parity, Cout, shift, colbase=None):
        Cin = w.shape[1]
        K = max(rb + Cin for rb, _, _ in blocks)
        if colbase is None:
            colbase = [g * Cout for g in range(Ghalf)]
        W = np.zeros((K, max(colbase) + Cout), np.float32)
        for rb, _, lrel in blocks:
            for g in range(Ghalf):
                pos = 2 * g + parity
                k = (lrel + shift) - pos + 1
                if 0 <= k < 3:
                    for ci in range(Cin):
                        W[rb + ci, colbase[g] + np.arange(Cout)] = w[:, ci, k]
        return W

    # conv1 output M-order: g0->0, g1->64, g2->96, g3->32 (C1B) so conv2's
    # boundary reads sit at legal rhs bases.
    C1B = [0, 64, 96, 32]

    # baseline-layout W1 blocks, then re-rowed for the s1 layout with the
    # ones row at 80: main [0:80], ones 80, hl [81:101], hr [101:121].
    w1e1_base = mk(w1, s1_rows(True, False), 4, 0, 32, 0, C1B)   # [100,128]
    w1e2 = mk(w1, [(rb, 20, lr + 4) for rb, _, lr in
                   s1_rows(False, False)], 4, 0, 32, 0, C1B)     # [80,128]
    w1o1 = mk(w1, s1_rows(False, False), 4, 1, 32, 0, C1B)       # [80,128]
    w1o2_blocks = ([(g * 20, 20, g + 4) for g in range(4)] +
                   [(80, 20, 1000), (100, 20, 8)])
    w1o2_base = mk(w1, w1o2_blocks, 4, 1, 32, 0, C1B)            # [120,128]
    b1t = np.tile(b1, 4)
    W1e1 = np.zeros((101, 128), np.float32)
    W1e1[0:80] = w1e1_base[0:80]
    W1e1[80] = b1t
    W1e1[81:101] = w1e1_base[80:100]
    W1o2 = np.zeros((121, 128), np.float32)
    W1o2[0:80] = w1o2_base[0:80]
    W1o2[80] = b1t
    W1o2[101:121] = w1o2_base[100:120]
    d["W1e1"] = W1e1.astype(BF)
    d["W1e2"] = w1e2.astype(BF)
    d["W1o1"] = w1o1.astype(BF)
    d["W1o2"] = W1o2.astype(BF)

    # conv2 (G=4, Ghalf=2, Cout=64): stored2 rows (g:4, o:32)->128
    s2_main = [(0, 32, 0), (64, 32, 1), (96, 32, 2), (32, 32, 3)]
    d["W2e1"] = mk(w2, [(0, 32, -1)], 2, 0, 64, 0).astype(BF)
    d["W2e2"] = mk(w2, s2_main, 2, 0, 64, 0).astype(BF)
    d["W2o1"] = mk(w2, s2_main, 2, 1, 64, 0).astype(BF)
    d["W2o2"] = mk(w2, [(0, 32, 4)], 2, 1, 64, 0).astype(BF)

    # conv3 (G=12, Ghalf=6, Cout=20): stored3 rows (g:2, o:64)->128.
    # Window w covers pre-pool pos [12w, 12w+12); MM t reads s3 col 6w+t
    # (l3 = 12w + 2t - 2 + g); M = (h:6, o:20) = 120.
    w3e = np.zeros((128, 7 * 120), np.float32)
    w3o = np.zeros((128, 7 * 120), np.float32)
    for t in range(7):
        w3e[:, t * 120:(t + 1) * 120] = mk(
            w3, [(0, 64, 2 * t - 2), (64, 64, 2 * t - 1)], 6, 0, 20, 0)
        w3o[:, t * 120:(t + 1) * 120] = mk(
            w3, [(0, 64, 2 * t), (64, 64, 2 * t + 1)], 6, 1, 20, 0)
    d["W3E"] = w3e.astype(BF)
    d["W3O"] = w3o.astype(BF)

    # fc: stored4 rows (h:6, o:20)->120, col w: feature (o, l4 = 6w + h)
    wl0 = ins["wl0"]
    WFC = np.zeros((120, 63 * 100), np.float32)
    for w in range(63):
        for h in range(6):
            l4 = 6 * w + h
            if l4 < 375:
                WFC[h * 20:(h + 1) * 20, w * 100:(w + 1) * 100] = \
                    wl0[:, np.arange(20)[:, None] * 375 + l4].T.reshape(
                        20, 100)
    d["WFC"] = WFC.astype(BF)

    d["B2"] = np.tile(ins["b2"], 2).astype(np.float32)[:, None]   # [128]
    d["B3"] = np.tile(ins["b3"], 6).astype(np.float32)[:, None]   # [120]
    # launch 2
    d["wcT"] = ins["wc"][:, :, 0].T.astype(np.float32).copy()      # [3, 20]
    d["bc"] = ins["bc"].astype(np.float32)[:, None]                # [20, 1]
    w0b = np.zeros((7, 20, 100), np.float32)
    for t in range(7):
        for o in range(20):
            w0b[t, o] = ins["wl0"][:, 7500 + o * 7 + t]
    d["w0bT"] = w0b
    d["bl0"] = ins["bl0"].astype(np.float32)[:, None]              # [100, 1]
    d["wl1T"] = ins["wl1"].T.astype(np.float32).copy()             # [100, 2]
    d["bl1"] = ins["bl1"].astype(np.float32)[:, None]              # [2, 1]
    return d


# ------------------------------------------------------------- launch 1 ----

def _build_launch1():
    nc = bacc.Bacc("TRN2", target_bir_lowering=False, debug=False,
                   num_devices=NCORES)
    dram = {}
    for nm, shape, dt in [
        ("x_winE", [28, NS, 750], BF16), ("x_winO", [28, NS, 750], BF16),
        ("x_f32", [NS, 3, L0], F32),
        ("W0", [28, 80], BF16),
        ("W1e1", [101, 128], BF16), ("W1e2", [80, 128], BF16),
        ("W1o1", [80, 128], BF16), ("W1o2", [121, 128], BF16),
        ("W2e1", [32, 128], BF16), ("W2e2", [128, 128], BF16),
        ("W2o1", [128, 128], BF16), ("W2o2", [32, 128], BF16),
        ("W3E", [128, 840], BF16), ("W3O", [128, 840], BF16),
        ("WFC", [120, 6300], BF16),
        ("B2", [128, 1], F32), ("B3", [120, 1], F32),
        ("INIT1", [1, BN, 750], BF16), ("INITZ", [128, BN, 1], BF16),
    ]:
        dram[nm] = nc.dram_tensor(nm, shape, dt, kind="ExternalInput").ap()
    out_p0 = nc.dram_tensor("partial0", [100, NS], F32,
                            kind="ExternalOutput").ap()
    out_mom = nc.dram_tensor("mom", [NS, 72], F32,
                             kind="ExternalOutput").ap()

    with tile.TileContext(nc) as tc:
        with tc.tile_pool(name="wpool", bufs=1) as wp, \
             tc.tile_pool(name="covp", bufs=1) as cvp, \
             tc.tile_pool(name="scrp", bufs=4) as scp, \
             tc.tile_pool(name="xw", bufs=2) as xwp, \
             tc.tile_pool(name="s1p", bufs=1) as s1p, \
             tc.tile_pool(name="s2p", bufs=1) as s2p, \
             tc.tile_pool(name="s3p", bufs=1) as s3p, \
             tc.tile_pool(name="s4p", bufs=1) as s4p, \
             tc.tile_pool(name="tep", bufs=4) as tep, \
             tc.tile_pool(name="ps", bufs=4, space="PSUM") as psp:

            xw_tiles = {}

            def issue_xw(b):
                if b >= NBLK or b in xw_tiles:
                    return
                n0 = b * BN
                te = xwp.tile([28, BN, 750], BF16, tag="xwE")
                nc.sync.dma_start(te[:], dram["x_winE"][:, n0:n0 + BN, :])
                to = xwp.tile([28, BN, 750], BF16, tag="xwO")
                nc.sync.dma_start(to[:], dram["x_winO"][:, n0:n0 + BN, :])
                xw_tiles[b] = (te, to)

            # The SP DMA queue is FIFO and a DMA holds HWDGE ~0.6us each:
            # preload ONLY what conv0(0)/conv1(0) need, trickle the rest
            # into the pipeline steps below via dma_sched.
            momt = cvp.tile([NS, 72], F32, tag="mom")
            p0sb = cvp.tile([100, NS], F32, tag="p0sb")
            s1 = s1p.tile([121, BN, 750], BF16, tag="s1")
            s2 = s2p.tile([128, BN, 377], BF16, tag="s2")
            s3 = s3p.tile([128, BN, 380], BF16, tag="s3")
            s4 = s4p.tile([120, NS, 63], BF16, tag="s4")
            xcv = cvp.tile([NS, 3, L0], F32, tag="xcv")

            Ws = {}
            for nm in ["W1e1", "W1e2", "W1o1", "W1o2", "W0",
                       "W2e2", "W2o1", "W2o2", "W3E", "W3O", "WFC"]:
                Ws[nm] = wp.tile(list(dram[nm].shape), BF16, name=nm, tag=nm)
            Wpad = wp.tile([64, 128], BF16, name="W2e1", tag="W2e1")
            Ws["W2e1"] = Wpad[32:64]
            B2t = wp.tile([128, 1], F32, tag="B2")
            B3t = wp.tile([120, 1], F32, tag="B3")

            def dma_w(nm):
                nc.sync.dma_start(Ws[nm][:], dram[nm][:])

            # prologue: conv0(0..1) + conv1(0) prerequisites only
            dma_w("W0")
            issue_xw(0)
            issue_xw(1)
            # ones row + hl/hr edges (engines cannot address partition
            # bases off the 0/32/64/96 grid; DMA can); conv1 prereqs ride
            # the otherwise-idle Act HWDGE queue in parallel
            nc.scalar.dma_start(s1[80:81, :, :], dram["INIT1"][:])
            nc.scalar.dma_start(s1[81:101, :, 0:1], dram["INITZ"][0:20])
            nc.scalar.dma_start(s1[101:121, :, 749:750], dram["INITZ"][0:20])
            for nm in ["W1e1", "W1e2", "W1o1", "W1o2"]:
                nc.scalar.dma_start(Ws[nm][:], dram[nm][:])

            def dmas_step0():
                nc.sync.dma_start(Wpad[32:64], dram["W2e1"][:])
                for nm in ["W2e2", "W2o1", "W2o2"]:
                    dma_w(nm)
                nc.sync.dma_start(B2t[:], dram["B2"][:])
                nc.sync.dma_start(s2[:, :, 0:1], dram["INITZ"][:])
                nc.sync.dma_start(s2[:, :, 376:377], dram["INITZ"][:])

            def dmas_step1():
                dma_w("W3E")

            def dmas_step2():
                dma_w("W3O")
                nc.sync.dma_start(B3t[:], dram["B3"][:])
                nc.sync.dma_start(s3[:, :, 0:1], dram["INITZ"][:])
                for cz in range(376, 380):
                    nc.sync.dma_start(s3[:, :, cz:cz + 1], dram["INITZ"][:])

            def make_xcv_piece(i):
                c0 = i * 375
                return lambda: nc.sync.dma_start(
                    xcv[:, :, c0:c0 + 375], dram["x_f32"][:, :, c0:c0 + 375])

            def make_wfc_piece(i):
                c0 = i * 1575
                return lambda: nc.sync.dma_start(
                    Ws["WFC"][:, c0:c0 + 1575], dram["WFC"][:, c0:c0 + 1575])

            dma_sched = {0: dmas_step0, 1: dmas_step1, 2: dmas_step2}
            for i in range(16):
                dma_sched[3 + i] = make_xcv_piece(i)
            for i in range(4):
                dma_sched[20 + 2 * i] = make_wfc_piece(i)

            # covariance moment ops, chunked 8x750 so no single op can
            # head-of-line block the Act/DVE queues; 72 partial sums,
            # host adds the 8 chunks per moment.
            cov_ops = []
            for ch in range(8):
                for c in range(3):
                    cov_ops.append(("mean", c, ch))
                    cov_ops.append(("sq", c, ch))
                for k, (c, dch) in enumerate([(0, 1), (0, 2), (1, 2)]):
                    cov_ops.append(("xy", k, c, dch, ch))
            # Act gets exactly one ~1us cov op per step, DVE one per two
            # steps: never more than the pipeline's per-step engine slack.
            act_ops = [op for op in cov_ops if op[0] != "xy"]
            dve_ops = [op for op in cov_ops if op[0] == "xy"]
            cov_sched = {}
            for i, op in enumerate(act_ops):
                cov_sched.setdefault(6 + i, []).append(op)
            for i, op in enumerate(dve_ops):
                cov_sched.setdefault(6 + 2 * i, []).append(op)

            def emit_cov(op):
                ch = op[-1]
                c0 = ch * 750
                sl = slice(c0, c0 + 750)
                scr = scp.tile([NS, 750], BF16, tag="cscr")
                if op[0] == "mean":
                    c = op[1]
                    nc.scalar.activation(scr[:], xcv[:, c, sl], ACTF.Copy,
                                         accum_out=momt[:, c * 8 + ch:
                                                        c * 8 + ch + 1])
                elif op[0] == "sq":
                    c = op[1]
                    nc.scalar.activation(scr[:], xcv[:, c, sl], ACTF.Square,
                                         accum_out=momt[:, 24 + c * 8 + ch:
                                                        24 + c * 8 + ch + 1])
                else:
                    _, k, c, dch, ch = op
                    nc.vector.scalar_tensor_tensor(
                        scr[:], xcv[:, c, sl], 1.0, xcv[:, dch, sl],
                        AOP.mult, AOP.mult,
                        accum_out=momt[:, 48 + k * 8 + ch:
                                       48 + k * 8 + ch + 1])

            def conv0(p):
                blk, nb = p // 4, (p % 4) * 2
                xwE, xwO = xw_tiles[blk]
                for ch in range(2):
                    c0 = ch * 375
                    psE = psp.tile([128, 2, 512], F32, tag="ps")
                    psO = psp.tile([128, 2, 512], F32, tag="ps")
                    for i in range(2):
                        nc.tensor.matmul(psE[0:80, i, 0:375], Ws["W0"][:],
                                         xwE[:, nb + i, c0:c0 + 375],
                                         start=True, stop=True)
                    for i in range(2):
                        nc.tensor.matmul(psO[0:80, i, 0:375], Ws["W0"][:],
                                         xwO[:, nb + i, c0:c0 + 375],
                                         start=True, stop=True)
                    tE = tep.tile([128, 2, 384], BF16, tag="tE")
                    nc.scalar.activation(tE[0:80, :, 0:375],
                                         psE[0:80, :, 0:375], ACTF.Relu)
                    nc.vector.scalar_tensor_tensor(
                        s1[0:80, nb:nb + 2, c0:c0 + 375],
                        psO[0:80, :, 0:375], 0.0, tE[0:80, :, 0:375],
                        AOP.max, AOP.max)
                # per-pair halo rows for conv1
                nc.sync.dma_start(s1[81:101, nb:nb + 2, 1:750],
                                  s1[60:80, nb:nb + 2, 0:749])
                nc.sync.dma_start(s1[101:121, nb:nb + 2, 0:749],
                                  s1[0:20, nb:nb + 2, 1:750])

            def conv1(p):
                nb = (p % 4) * 2
                psE = psp.tile([128, 2, 512], F32, tag="ps")
                psO = psp.tile([128, 2, 512], F32, tag="ps")
                for i in range(2):
                    n = nb + i
                    nc.tensor.matmul(psE[0:128, i, 0:375], Ws["W1e1"][:],
                                     s1[0:101, n, 0:750:2],
                                     start=True, stop=False)
                    nc.tensor.matmul(psE[0:128, i, 0:375], Ws["W1e2"][:],
                                     s1[0:80, n, 1:750:2],
                                     start=False, stop=True)
                for i in range(2):
                    n = nb + i
                    nc.tensor.matmul(psO[0:128, i, 0:375], Ws["W1o1"][:],
                                     s1[0:80, n, 0:750:2],
                                     start=True, stop=False)
                    nc.tensor.matmul(psO[0:128, i, 0:375], Ws["W1o2"][:],
                                     s1[0:121, n, 1:750:2],
                                     start=False, stop=True)
                tE = tep.tile([128, 2, 384], BF16, tag="tE")
                nc.scalar.activation(tE[0:128, :, 0:375],
                                     psE[0:128, :, 0:375], ACTF.Relu)
                nc.vector.scalar_tensor_tensor(
                    s2[0:128, nb:nb + 2, 1:376],
                    psO[0:128, :, 0:375], 0.0, tE[0:128, :, 0:375],
                    AOP.max, AOP.max)

            def conv2(p):
                nb = (p % 4) * 2
                psE = psp.tile([128, 2, 512], F32, tag="ps")
                psO = psp.tile([128, 2, 512], F32, tag="ps")
                for i in range(2):
                    n = nb + i
                    nc.tensor.matmul(psE[0:128, i, 0:375], Ws["W2e1"],
                                     s2[32:64, n, 0:375],
                                     start=True, stop=False)
                    nc.tensor.matmul(psE[0:128, i, 0:375], Ws["W2e2"][:],
                                     s2[0:128, n, 1:376],
                                     start=False, stop=True)
                for i in range(2):
                    n = nb + i
                    nc.tensor.matmul(psO[0:128, i, 0:375], Ws["W2o1"][:],
                                     s2[0:128, n, 1:376],
                                     start=True, stop=False)
                    nc.tensor.matmul(psO[0:128, i, 0:375], Ws["W2o2"][:],
                                     s2[0:32, n, 2:377],
                                     start=False, stop=True)
                tE = tep.tile([128, 2, 384], BF16, tag="tE")
                nc.scalar.activation(tE[0:128, :, 0:375],
                                     psE[0:128, :, 0:375], ACTF.Relu,
                                     bias=B2t[:])
                nc.vector.scalar_tensor_tensor(
                    s3[0:128, nb:nb + 2, 1:376],
                    psO[0:128, :, 0:375], B2t[:], tE[0:128, :, 0:375],
                    AOP.add, AOP.max)

            def conv3(qd):
                blk, nq = qd // 2, (qd % 2) * 4
                n0 = blk * BN + nq
                psE = psp.tile([128, 2, 512], F32, tag="ps")
                psO = psp.tile([128, 2, 512], F32, tag="ps")
                for t in range(7):
                    nc.tensor.matmul(
                        psE[0:120, 0, 0:252],
                        Ws["W3E"][:, t * 120:(t + 1) * 120],
                        s3[0:128, nq:nq + 4, t:t + 373:6],
                        start=(t == 0), stop=(t == 6))
                for t in range(7):
                    nc.tensor.matmul(
                        psO[0:120, 0, 0:252],
                        Ws["W3O"][:, t * 120:(t + 1) * 120],
                        s3[0:128, nq:nq + 4, t + 1:t + 374:6],
                        start=(t == 0), stop=(t == 6))
                tE = tep.tile([128, 512], BF16, tag="tE3")
                nc.scalar.activation(tE[0:120, 0:252], psE[0:120, 0, 0:252],
                                     ACTF.Relu, bias=B3t[:])
                nc.vector.scalar_tensor_tensor(
                    s4[0:120, n0:n0 + 4, 0:63],
                    psO[0:120, 0, 0:252].rearrange("p (n l) -> p n l", n=4),
                    B3t[:],
                    tE[0:120, 0:252].rearrange("p (n l) -> p n l", n=4),
                    AOP.add, AOP.max)

            def fc(half):
                c0 = half * 64
                psfc = psp.tile([128, 2, 512], F32, tag="ps")
                for w in range(63):
                    nc.tensor.matmul(
                        psfc[0:100, 0, 0:64],
                        Ws["WFC"][:, w * 100:(w + 1) * 100],
                        s4[:, c0:c0 + 64, w], start=(w == 0), stop=(w == 62))
                nc.scalar.copy(p0sb[:, c0:c0 + 64], psfc[0:100, 0, 0:64])

            # pipeline: conv0(p) | conv1(p-1) | conv2(p-2) |
            #           conv3((p-3)/2 @ odd p) | fc halves at p=35/66
            for p in range(68):
                if p < NPAIR:
                    if p % 4 == 0:
                        issue_xw(p // 4 + 2)
                    conv0(p)
                if 0 <= p - 1 < NPAIR:
                    conv1(p - 1)
                if 0 <= p - 2 < NPAIR:
                    conv2(p - 2)
                if p % 2 == 0 and 0 <= (p - 4) // 2 < 32:
                    conv3((p - 4) // 2)
                if p == 35 or p == 67:
                    fc(0 if p == 35 else 1)
                for op in cov_sched.get(p, ()):
                    emit_cov(op)
                if p in dma_sched:
                    dma_sched[p]()

            nc.sync.dma_start(out_p0[:], p0sb[:])
            nc.sync.dma_start(out_mom[:], momt[:])

    nc.compile()
    return nc


# ------------------------------------------------------------- launch 2 ----

def _build_launch2():
    nc = bacc.Bacc("TRN2", target_bir_lowering=False, debug=False,
                   num_devices=NCORES)
    # packed inputs: one DMA per partition-width class
    dA = nc.dram_tensor("PKA", [3, 916], F32, kind="ExternalInput").ap()
    dB = nc.dram_tensor("PKB", [20, 701], F32, kind="ExternalInput").ap()
    dC = nc.dram_tensor("PKC", [100, 132], F32, kind="ExternalInput").ap()
    out2 = nc.dram_tensor("out2", [2, NS], F32, kind="ExternalOutput").ap()

    with tile.TileContext(nc) as tc:
        with tc.tile_pool(name="w2p", bufs=1) as wp, \
             tc.tile_pool(name="ps2", bufs=2, space="PSUM") as psp:
            A = wp.tile([3, 916], F32, tag="A")
            nc.sync.dma_start(A[:], dA[:])
            B = wp.tile([20, 701], F32, tag="B")
            nc.sync.dma_start(B[:], dB[:])
            C = wp.tile([100, 132], F32, tag="C")
            nc.sync.dma_start(C[:], dC[:])

            # h1 = relu(wc @ feats + bc): [20, (t, n)]
            h1 = wp.tile([20, 7 * NS], F32, tag="h1")
            for half in range(2):
                c0 = half * 448
                ps = psp.tile([32, 448], F32, tag="ph")
                nc.tensor.matmul(ps[0:20, :], A[:, 896:916],
                                 A[:, c0:c0 + 448], start=True, stop=True)
                nc.scalar.activation(h1[:, c0:c0 + 448], ps[0:20, :],
                                     ACTF.Relu, bias=B[:, 700:701])
            # z = relu(p0 + sum_t w0b_t.T @ h1_t + bl0)
            psz = psp.tile([100, NS], F32, tag="pz")
            for t in range(7):
                nc.tensor.matmul(psz[:], B[:, t * 100:(t + 1) * 100],
                                 h1[:, t * NS:(t + 1) * NS],
                                 start=(t == 0), stop=(t == 6))
            z = wp.tile([100, NS], F32, tag="z")
            nc.vector.scalar_tensor_tensor(z[:], psz[:], C[:, 128:129],
                                           C[:, 0:128], AOP.add, AOP.add)
            nc.vector.tensor_scalar_max(z[:], z[:], 0.0)
            pso = psp.tile([32, NS], F32, tag="po")
            nc.tensor.matmul(pso[0:2, :], C[:, 129:131], z[:],
                             start=True, stop=True)
            osb = wp.tile([2, NS], F32, tag="osb")
            nc.vector.tensor_scalar(osb[:], pso[0:2, :], C[0:2, 131:132],
                                    None, AOP.add)
            nc.sync.dma_start(out2[:], osb[:])

    nc.compile()
    return nc


# --------------------------------------------------------------- kernel ----

def kernel(**inputs):
    ins = {k: np.asarray(v) for k, v in inputs.items()}
    x = ins["x"].astype(np.float32)

    if "l1" not in _CACHE:
        _CACHE["l1"] = _build_launch1()
    if "l2" not in _CACHE:
        _CACHE["l2"] = _build_launch2()
    w = _prep_weights(ins)

    xbf = x.astype(BF)
    xwE = np.zeros((28, x.shape[0], 750), BF)
    xwO = np.zeros((28, x.shape[0], 750), BF)
    for c in range(3):
        for j in range(9):
            # even cols: l = 8q - 1 + j ; odd cols: l = 8q + j
            if j == 0:
                xwE[c * 9 + 0, :, 1:750] = xbf[:, c, 7:5992:8]
            else:
                xwE[c * 9 + j] = xbf[:, c, j - 1::8]
            if j == 8:
                xwO[c * 9 + 8, :, 0:749] = xbf[:, c, 8:6000:8]
            else:
                xwO[c * 9 + j] = xbf[:, c, j::8]
    xwE[27] = 1.0
    xwO[27] = 1.0
    shards = [x[i * NS:(i + 1) * NS] for i in range(NCORES)]
    in1 = []
    for i, sh in enumerate(shards):
        sl = slice(i * NS, (i + 1) * NS)
        m = {"x_winE": np.ascontiguousarray(xwE[:, sl]),
             "x_winO": np.ascontiguousarray(xwO[:, sl]),
             "x_f32": sh}
        for nm in ["W0", "W1e1", "W1e2", "W1o1", "W1o2",
                   "W2e1", "W2e2", "W2o1", "W2o2", "W3E", "W3O",
                   "WFC", "B2", "B3"]:
            m[nm] = w[nm]
        m["INIT1"] = np.ones((1, BN, 750), BF)
        m["INITZ"] = np.zeros((128, BN, 1), BF)
        in1.append(m)
    t0 = time.time()
    res1 = run_bass_kernel_spmd(_CACHE["l1"], in1, list(range(NCORES)))
    LAST_EXEC_NS[0] = res1.exec_time_ns
    LAST_WALL_S[0] = time.time() - t0

    mom72 = np.concatenate([res1.results[i]["mom"] for i in range(NCORES)],
                           0).astype(np.float32)
    mom = mom72.reshape(-1, 9, 8).sum(-1)
    partial0 = np.concatenate(
        [res1.results[i]["partial0"].T for i in range(NCORES)], 0)

    # host: cov assembly (fp32) + LAPACK-clone eigh + global normalizers
    Sx = mom[:, 0:3].astype(np.float32)
    Sxx = mom[:, 3:6].astype(np.float32)
    Sxy = mom[:, 6:9].astype(np.float32)
    L = np.float32(L0)
    cov = np.empty((x.shape[0], 3, 3), np.float32)
    for idx, (c, dch) in enumerate([(0, 1), (0, 2), (1, 2)]):
        v = (Sxy[:, idx] - Sx[:, c] * Sx[:, dch] / L) / np.float32(L0 - 1)
        cov[:, c, dch] = v
        cov[:, dch, c] = v
    for c in range(3):
        cov[:, c, c] = (Sxx[:, c] - Sx[:, c] * Sx[:, c] / L) / np.float32(
            L0 - 1)
    vals, vecs = _eigh3_batch(cov)
    covn = cov / np.abs(cov).max()
    valsn = (vals / vals.max())[..., None]
    feats = np.concatenate([covn, valsn, vecs], axis=-1).astype(np.float32)

    in2 = []
    for i in range(NCORES):
        sl = slice(i * NS, (i + 1) * NS)
        fT = feats[sl].transpose(1, 2, 0).reshape(3, 7 * NS)
        pkA = np.concatenate([fT, w["wcT"]], 1).astype(np.float32)
        pkB = np.concatenate(
            [np.concatenate([w["w0bT"][t] for t in range(7)], 1),
             w["bc"]], 1).astype(np.float32)
        bl1col = np.zeros((100, 1), np.float32)
        bl1col[0:2] = w["bl1"]
        pkC = np.concatenate(
            [partial0[sl].T, w["bl0"], w["wl1T"], bl1col],
            1).astype(np.float32)
        in2.append({"PKA": np.ascontiguousarray(pkA),
                    "PKB": np.ascontiguousarray(pkB),
                    "PKC": np.ascontiguousarray(pkC)})
    t0 = time.time()
    res2 = run_bass_kernel_spmd(_CACHE["l2"], in2, list(range(NCORES)))
    LAST_EXEC_NS[1] = res2.exec_time_ns
    LAST_WALL_S[1] = time.time() - t0

    out = np.concatenate([res2.results[i]["out2"].T for i in range(NCORES)],
                         0).astype(np.float32)
    return (out[:, 0:1], out[:, 1:2])
